# revision 1
# baseline (speedup 1.0000x reference)
"""Trainium2 Bass kernel for CustomLossWithCovariance.

loss = abs(logdet(sigma) + mean_b[(p_b - t_b)^T sigma^{-1} (p_b - t_b)])

Only the 3x3 Gram matrix G = sum_b d_b d_b^T (d = pred - targ) requires
touching the [B, 3] data; the device computes per-core partial pair-sums
of G, and the host finishes with the tiny 3x3 algebra:
    mean_mahalanobis = <sigma_inv, G> / B
    loss = |logdet(sigma) + mean_mahalanobis|

Sharding: data-parallel over the batch across 8 NeuronCores (each core
streams a contiguous [B/8, 3] shard; partial sums gathered on host).

Production path: build_gram_kernel_v4 (raw Bacc, manual semaphores).
Per tile: one dma_start brings pred|targ halves; DVE subtracts into a
row-interleaved bf16 d buffer (unit-stride write — scatter writes are
4.6x slower on DVE); DVE fused multiply-reduces (stride-3 component
reads, grouped across tiles) produce the cross sums and ACT Square
accumulate produces the diagonals. Tile sizes ramp up at the head (so
DVE starts ~5us earlier) and shrink at the tail (so the post-last-DMA
dependency chain is short). The fp32 ring slot is freed by the sub
alone, letting the DMA stream run n_bufs tiles ahead. Bass's two
__init__ all-engine barriers are stripped (saves ~1.5us; the one real
dependency — gpsimd const memsets before ACT bias reads — is re-fenced
with boot_sem). Accumulator tensors are pre-zeroed so any flush race
degrades to ~1e-4 error instead of garbage, and kernel() additionally
sanity-gates the device result against a host subsample estimate with
retry + exact-host fallback.

Older variants (build_gram_kernel, build_gram_kernel_raw,
build_gram_kernel_v3) are kept for reference only.
"""

import numpy as np

import concourse.bass as bass
import concourse.bacc as bacc
import concourse.mybir as mybir
from concourse import tile
from concourse.bass_utils import run_bass_kernel_spmd

N_CORES = 8
B_FULL = 8388608
P = 128

_PAIRS = [(0, 1), (0, 2), (1, 2)]


def build_gram_kernel(n_rows: int, n_tiles: int, use_act: bool = True):
    """Build the per-core Bass module.

    Input: pt [2, n_rows, 3] f32 (pred stacked with targ)
    Output: partials [128, 6 * n_tiles] f32
        col t*3+i            : sum over this tile/partition of d_i^2
        col 3*n_tiles + t*3+k: sum of d_i*d_j for pair k in _PAIRS
    """
    assert n_rows % (P * n_tiles) == 0
    r = n_rows // (P * n_tiles)  # rows per partition per tile
    m = 3 * r                    # flat f32 elements per partition per tile
    f32 = mybir.dt.float32

    # Bacc (not plain Bass): its compile() pass legalizes semaphore waits
    # (each TRN2 instruction holds at most one wait slot).
    nc = bacc.Bacc("TRN2", target_bir_lowering=False, debug=False)
    pt = nc.dram_tensor("pt", [2, n_rows, 3], f32, kind="ExternalInput")
    out = nc.dram_tensor("partials", [P, 6 * n_tiles], f32, kind="ExternalOutput")

    # [t][p][w(2), m] — per tile/partition: pred chunk and targ chunk, each
    # m contiguous f32 in DRAM.
    pt_v = pt[:].rearrange("w (t p r) c -> t p w (r c)", t=n_tiles, p=P)

    with tile.TileContext(nc) as tc:
        with (
            tc.tile_pool(name="io", bufs=3) as io_pool,
            tc.tile_pool(name="dve_scr", bufs=2) as dve_scr,
            tc.tile_pool(name="act_scr", bufs=2) as act_scr,
            tc.tile_pool(name="acc", bufs=1) as acc_pool,
        ):
            acc_sq = acc_pool.tile([P, 3 * n_tiles], f32)
            acc_cr = acc_pool.tile([P, 3 * n_tiles], f32)

            for t in range(n_tiles):
                buf = io_pool.tile([P, 2 * m], f32, tag="buf")
                nc.sync.dma_start(
                    out=buf[:].rearrange("p (w m) -> p w m", w=2),
                    in_=pt_v[t],
                )

                # In-place: d = pred - targ, overwriting the pred half.
                nc.vector.tensor_tensor(
                    out=buf[:, 0:m],
                    in0=buf[:, 0:m],
                    in1=buf[:, m : 2 * m],
                    op=mybir.AluOpType.subtract,
                )
                d3 = buf[:, 0:m].rearrange("p (r c) -> p c r", c=3)

                # Diagonal sums on the scalar engine (Square + accum_out),
                # overlapping with the DVE cross-products.
                if use_act:
                    for i in range(3):
                        sq = act_scr.tile([P, r], f32, tag="sq")
                        nc.scalar.activation(
                            out=sq[:],
                            in_=d3[:, i, :],
                            func=mybir.ActivationFunctionType.Square,
                            accum_out=acc_sq[:, t * 3 + i : t * 3 + i + 1],
                        )
                else:
                    for i in range(3):
                        sq = dve_scr.tile([P, r], f32, tag="pr")
                        nc.vector.scalar_tensor_tensor(
                            out=sq[:],
                            in0=d3[:, i, :],
                            scalar=1.0,
                            in1=d3[:, i, :],
                            op0=mybir.AluOpType.mult,
                            op1=mybir.AluOpType.mult,
                            accum_out=acc_sq[:, t * 3 + i : t * 3 + i + 1],
                        )
                # Cross sums: fused multiply+reduce on DVE
                # (scalar_tensor_tensor: out = (in0 * 1.0) * in1, accum = sum).
                for k, (i, j) in enumerate(_PAIRS):
                    pr = dve_scr.tile([P, r], f32, tag="pr")
                    nc.vector.scalar_tensor_tensor(
                        out=pr[:],
                        in0=d3[:, i, :],
                        scalar=1.0,
                        in1=d3[:, j, :],
                        op0=mybir.AluOpType.mult,
                        op1=mybir.AluOpType.mult,
                        accum_out=acc_cr[:, t * 3 + k : t * 3 + k + 1],
                    )

            nc.sync.dma_start(out=out[:, 0 : 3 * n_tiles], in_=acc_sq[:])
            nc.sync.dma_start(out=out[:, 3 * n_tiles : 6 * n_tiles], in_=acc_cr[:])

    nc.compile()
    return nc


def build_gram_kernel_raw(n_rows: int, n_tiles: int = 32, n_bufs: int = 24,
                          group: int = 4, skip_exit_barrier: bool = True):
    """Raw-Bacc variant: manual semaphores, no TileContext.

    Skips Tile's prologue/epilogue (drain + two all-engine EVSEM
    barriers, ~16 us) — the only sync needed is a three-semaphore chain:
    DMA loads (one HWDGE ring) -> DVE -> ACT.

    The ring of tile buffers lives in ONE SBUF tensor so the fused
    multiply-reduces can span `group` consecutive tiles with a single
    instruction (free-dim AP [group, r]) — amortizing the per-op fixed
    cost and the accumulator-drain, which keeps both compute engines
    well under the DMA pace.

    Input: pt [2, n_rows, 3] f32. Output: partials [128, 6 * n_groups]
    (same slot layout as build_gram_kernel, with n_groups slots).
    """
    assert n_tiles % group == 0 and n_bufs % group == 0
    assert n_rows % (P * n_tiles) == 0
    n_groups = n_tiles // group
    r = n_rows // (P * n_tiles)
    m = 3 * r
    f32 = mybir.dt.float32

    nc = bacc.Bacc("TRN2", target_bir_lowering=False, debug=False)
    pt = nc.dram_tensor("pt", [2, n_rows, 3], f32, kind="ExternalInput")
    out = nc.dram_tensor("partials", [P, 6 * n_groups], f32, kind="ExternalOutput")
    pt_v = pt[:].rearrange("w (t p r) c -> t p w (r c)", t=n_tiles, p=P)

    ring = nc.alloc_sbuf_tensor("ring", [P, n_bufs * 2 * m], f32).ap()

    def buf(t):
        s = t % n_bufs
        return ring[:, s * 2 * m : (s + 1) * 2 * m]

    def dgroup(g, i):
        # component i of the diff halves of tiles 4g..4g+3: [128, group, r]
        s0 = (g * group) % n_bufs
        w = ring[:, s0 * 2 * m : (s0 + group) * 2 * m]
        return w.rearrange("p (t w r c) -> p t w c r", t=group, w=2, c=3)[:, :, 0, i, :]

    acc_sq = nc.alloc_sbuf_tensor("acc_sq", [P, 3 * n_groups], f32).ap()
    acc_cr = nc.alloc_sbuf_tensor("acc_cr", [P, 3 * n_groups], f32).ap()
    # Rotated scratch (dead stores of the fused ops), 2 groups deep so each
    # group's single stale semaphore wait also covers the scratch WAW from
    # two groups back.
    pr_scrs = [
        nc.alloc_sbuf_tensor(f"pr_scr{k}", [P, group * r], f32).ap() for k in range(6)
    ]
    sq_scrs = [
        nc.alloc_sbuf_tensor(f"sq_scr{k}", [P, group * r], f32).ap() for k in range(6)
    ]

    # One DMA-completion semaphore per ring buffer: a single shared sem
    # would be unsound — each dma_start is split across 16 SDMA engines
    # whose sub-completions interleave across in-flight DMAs.
    dma_sems = [nc.alloc_semaphore(f"dma_sem{i}") for i in range(n_bufs)]
    out_sem = nc.alloc_semaphore("out_sem")
    dve_sem = nc.alloc_semaphore("dve_sem")
    act_sem = nc.alloc_semaphore("act_sem")

    # DVE emission order: subs run ahead; the grouped multiply-reduces for
    # group g are emitted after sub(4g+4) so their drain-wait on the last
    # sub of the group is already satisfied when it executes (DVE writes
    # drain asynchronously). Only the last group trails the final sub.
    dve_order = []
    for t in range(n_tiles):
        dve_order.append(("sub", t))
        if t % group == 0 and t >= group:
            # one sub of stagger after the group's last sub
            dve_order.append(("stt", t // group - 1))
    dve_order.append(("stt", n_groups - 1))
    sub_done, sttg_done = {}, {}
    v = 0
    for kind, x in dve_order:
        if kind == "sub":
            v += 1
            sub_done[x] = v
        else:
            v += 3
            sttg_done[x] = v

    # Output chunks: flush finished accumulator columns while later tiles
    # still stream, so the tail only waits on the last small chunk.
    chunk = max(1, n_groups // 2)
    chunks = [(c, min(c + chunk, n_groups)) for c in range(0, n_groups, chunk)]

    import contextlib

    @contextlib.contextmanager
    def _block():
        # no_gpsimd_drain=True emits per-engine drains explicitly and then a
        # sem-only all-engine butterfly. The butterfly only delays NEFF end
        # (outputs are already fenced by the sequencer's out_sem wait), so
        # optionally no-op it during Block.__exit__.
        with nc.Block(no_gpsimd_drain=True) as blk:
            try:
                yield blk
            finally:
                if skip_exit_barrier:
                    nc.all_engine_barrier = lambda **kw: None
        if skip_exit_barrier:
            del nc.all_engine_barrier  # restore class method

    with _block() as block:

        @block.sync
        def _(sync):
            for t in range(n_tiles):
                if head_dma_on_vector and t < h0:
                    continue  # issued from the vector queue (clears the
                              # NEFF entry barrier ~1.4us before sync)
                if t >= n_bufs:
                    # ring reuse: all consumers of the buffer's previous
                    # occupant (tile t - n_bufs) must be done
                    prev = t - n_bufs
                    sync.wait_ge(dve_sem, sttg_done[prev // group])
                    sync.wait_ge(act_sem, 3 * (prev // group + 1))
                sync.dma_start(
                    out=buf(t).rearrange("p (w m) -> p w m", w=2),
                    in_=pt_v[t],
                ).then_inc(dma_sems[t % n_bufs], 16)
            n_out = 0
            for lo, hi in chunks:
                sync.wait_ge(act_sem, 3 * hi)
                sync.dma_start(
                    out=out[:, 3 * lo : 3 * hi], in_=acc_sq[:, 3 * lo : 3 * hi]
                ).then_inc(out_sem, 16)
                sync.wait_ge(dve_sem, sttg_done[hi - 1])
                sync.dma_start(
                    out=out[:, 3 * (n_groups + lo) : 3 * (n_groups + hi)],
                    in_=acc_cr[:, 3 * lo : 3 * hi],
                ).then_inc(out_sem, 16)
                n_out += 32
            sync.wait_ge(out_sem, n_out)

        @block.vector
        def _(vector):
            for kind, x in dve_order:
                if kind == "sub":
                    b = buf(x)
                    vector.wait_ge(dma_sems[x % n_bufs], 16 * (x // n_bufs + 1))
                    vector.tensor_tensor(
                        out=b[:, 0:m],
                        in0=b[:, 0:m],
                        in1=b[:, m : 2 * m],
                        op=mybir.AluOpType.subtract,
                    ).then_inc(dve_sem, 1)
                else:
                    vector.wait_ge(dve_sem, sub_done[(x + 1) * group - 1])
                    for k, (i, j) in enumerate(_PAIRS):
                        vector.scalar_tensor_tensor(
                            out=pr_scrs[(x % 2) * 3 + k][:].rearrange(
                                "p (t r) -> p t r", t=group
                            ),
                            in0=dgroup(x, i),
                            scalar=1.0,
                            in1=dgroup(x, j),
                            op0=mybir.AluOpType.mult,
                            op1=mybir.AluOpType.mult,
                            accum_out=acc_cr[:, x * 3 + k : x * 3 + k + 1],
                        ).then_inc(dve_sem, 1)

        @block.scalar
        def _(scalar):
            for g in range(n_groups):
                scalar.wait_ge(dve_sem, sub_done[(g + 1) * group - 1])
                if g >= 2:
                    # scratch slot reuse from two groups back
                    scalar.wait_ge(act_sem, 3 * (g - 1))
                for i in range(3):
                    scalar.activation(
                        out=sq_scrs[(g % 2) * 3 + i][:].rearrange(
                            "p (t r) -> p t r", t=group
                        ),
                        in_=dgroup(g, i),
                        func=mybir.ActivationFunctionType.Square,
                        accum_out=acc_sq[:, g * 3 + i : g * 3 + i + 1],
                    ).then_inc(act_sem, 1)

    nc.compile()
    return nc

def _strip_entry_barriers(nc):
    """Remove the two all-engine entry barriers Bass.__init__ emits.

    They serialize ~4us of semaphore round-trips before the first DMA can
    issue. The only cross-engine ordering they provide that this kernel
    needs is gpsimd-const-AP-memset -> ACT-bias-read, which is re-fenced
    explicitly with boot_sem in build_gram_kernel_v3.
    """
    bar = set(nc.barrier_sems)
    blk = nc.main_func.blocks[0]
    drop = []
    for ins in blk.instructions:
        si = getattr(ins, "sync_info", None)
        if si is None:
            continue
        sems = {w.id for w in si.on_wait or []}
        sems |= {u.id for u in si.on_update or []}
        if sems & bar:
            drop.append(ins)
    for ins in drop:
        blk.instructions.remove(ins)
    return len(drop)


def build_gram_kernel_v3(n_rows: int, n_tiles: int = 16, n_bufs: int = 8,
                         group: int = 4, strip_barriers: bool = True,
                         skip_exit_barrier: bool = True):
    """v3: planar-bf16 d + 2x DVE reduces + ACT squares.

    Per tile: DMA both halves -> DVE sub (fp32 in, planar bf16 out:
    component planes x|y|z so reduce operands are unit-stride 2-byte,
    unlocking the DVE 2x perf mode) -> DVE cross-product reduces (grouped
    `group` tiles per instr) + ACT Square reduces (grouped; per-tile for
    the last group so the post-DMA tail stays short).

    The fp32 ring slot is freed by the sub alone (d lives in its own
    full-size buffer), so the DMA stream runs ~n_bufs tiles ahead of
    compute and never stalls on the reduce bursts.

    Output layout [128, 3*n_groups + 3*(n_groups-1) + 3*group]:
      cols 0 .. 3*n_groups-1: cross sums (group g, pair k at 3g+k)
      then squares: full groups 0..n_groups-2 (3 each), then the last
      group's tiles individually (3 each).
    """
    assert n_rows % (P * n_tiles) == 0 and n_tiles % group == 0
    r = n_rows // (P * n_tiles)
    m = 3 * r
    n_groups = n_tiles // group
    full_sq = n_groups - 1           # square-groups emitted grouped
    tail0 = full_sq * group          # first per-tile-squares tile
    ncr = 3 * n_groups
    nsq = 3 * full_sq + 3 * group
    f32, bf16 = mybir.dt.float32, mybir.dt.bfloat16

    nc = bacc.Bacc("TRN2", target_bir_lowering=False, debug=False)
    if strip_barriers:
        _strip_entry_barriers(nc)
    pt = nc.dram_tensor("pt", [2, n_rows, 3], f32, kind="ExternalInput")
    out = nc.dram_tensor("partials", [P, ncr + nsq], f32, kind="ExternalOutput")
    pt_v = pt[:].rearrange("w (t p r) c -> t p w (r c)", t=n_tiles, p=P)

    ring = nc.alloc_sbuf_tensor("ring", [P, n_bufs * 2 * m], f32).ap()
    d_all = nc.alloc_sbuf_tensor("d_all", [P, n_tiles * m], bf16).ap()
    d_t = d_all.rearrange("p (t c r) -> p t c r", t=n_tiles, c=3)
    acc_cr = nc.alloc_sbuf_tensor("acc_cr", [P, ncr], f32).ap()
    acc_sq = nc.alloc_sbuf_tensor("acc_sq", [P, nsq], f32).ap()
    # Dead stores of the fused reduces; single slot per engine (each
    # engine executes its own stream in order, so WAW is safe).
    cr_scr = nc.alloc_sbuf_tensor("cr_scr", [P, group * r], bf16).ap()
    sq_scr = nc.alloc_sbuf_tensor("sq_scr", [P, group * r], bf16).ap()

    dma_sems = [nc.alloc_semaphore(f"dma{i}") for i in range(n_bufs)]
    sub_sem = nc.alloc_semaphore("sub_sem")
    red_sem = nc.alloc_semaphore("red_sem")
    act_sem = nc.alloc_semaphore("act_sem")
    out_sem = nc.alloc_semaphore("out_sem")
    boot_sem = nc.alloc_semaphore("boot_sem")

    def dcomp(t0, nt, i):
        # component i of tiles t0..t0+nt-1: [128, nt, r] unit-stride bf16
        v = d_t[:, t0 : t0 + nt, i, :]
        return v

    import contextlib

    @contextlib.contextmanager
    def _block():
        with nc.Block(no_gpsimd_drain=True) as blk:
            try:
                yield blk
            finally:
                if skip_exit_barrier:
                    nc.all_engine_barrier = lambda **kw: None
        if skip_exit_barrier:
            del nc.all_engine_barrier  # restore class method

    with _block() as block:

        @block.gpsimd
        def _(gpsimd):
            # Const-AP memsets (ACT bias) are earlier in gpsimd's stream;
            # this inc publishes their completion to the scalar queue.
            gpsimd.sem_inc(boot_sem, 1)

        @block.sync
        def _(sync):
            for t in range(n_tiles):
                if head_dma_on_vector and t < h0:
                    continue  # issued from the vector queue (clears the
                              # NEFF entry barrier ~1.4us before sync)
                if t >= n_bufs:
                    # ring slot free once its previous occupant was subbed
                    sync.wait_ge(sub_sem, t - n_bufs + 1)
                sync.dma_start(
                    out=ring[:, (t % n_bufs) * 2 * m : (t % n_bufs + 1) * 2 * m]
                    .rearrange("p (w m) -> p w m", w=2),
                    in_=pt_v[t],
                ).then_inc(dma_sems[t % n_bufs], 16)
            # accumulator flush: big chunks early, last-group slivers at end
            sync.wait_ge(red_sem, 3 * (n_groups - 1))
            sync.dma_start(
                out=out[:, 0 : 3 * (n_groups - 1)],
                in_=acc_cr[:, 0 : 3 * (n_groups - 1)],
            ).then_inc(out_sem, 16)
            sync.wait_ge(act_sem, 3 * full_sq)
            sync.dma_start(
                out=out[:, ncr : ncr + 3 * full_sq],
                in_=acc_sq[:, 0 : 3 * full_sq],
            ).then_inc(out_sem, 16)
            sync.wait_ge(red_sem, 3 * n_groups)
            sync.dma_start(
                out=out[:, 3 * (n_groups - 1) : ncr],
                in_=acc_cr[:, 3 * (n_groups - 1) : ncr],
            ).then_inc(out_sem, 16)
            sync.wait_ge(act_sem, nsq)
            sync.dma_start(
                out=out[:, ncr + 3 * full_sq : ncr + nsq],
                in_=acc_sq[:, 3 * full_sq : nsq],
            ).then_inc(out_sem, 16)
            sync.wait_ge(out_sem, 64)

        @block.vector
        def _(vector):
            for t in range(n_tiles):
                s = t % n_bufs
                buf = ring[:, s * 2 * m : (s + 1) * 2 * m]
                vector.wait_ge(dma_sems[s], 16 * (t // n_bufs + 1))
                # d = pred - targ, downcast to bf16, scattered into
                # component planes (write AP [r, 3] w/ strides [1, r])
                vector.tensor_tensor(
                    out=d_all[:, t * m : (t + 1) * m].rearrange(
                        "p (c r) -> p r c", c=3
                    ),
                    in0=buf[:, 0:m],
                    in1=buf[:, m : 2 * m],
                    op=mybir.AluOpType.subtract,
                ).then_inc(sub_sem, 1)
                if t % group == group - 1:
                    g = t // group
                    for k, (i, j) in enumerate(_PAIRS):
                        vector.scalar_tensor_tensor(
                            out=cr_scr[:].rearrange("p (t r) -> p t r", t=group),
                            in0=dcomp(g * group, group, i),
                            scalar=1.0,
                            in1=dcomp(g * group, group, j),
                            op0=mybir.AluOpType.mult,
                            op1=mybir.AluOpType.mult,
                            accum_out=acc_cr[:, g * 3 + k : g * 3 + k + 1],
                        ).then_inc(red_sem, 1)

        @block.scalar
        def _(scalar):
            scalar.wait_ge(boot_sem, 1)
            for g in range(full_sq):
                scalar.wait_ge(sub_sem, group * (g + 1))
                for i in range(3):
                    scalar.activation(
                        out=sq_scr[:].rearrange("p (t r) -> p t r", t=group),
                        in_=dcomp(g * group, group, i),
                        func=mybir.ActivationFunctionType.Square,
                        accum_out=acc_sq[:, g * 3 + i : g * 3 + i + 1],
                    ).then_inc(act_sem, 1)
            for w, t in enumerate(range(tail0, n_tiles)):
                scalar.wait_ge(sub_sem, t + 1)
                for i in range(3):
                    c = 3 * full_sq + w * 3 + i
                    scalar.activation(
                        out=sq_scr[:, 0:r],
                        in_=dcomp(t, 1, i),
                        func=mybir.ActivationFunctionType.Square,
                        accum_out=acc_sq[:, c : c + 1],
                    ).then_inc(act_sem, 1)

    nc.compile()
    nc._v3_meta = (n_tiles, group)
    return nc


def build_gram_kernel_v4(n_rows: int, bulk_r: int = 512, n_bufs: int = 12,
                         group: int = 4, gp_stride: int = 0,
                         head_rs: tuple = (128, 256, 256, 384),
                         tail_rs: tuple = (256, 128, 64, 32, 32),
                         dpad: int = 3, head_dma_on_vector: bool = True,
                         strip_barriers: bool = True,
                         skip_exit_barrier: bool = True):
    """v4: interleaved-bf16 d, measured-cost engine mix, shrinking tail.

    Measured HW rates (ns per 128-wide column): DVE sub fp32->bf16 unit
    1.28; DVE stt reduce bf16 stride-3 1.32; ACT Square ~1.0-1.4 + 740
    fixed; GPSIMD sub ~3.5. Writes must be unit-stride (scatter = 4.6x);
    strided reads are cheap. So d stays row-interleaved bf16.

    - bulk tiles of r=bulk_r rows/partition; every gp_stride-th bulk tile's
      sub runs on GPSIMD to keep DVE under the DMA pace.
    - cross-products: DVE stt grouped over `group` consecutive bulk tiles.
    - squares: ACT, same grouping; tail tiles per-tile; r<=32 tails on DVE.
    - tail tiles shrink so the post-last-DMA dependency chain is tiny.
    """
    R = n_rows // P
    assert n_rows % P == 0
    bulk_n = (R - sum(head_rs) - sum(tail_rs)) // bulk_r
    assert sum(head_rs) + bulk_n * bulk_r + sum(tail_rs) == R
    rs = list(head_rs) + [bulk_r] * bulk_n + list(tail_rs)
    n_tiles = len(rs)
    h0 = len(head_rs)               # first bulk tile index
    t0_tail = h0 + bulk_n           # first tail tile index
    cum = [0]
    for r in rs:
        cum.append(cum[-1] + r)
    # bulk groups: chunks of `group` (absolute tile indices)
    groups = [list(range(s, min(s + group, t0_tail)))
              for s in range(h0, t0_tail, group)]
    group_last = {g[-1]: g for g in groups}
    # per-tile (ungrouped) reduce tiles: head + tail
    per_tile = set(range(0, h0)) | set(range(t0_tail, n_tiles))
    # every gp_stride-th bulk tile's sub runs on GPSIMD (0 = none)
    gp_tiles = (set(range(h0, t0_tail, gp_stride)) if gp_stride else set())
    f32, bf16 = mybir.dt.float32, mybir.dt.bfloat16

    nc = bacc.Bacc("TRN2", target_bir_lowering=False, debug=False)
    if strip_barriers:
        _strip_entry_barriers(nc)
    pt = nc.dram_tensor("pt", [2, n_rows, 3], f32, kind="ExternalInput")

    # per-tile engine assignment of the sub + cumulative sem targets
    dve_idx, gp_idx = {}, {}
    for t in range(n_tiles):
        if t in gp_tiles:
            gp_idx[t] = len(gp_idx)
        else:
            dve_idx[t] = len(dve_idx)

    def sub_waits(last_t):
        """(sub_sem target, gsub_sem target) covering tiles 0..last_t."""
        d = sum(1 for t, i in dve_idx.items() if t <= last_t)
        g = sum(1 for t, i in gp_idx.items() if t <= last_t)
        return d, g

    # reduce slot counts (order finalized at emission)
    n_dve = 3 * len(groups) + 3 * len(per_tile) + 3 * sum(
        1 for t in per_tile if rs[t] <= 32)
    plan_act = []
    for t in sorted(per_tile):
        if t < h0 and rs[t] > 32:
            for i in range(3):
                plan_act.append((i, i))
    for g_tiles in groups:
        for i in range(3):
            plan_act.append((i, i))
    for t in sorted(per_tile):
        if t >= t0_tail and rs[t] > 32:
            for i in range(3):
                plan_act.append((i, i))
    n_act = len(plan_act)
    out = nc.dram_tensor("partials", [P, n_dve + n_act], f32,
                         kind="ExternalOutput")

    m_bulk = 3 * bulk_r
    ring = nc.alloc_sbuf_tensor("ring", [P, n_bufs * 2 * m_bulk], f32).ap()
    d_all = nc.alloc_sbuf_tensor("d_all", [P, dpad * R], bf16).ap()
    acc_dve = nc.alloc_sbuf_tensor("acc_dve", [P, n_dve], f32).ap()
    acc_act = nc.alloc_sbuf_tensor("acc_act", [P, n_act], f32).ap()
    cr_scr = nc.alloc_sbuf_tensor("cr_scr", [P, group * bulk_r], bf16).ap()
    sq_scr = nc.alloc_sbuf_tensor("sq_scr", [P, group * bulk_r], bf16).ap()

    dma_sems = [nc.alloc_semaphore(f"dma{i}") for i in range(n_bufs)]
    sub_sem = nc.alloc_semaphore("sub_sem")
    gsub_sem = nc.alloc_semaphore("gsub_sem")
    red_sem = nc.alloc_semaphore("red_sem")
    act_sem = nc.alloc_semaphore("act_sem")
    out_sem = nc.alloc_semaphore("out_sem")
    boot_sem = nc.alloc_semaphore("boot_sem")

    def ring_slot(t):
        s = t % n_bufs
        return ring[:, s * 2 * m_bulk : s * 2 * m_bulk + 2 * 3 * rs[t]]

    def pt_tile(t):
        sl = pt[:, cum[t] * P : cum[t + 1] * P, :]
        return sl.rearrange("w (p r) c -> p w (r c)", p=P)

    def dseg(t0, nt, i):
        # component i of tiles t0..t0+nt-1 (equal r), stride-dpad reads
        v = d_all[:, dpad * cum[t0] : dpad * cum[t0 + nt]]
        return v.rearrange("p (t r c) -> p t c r", t=nt, c=dpad)[:, :, i, :]

    def dsub_out(t):
        # write view for the sub: rows of 3 packed comps, dpad-elem row pitch
        v = d_all[:, dpad * cum[t] : dpad * cum[t + 1]]
        if dpad == 3:
            return v
        return v.rearrange("p (r c) -> p r c", c=dpad)[:, :, 0:3]

    # early-flush boundaries: head reduces + all-but-last bulk group are
    # guaranteed emitted before any tail reduce; ACT head+bulk likewise
    red_early = 3 * h0 + 3 * (len(groups) - 1)
    act_bulk = 3 * h0 + 3 * len(groups)
    _red_pen = max(red_early, n_dve - 6)
    _act_pen = max(act_bulk, n_act - 3)
    n_flush_dve = sum(1 for lo, hi in ((0, red_early), (red_early, _red_pen),
                                       (_red_pen, n_dve)) if hi > lo)
    act_chunks = [(act_bulk, 0, act_bulk), (_act_pen, act_bulk, _act_pen),
                  (n_act, _act_pen, n_act)]
    act_chunks = [c for c in act_chunks if c[2] > c[1]]
    n_flush_act = len(act_chunks)

    import contextlib

    @contextlib.contextmanager
    def _block():
        with nc.Block(no_gpsimd_drain=True) as blk:
            try:
                yield blk
            finally:
                if skip_exit_barrier:
                    nc.all_engine_barrier = lambda **kw: None
        if skip_exit_barrier:
            del nc.all_engine_barrier

    with _block() as block:

        @block.sync
        def _(sync):
            for t in range(n_tiles):
                if head_dma_on_vector and t < h0:
                    continue  # issued from the vector queue (clears the
                              # NEFF entry barrier ~1.4us before sync)
                if t >= n_bufs:
                    u = t - n_bufs
                    if u in gp_idx:
                        sync.wait_ge(gsub_sem, gp_idx[u] + 1)
                    else:
                        sync.wait_ge(sub_sem, dve_idx[u] + 1)
                sync.dma_start(
                    out=ring_slot(t).rearrange("p (w m) -> p w m", w=2),
                    in_=pt_tile(t),
                ).then_inc(dma_sems[t % n_bufs], 16)
            # acc_dve flushes stay here; acc_act flushes issue from the
            # scalar queue (ACT finishes ~2.5us before DVE, so they then
            # complete during the DVE tail instead of serializing after it)
            red_pen = max(red_early, n_dve - 6)   # all but last tail tile
            for val, lo, hi in ((red_early, 0, red_early),
                                (red_pen, red_early, red_pen),
                                (n_dve, red_pen, n_dve)):
                if hi <= lo:
                    continue
                sync.wait_ge(red_sem, val)
                sync.dma_start(out=out[:, lo:hi],
                               in_=acc_dve[:, lo:hi]).then_inc(out_sem, 16)
            sync.wait_ge(out_sem, 16 * (n_flush_dve + n_flush_act))

        @block.vector
        def _(vector):
            red_c = 0
            plan_dyn = []
            pending = []  # deferred bulk-group reduces, drained 1-per-sub

            def emit_one(t0, nt, i, j):
                nonlocal red_c
                r = rs[t0]
                vector.scalar_tensor_tensor(
                    out=cr_scr[:, 0 : nt * r].rearrange(
                        "p (t r) -> p t r", t=nt),
                    in0=dseg(t0, nt, i),
                    scalar=1.0,
                    in1=dseg(t0, nt, j),
                    op0=mybir.AluOpType.mult,
                    op1=mybir.AluOpType.mult,
                    accum_out=acc_dve[:, red_c : red_c + 1],
                ).then_inc(red_sem, 1)
                plan_dyn.append((i, j))
                red_c += 1

            for t in range(n_tiles):
                if t not in gp_tiles:
                    buf = ring_slot(t)
                    mt = 3 * rs[t]
                    vector.wait_ge(dma_sems[t % n_bufs], 16 * (t // n_bufs + 1))
                    vector.tensor_tensor(
                        out=dsub_out(t),
                        in0=buf[:, 0:mt],
                        in1=buf[:, mt : 2 * mt],
                        op=mybir.AluOpType.subtract,
                    ).then_inc(sub_sem, 1)
                    # spread deferred group reduces between subs so the DMA
                    # ring (freed by subs) never stalls on a reduce burst
                    if pending:
                        emit_one(*pending.pop(0))
                if t in group_last:
                    g_tiles = group_last[t]
                    _, gw = sub_waits(g_tiles[-1])
                    if gw:
                        vector.wait_ge(gsub_sem, gw)
                    for (i, j) in _PAIRS:
                        pending.append((g_tiles[0], len(g_tiles), i, j))
                elif t in per_tile:
                    for (i, j) in _PAIRS:
                        emit_one(t, 1, i, j)
                    if rs[t] <= 32:
                        for i in range(3):
                            emit_one(t, 1, i, i)
            while pending:
                emit_one(*pending.pop(0))
            nc._v4_plan_dve = plan_dyn

        @block.scalar
        def _(scalar):
            if head_dma_on_vector:
                # Scalar clears the NEFF entry-barrier chain ~2us before
                # Sync; issuing the head-tile loads here starts the stream
                # (and so the first sub) earlier. Ring slots are fresh, no
                # waits needed; the const-AP fence only matters for the
                # activations below.
                for t in range(h0):
                    scalar.dma_start(
                        out=ring_slot(t).rearrange("p (w m) -> p w m", w=2),
                        in_=pt_tile(t),
                    ).then_inc(dma_sems[t % n_bufs], 16)
            scalar.wait_ge(boot_sem, 1)
            act_c = 0

            def emit_squares(t0, nt):
                nonlocal act_c
                r = rs[t0]
                for i in range(3):
                    scalar.activation(
                        out=sq_scr[:, 0 : nt * r].rearrange(
                            "p (t r) -> p t r", t=nt),
                        in_=dseg(t0, nt, i),
                        func=mybir.ActivationFunctionType.Square,
                        accum_out=acc_act[:, act_c : act_c + 1],
                    ).then_inc(act_sem, 1)
                    act_c += 1

            def sq_waits(last_t):
                d, g = sub_waits(last_t)
                scalar.wait_ge(sub_sem, d)
                if g:
                    scalar.wait_ge(gsub_sem, g)

            for t in sorted(per_tile):
                if t < h0 and rs[t] > 32:
                    sq_waits(t)
                    emit_squares(t, 1)
            for g_tiles in groups:
                sq_waits(g_tiles[-1])
                emit_squares(g_tiles[0], len(g_tiles))
            for t in sorted(per_tile):
                if t >= t0_tail and rs[t] > 32:
                    sq_waits(t)
                    emit_squares(t, 1)
            for val, lo, hi in act_chunks:
                scalar.wait_ge(act_sem, val)
                scalar.dma_start(out=out[:, n_dve + lo : n_dve + hi],
                                 in_=acc_act[:, lo:hi]).then_inc(out_sem, 16)

        @block.gpsimd
        def _(gpsimd):
            # Zero the accumulator tensors so a (never-observed, but cheap
            # to insure against) flush-before-drain race reads zeros - a
            # ~1e-4 relative error - instead of stale SBUF garbage.
            gpsimd.memset(acc_dve, 0.0)
            gpsimd.memset(acc_act, 0.0)
            gpsimd.sem_inc(boot_sem, 1)
            for t in sorted(gp_tiles):
                buf = ring_slot(t)
                mt = 3 * rs[t]
                gpsimd.wait_ge(dma_sems[t % n_bufs], 16 * (t // n_bufs + 1))
                gpsimd.tensor_tensor(
                    out=dsub_out(t),
                    in0=buf[:, 0:mt],
                    in1=buf[:, mt : 2 * mt],
                    op=mybir.AluOpType.subtract,
                ).then_inc(gsub_sem, 1)

    nc.compile()
    assert len(nc._v4_plan_dve) == n_dve
    nc._v4_plan = (nc._v4_plan_dve, plan_act)
    return nc


def gram_from_partials_v4(partials: np.ndarray, plan) -> np.ndarray:
    plan_dve, plan_act = plan
    s = partials.astype(np.float64).reshape(-1, partials.shape[-1]).sum(axis=0)
    g = np.zeros((3, 3), dtype=np.float64)
    for c, (i, j) in enumerate(plan_dve + plan_act):
        if i == j:
            g[i, i] += s[c]
        else:
            g[i, j] += s[c]
            g[j, i] += s[c]
    return g


def gram_from_partials_v3(partials: np.ndarray, n_tiles: int, group: int) -> np.ndarray:
    n_groups = n_tiles // group
    ncr = 3 * n_groups
    s = partials.astype(np.float64).reshape(-1, partials.shape[-1]).sum(axis=0)
    cr = s[:ncr].reshape(-1, 3).sum(axis=0)
    sq = s[ncr:].reshape(-1, 3).sum(axis=0)
    g = np.empty((3, 3), dtype=np.float64)
    g[0, 0], g[1, 1], g[2, 2] = sq
    for k, (i, j) in enumerate(_PAIRS):
        g[i, j] = g[j, i] = cr[k]
    return g


_NC_CACHE: dict[tuple, object] = {}


def _get_nc(n_rows: int, n_tiles: int, use_act: bool, raw: bool = False,
            group: int = 4, version: int = 4, n_bufs: int = 12,
            strip_barriers: bool = True, gp_stride: int = 0,
            bulk_r: int = 512, tail_rs: tuple = (256, 128, 64, 32, 32),
            head_rs: tuple = (128, 256, 256, 384), dpad: int = 3,
            head_dma_on_vector: bool = True):
    key = (n_rows, n_tiles, use_act, raw, group, version, n_bufs,
           strip_barriers, gp_stride, bulk_r, tail_rs, head_rs, dpad,
           head_dma_on_vector)
    if key not in _NC_CACHE:
        if version == 4:
            _NC_CACHE[key] = build_gram_kernel_v4(
                n_rows, bulk_r=bulk_r, n_bufs=n_bufs, group=group,
                gp_stride=gp_stride, head_rs=head_rs, tail_rs=tail_rs,
                dpad=dpad, head_dma_on_vector=head_dma_on_vector,
                strip_barriers=strip_barriers)
        elif version == 3:
            _NC_CACHE[key] = build_gram_kernel_v3(
                n_rows, n_tiles, n_bufs=n_bufs, group=group,
                strip_barriers=strip_barriers)
        elif raw:
            _NC_CACHE[key] = build_gram_kernel_raw(n_rows, n_tiles, group=group)
        else:
            _NC_CACHE[key] = build_gram_kernel(n_rows, n_tiles, use_act)
    return _NC_CACHE[key]


def gram_from_partials(partials: np.ndarray, n_tiles: int | None = None) -> np.ndarray:
    """[..., 128, 6*slots] partials -> full 3x3 Gram matrix (float64)."""
    slots = partials.shape[-1] // 6
    s = partials.astype(np.float64).reshape(-1, 6 * slots).sum(axis=0)
    sq = s[: 3 * slots].reshape(slots, 3).sum(axis=0)
    cr = s[3 * slots :].reshape(slots, 3).sum(axis=0)
    g = np.empty((3, 3), dtype=np.float64)
    g[0, 0], g[1, 1], g[2, 2] = sq
    for k, (i, j) in enumerate(_PAIRS):
        g[i, j] = g[j, i] = cr[k]
    return g


def run_device_partials(predictions: np.ndarray, targets: np.ndarray,
                        n_tiles: int = 4, use_act: bool = True,
                        raw: bool = False, group: int = 4, version: int = 4,
                        n_bufs: int = 12, strip_barriers: bool = True,
                        gp_stride: int = 0, bulk_r: int = 512,
                        tail_rs: tuple = (256, 128, 64, 32, 32),
                        head_rs: tuple = (128, 256, 256, 384), dpad: int = 3,
                        head_dma_on_vector: bool = True,
                        **run_kwargs):
    """Shard over N_CORES, run on device, return per-core partials + results."""
    b = predictions.shape[0]
    assert b % N_CORES == 0
    n_rows = b // N_CORES
    nc = _get_nc(n_rows, n_tiles, use_act, raw, group, version, n_bufs,
                 strip_barriers, gp_stride, bulk_r, tail_rs, head_rs, dpad,
                 head_dma_on_vector)
    preds = np.ascontiguousarray(predictions, dtype=np.float32).reshape(
        N_CORES, n_rows, 3
    )
    targs = np.ascontiguousarray(targets, dtype=np.float32).reshape(
        N_CORES, n_rows, 3
    )
    in_maps = [
        {"pt": np.stack([preds[c], targs[c]])} for c in range(N_CORES)
    ]
    res = run_bass_kernel_spmd(nc, in_maps, list(range(N_CORES)), **run_kwargs)
    partials = np.stack([r["partials"] for r in res.results])
    return partials, res, nc


def _host_loss(predictions, targets, sigma_inv, logdet, lo=0, hi=None):
    """Exact (float64) loss over rows [lo, hi) on the host, chunked."""
    hi = predictions.shape[0] if hi is None else hi
    tot = 0.0
    for s in range(lo, hi, 1 << 20):
        e = min(s + (1 << 20), hi)
        d = predictions[s:e].astype(np.float64) - targets[s:e].astype(np.float64)
        tot += float(np.einsum("bi,ij,bj->", d, sigma_inv, d))
    return abs(logdet + tot / (hi - lo))


def kernel(predictions: np.ndarray, targets: np.ndarray, sigma: np.ndarray) -> np.ndarray:
    predictions = np.asarray(predictions, dtype=np.float32)
    targets = np.asarray(targets, dtype=np.float32)
    sigma64 = np.asarray(sigma, dtype=np.float64)
    sigma_inv = np.linalg.inv(sigma64)
    _, logdet = np.linalg.slogdet(sigma64)

    # Cheap subsample estimate (~0.3% rel) to sanity-gate the device result.
    est = _host_loss(predictions, targets, sigma_inv, logdet,
                     0, min(1 << 16, predictions.shape[0]))

    loss = None
    for _attempt in range(2):
        partials, _, nc = run_device_partials(predictions, targets, version=4)
        g = gram_from_partials_v4(partials, nc._v4_plan)
        mean_mahal = float((sigma_inv * g).sum()) / predictions.shape[0]
        loss = abs(logdet + mean_mahal)
        if np.isfinite(loss) and abs(loss - est) <= 0.05 * max(abs(est), 1e-9):
            return np.float32(loss)
    # Device result failed the sanity gate twice: fall back to exact host.
    return np.float32(_host_loss(predictions, targets, sigma_inv, logdet))



# revision 6
# speedup vs baseline: 1.1668x; 1.1668x over previous
"""Trainium2 Bass kernel for CustomLossWithCovariance.

loss = abs(logdet(sigma) + mean_b[(p_b - t_b)^T sigma^{-1} (p_b - t_b)])

Only the 3x3 Gram matrix G = sum_b d_b d_b^T (d = pred - targ) requires
touching the [B, 3] data; the device computes per-core partial pair-sums
of G, and the host finishes with the tiny 3x3 algebra:
    mean_mahalanobis = <sigma_inv, G> / B
    loss = |logdet(sigma) + mean_mahalanobis|

Sharding: data-parallel over the batch across 8 NeuronCores (each core
streams a contiguous [B/8, 3] shard; partial sums gathered on host).

Production path: build_gram_kernel_v4 (raw Bacc, manual semaphores).
Per tile: one dma_start brings pred|targ halves; DVE subtracts into a
row-interleaved bf16 d buffer (unit-stride write — scatter writes are
4.6x slower on DVE); DVE fused multiply-reduces (stride-3 component
reads, grouped across tiles) produce the cross sums and ACT Square
accumulate produces the diagonals. Tile sizes ramp up at the head (so
DVE starts ~5us earlier) and shrink at the tail (so the post-last-DMA
dependency chain is short). The fp32 ring slot is freed by the sub
alone, letting the DMA stream run n_bufs tiles ahead. Bass's two
__init__ all-engine barriers are stripped (saves ~1.5us; the one real
dependency — gpsimd const memsets before ACT bias reads — is re-fenced
with boot_sem). Accumulator tensors are pre-zeroed so any flush race
degrades to ~1e-4 error instead of garbage, and kernel() additionally
sanity-gates the device result against a host subsample estimate with
retry + exact-host fallback.

Older variants (build_gram_kernel, build_gram_kernel_raw,
build_gram_kernel_v3) are kept for reference only.
"""

import numpy as np

import concourse.bass as bass
import concourse.bacc as bacc
import concourse.mybir as mybir
from concourse import tile
from concourse.bass_utils import run_bass_kernel_spmd

N_CORES = 8
B_FULL = 8388608
P = 128

_PAIRS = [(0, 1), (0, 2), (1, 2)]


def build_gram_kernel(n_rows: int, n_tiles: int, use_act: bool = True):
    """Build the per-core Bass module.

    Input: pt [2, n_rows, 3] f32 (pred stacked with targ)
    Output: partials [128, 6 * n_tiles] f32
        col t*3+i            : sum over this tile/partition of d_i^2
        col 3*n_tiles + t*3+k: sum of d_i*d_j for pair k in _PAIRS
    """
    assert n_rows % (P * n_tiles) == 0
    r = n_rows // (P * n_tiles)  # rows per partition per tile
    m = 3 * r                    # flat f32 elements per partition per tile
    f32 = mybir.dt.float32

    # Bacc (not plain Bass): its compile() pass legalizes semaphore waits
    # (each TRN2 instruction holds at most one wait slot).
    nc = bacc.Bacc("TRN2", target_bir_lowering=False, debug=False)
    pt = nc.dram_tensor("pt", [2, n_rows, 3], f32, kind="ExternalInput")
    out = nc.dram_tensor("partials", [P, 6 * n_tiles], f32, kind="ExternalOutput")

    # [t][p][w(2), m] — per tile/partition: pred chunk and targ chunk, each
    # m contiguous f32 in DRAM.
    pt_v = pt[:].rearrange("w (t p r) c -> t p w (r c)", t=n_tiles, p=P)

    with tile.TileContext(nc) as tc:
        with (
            tc.tile_pool(name="io", bufs=3) as io_pool,
            tc.tile_pool(name="dve_scr", bufs=2) as dve_scr,
            tc.tile_pool(name="act_scr", bufs=2) as act_scr,
            tc.tile_pool(name="acc", bufs=1) as acc_pool,
        ):
            acc_sq = acc_pool.tile([P, 3 * n_tiles], f32)
            acc_cr = acc_pool.tile([P, 3 * n_tiles], f32)

            for t in range(n_tiles):
                buf = io_pool.tile([P, 2 * m], f32, tag="buf")
                nc.sync.dma_start(
                    out=buf[:].rearrange("p (w m) -> p w m", w=2),
                    in_=pt_v[t],
                )

                # In-place: d = pred - targ, overwriting the pred half.
                nc.vector.tensor_tensor(
                    out=buf[:, 0:m],
                    in0=buf[:, 0:m],
                    in1=buf[:, m : 2 * m],
                    op=mybir.AluOpType.subtract,
                )
                d3 = buf[:, 0:m].rearrange("p (r c) -> p c r", c=3)

                # Diagonal sums on the scalar engine (Square + accum_out),
                # overlapping with the DVE cross-products.
                if use_act:
                    for i in range(3):
                        sq = act_scr.tile([P, r], f32, tag="sq")
                        nc.scalar.activation(
                            out=sq[:],
                            in_=d3[:, i, :],
                            func=mybir.ActivationFunctionType.Square,
                            accum_out=acc_sq[:, t * 3 + i : t * 3 + i + 1],
                        )
                else:
                    for i in range(3):
                        sq = dve_scr.tile([P, r], f32, tag="pr")
                        nc.vector.scalar_tensor_tensor(
                            out=sq[:],
                            in0=d3[:, i, :],
                            scalar=1.0,
                            in1=d3[:, i, :],
                            op0=mybir.AluOpType.mult,
                            op1=mybir.AluOpType.mult,
                            accum_out=acc_sq[:, t * 3 + i : t * 3 + i + 1],
                        )
                # Cross sums: fused multiply+reduce on DVE
                # (scalar_tensor_tensor: out = (in0 * 1.0) * in1, accum = sum).
                for k, (i, j) in enumerate(_PAIRS):
                    pr = dve_scr.tile([P, r], f32, tag="pr")
                    nc.vector.scalar_tensor_tensor(
                        out=pr[:],
                        in0=d3[:, i, :],
                        scalar=1.0,
                        in1=d3[:, j, :],
                        op0=mybir.AluOpType.mult,
                        op1=mybir.AluOpType.mult,
                        accum_out=acc_cr[:, t * 3 + k : t * 3 + k + 1],
                    )

            nc.sync.dma_start(out=out[:, 0 : 3 * n_tiles], in_=acc_sq[:])
            nc.sync.dma_start(out=out[:, 3 * n_tiles : 6 * n_tiles], in_=acc_cr[:])

    nc.compile()
    return nc


def build_gram_kernel_raw(n_rows: int, n_tiles: int = 32, n_bufs: int = 24,
                          group: int = 4, skip_exit_barrier: bool = True):
    """Raw-Bacc variant: manual semaphores, no TileContext.

    Skips Tile's prologue/epilogue (drain + two all-engine EVSEM
    barriers, ~16 us) — the only sync needed is a three-semaphore chain:
    DMA loads (one HWDGE ring) -> DVE -> ACT.

    The ring of tile buffers lives in ONE SBUF tensor so the fused
    multiply-reduces can span `group` consecutive tiles with a single
    instruction (free-dim AP [group, r]) — amortizing the per-op fixed
    cost and the accumulator-drain, which keeps both compute engines
    well under the DMA pace.

    Input: pt [2, n_rows, 3] f32. Output: partials [128, 6 * n_groups]
    (same slot layout as build_gram_kernel, with n_groups slots).
    """
    assert n_tiles % group == 0 and n_bufs % group == 0
    assert n_rows % (P * n_tiles) == 0
    n_groups = n_tiles // group
    r = n_rows // (P * n_tiles)
    m = 3 * r
    f32 = mybir.dt.float32

    nc = bacc.Bacc("TRN2", target_bir_lowering=False, debug=False)
    pt = nc.dram_tensor("pt", [2, n_rows, 3], f32, kind="ExternalInput")
    out = nc.dram_tensor("partials", [P, 6 * n_groups], f32, kind="ExternalOutput")
    pt_v = pt[:].rearrange("w (t p r) c -> t p w (r c)", t=n_tiles, p=P)

    ring = nc.alloc_sbuf_tensor("ring", [P, n_bufs * 2 * m], f32).ap()

    def buf(t):
        s = t % n_bufs
        return ring[:, s * 2 * m : (s + 1) * 2 * m]

    def dgroup(g, i):
        # component i of the diff halves of tiles 4g..4g+3: [128, group, r]
        s0 = (g * group) % n_bufs
        w = ring[:, s0 * 2 * m : (s0 + group) * 2 * m]
        return w.rearrange("p (t w r c) -> p t w c r", t=group, w=2, c=3)[:, :, 0, i, :]

    acc_sq = nc.alloc_sbuf_tensor("acc_sq", [P, 3 * n_groups], f32).ap()
    acc_cr = nc.alloc_sbuf_tensor("acc_cr", [P, 3 * n_groups], f32).ap()
    # Rotated scratch (dead stores of the fused ops), 2 groups deep so each
    # group's single stale semaphore wait also covers the scratch WAW from
    # two groups back.
    pr_scrs = [
        nc.alloc_sbuf_tensor(f"pr_scr{k}", [P, group * r], f32).ap() for k in range(6)
    ]
    sq_scrs = [
        nc.alloc_sbuf_tensor(f"sq_scr{k}", [P, group * r], f32).ap() for k in range(6)
    ]

    # One DMA-completion semaphore per ring buffer: a single shared sem
    # would be unsound — each dma_start is split across 16 SDMA engines
    # whose sub-completions interleave across in-flight DMAs.
    dma_sems = [nc.alloc_semaphore(f"dma_sem{i}") for i in range(n_bufs)]
    out_sem = nc.alloc_semaphore("out_sem")
    dve_sem = nc.alloc_semaphore("dve_sem")
    act_sem = nc.alloc_semaphore("act_sem")

    # DVE emission order: subs run ahead; the grouped multiply-reduces for
    # group g are emitted after sub(4g+4) so their drain-wait on the last
    # sub of the group is already satisfied when it executes (DVE writes
    # drain asynchronously). Only the last group trails the final sub.
    dve_order = []
    for t in range(n_tiles):
        dve_order.append(("sub", t))
        if t % group == 0 and t >= group:
            # one sub of stagger after the group's last sub
            dve_order.append(("stt", t // group - 1))
    dve_order.append(("stt", n_groups - 1))
    sub_done, sttg_done = {}, {}
    v = 0
    for kind, x in dve_order:
        if kind == "sub":
            v += 1
            sub_done[x] = v
        else:
            v += 3
            sttg_done[x] = v

    # Output chunks: flush finished accumulator columns while later tiles
    # still stream, so the tail only waits on the last small chunk.
    chunk = max(1, n_groups // 2)
    chunks = [(c, min(c + chunk, n_groups)) for c in range(0, n_groups, chunk)]

    import contextlib

    @contextlib.contextmanager
    def _block():
        # no_gpsimd_drain=True emits per-engine drains explicitly and then a
        # sem-only all-engine butterfly. The butterfly only delays NEFF end
        # (outputs are already fenced by the sequencer's out_sem wait), so
        # optionally no-op it during Block.__exit__.
        with nc.Block(no_gpsimd_drain=True) as blk:
            try:
                yield blk
            finally:
                if skip_exit_barrier:
                    nc.all_engine_barrier = lambda **kw: None
        if skip_exit_barrier:
            del nc.all_engine_barrier  # restore class method

    with _block() as block:

        @block.sync
        def _(sync):
            for t in range(n_tiles):
                if head_dma_on_vector and t < h0:
                    continue  # issued from the vector queue (clears the
                              # NEFF entry barrier ~1.4us before sync)
                if t >= n_bufs:
                    # ring reuse: all consumers of the buffer's previous
                    # occupant (tile t - n_bufs) must be done
                    prev = t - n_bufs
                    sync.wait_ge(dve_sem, sttg_done[prev // group])
                    sync.wait_ge(act_sem, 3 * (prev // group + 1))
                sync.dma_start(
                    out=buf(t).rearrange("p (w m) -> p w m", w=2),
                    in_=pt_v[t],
                ).then_inc(dma_sems[t % n_bufs], 16)
            n_out = 0
            for lo, hi in chunks:
                sync.wait_ge(act_sem, 3 * hi)
                sync.dma_start(
                    out=out[:, 3 * lo : 3 * hi], in_=acc_sq[:, 3 * lo : 3 * hi]
                ).then_inc(out_sem, 16)
                sync.wait_ge(dve_sem, sttg_done[hi - 1])
                sync.dma_start(
                    out=out[:, 3 * (n_groups + lo) : 3 * (n_groups + hi)],
                    in_=acc_cr[:, 3 * lo : 3 * hi],
                ).then_inc(out_sem, 16)
                n_out += 32
            sync.wait_ge(out_sem, n_out)

        @block.vector
        def _(vector):
            for kind, x in dve_order:
                if kind == "sub":
                    b = buf(x)
                    vector.wait_ge(dma_sems[x % n_bufs], 16 * (x // n_bufs + 1))
                    vector.tensor_tensor(
                        out=b[:, 0:m],
                        in0=b[:, 0:m],
                        in1=b[:, m : 2 * m],
                        op=mybir.AluOpType.subtract,
                    ).then_inc(dve_sem, 1)
                else:
                    vector.wait_ge(dve_sem, sub_done[(x + 1) * group - 1])
                    for k, (i, j) in enumerate(_PAIRS):
                        vector.scalar_tensor_tensor(
                            out=pr_scrs[(x % 2) * 3 + k][:].rearrange(
                                "p (t r) -> p t r", t=group
                            ),
                            in0=dgroup(x, i),
                            scalar=1.0,
                            in1=dgroup(x, j),
                            op0=mybir.AluOpType.mult,
                            op1=mybir.AluOpType.mult,
                            accum_out=acc_cr[:, x * 3 + k : x * 3 + k + 1],
                        ).then_inc(dve_sem, 1)

        @block.scalar
        def _(scalar):
            for g in range(n_groups):
                scalar.wait_ge(dve_sem, sub_done[(g + 1) * group - 1])
                if g >= 2:
                    # scratch slot reuse from two groups back
                    scalar.wait_ge(act_sem, 3 * (g - 1))
                for i in range(3):
                    scalar.activation(
                        out=sq_scrs[(g % 2) * 3 + i][:].rearrange(
                            "p (t r) -> p t r", t=group
                        ),
                        in_=dgroup(g, i),
                        func=mybir.ActivationFunctionType.Square,
                        accum_out=acc_sq[:, g * 3 + i : g * 3 + i + 1],
                    ).then_inc(act_sem, 1)

    nc.compile()
    return nc

def _strip_entry_barriers(nc):
    """Remove the two all-engine entry barriers Bass.__init__ emits.

    They serialize ~4us of semaphore round-trips before the first DMA can
    issue. The only cross-engine ordering they provide that this kernel
    needs is gpsimd-const-AP-memset -> ACT-bias-read, which is re-fenced
    explicitly with boot_sem in build_gram_kernel_v3.
    """
    bar = set(nc.barrier_sems)
    blk = nc.main_func.blocks[0]
    drop = []
    for ins in blk.instructions:
        si = getattr(ins, "sync_info", None)
        if si is None:
            continue
        sems = {w.id for w in si.on_wait or []}
        sems |= {u.id for u in si.on_update or []}
        if sems & bar:
            drop.append(ins)
    for ins in drop:
        blk.instructions.remove(ins)
    return len(drop)


def build_gram_kernel_v3(n_rows: int, n_tiles: int = 16, n_bufs: int = 8,
                         group: int = 4, strip_barriers: bool = True,
                         skip_exit_barrier: bool = True):
    """v3: planar-bf16 d + 2x DVE reduces + ACT squares.

    Per tile: DMA both halves -> DVE sub (fp32 in, planar bf16 out:
    component planes x|y|z so reduce operands are unit-stride 2-byte,
    unlocking the DVE 2x perf mode) -> DVE cross-product reduces (grouped
    `group` tiles per instr) + ACT Square reduces (grouped; per-tile for
    the last group so the post-DMA tail stays short).

    The fp32 ring slot is freed by the sub alone (d lives in its own
    full-size buffer), so the DMA stream runs ~n_bufs tiles ahead of
    compute and never stalls on the reduce bursts.

    Output layout [128, 3*n_groups + 3*(n_groups-1) + 3*group]:
      cols 0 .. 3*n_groups-1: cross sums (group g, pair k at 3g+k)
      then squares: full groups 0..n_groups-2 (3 each), then the last
      group's tiles individually (3 each).
    """
    assert n_rows % (P * n_tiles) == 0 and n_tiles % group == 0
    r = n_rows // (P * n_tiles)
    m = 3 * r
    n_groups = n_tiles // group
    full_sq = n_groups - 1           # square-groups emitted grouped
    tail0 = full_sq * group          # first per-tile-squares tile
    ncr = 3 * n_groups
    nsq = 3 * full_sq + 3 * group
    f32, bf16 = mybir.dt.float32, mybir.dt.bfloat16

    nc = bacc.Bacc("TRN2", target_bir_lowering=False, debug=False)
    if strip_barriers:
        _strip_entry_barriers(nc)
    pt = nc.dram_tensor("pt", [2, n_rows, 3], f32, kind="ExternalInput")
    out = nc.dram_tensor("partials", [P, ncr + nsq], f32, kind="ExternalOutput")
    pt_v = pt[:].rearrange("w (t p r) c -> t p w (r c)", t=n_tiles, p=P)

    ring = nc.alloc_sbuf_tensor("ring", [P, n_bufs * 2 * m], f32).ap()
    d_all = nc.alloc_sbuf_tensor("d_all", [P, n_tiles * m], bf16).ap()
    d_t = d_all.rearrange("p (t c r) -> p t c r", t=n_tiles, c=3)
    acc_cr = nc.alloc_sbuf_tensor("acc_cr", [P, ncr], f32).ap()
    acc_sq = nc.alloc_sbuf_tensor("acc_sq", [P, nsq], f32).ap()
    # Dead stores of the fused reduces; single slot per engine (each
    # engine executes its own stream in order, so WAW is safe).
    cr_scr = nc.alloc_sbuf_tensor("cr_scr", [P, group * r], bf16).ap()
    sq_scr = nc.alloc_sbuf_tensor("sq_scr", [P, group * r], bf16).ap()

    dma_sems = [nc.alloc_semaphore(f"dma{i}") for i in range(n_bufs)]
    sub_sem = nc.alloc_semaphore("sub_sem")
    red_sem = nc.alloc_semaphore("red_sem")
    act_sem = nc.alloc_semaphore("act_sem")
    out_sem = nc.alloc_semaphore("out_sem")
    boot_sem = nc.alloc_semaphore("boot_sem")

    def dcomp(t0, nt, i):
        # component i of tiles t0..t0+nt-1: [128, nt, r] unit-stride bf16
        v = d_t[:, t0 : t0 + nt, i, :]
        return v

    import contextlib

    @contextlib.contextmanager
    def _block():
        with nc.Block(no_gpsimd_drain=True) as blk:
            try:
                yield blk
            finally:
                if skip_exit_barrier:
                    nc.all_engine_barrier = lambda **kw: None
        if skip_exit_barrier:
            del nc.all_engine_barrier  # restore class method

    with _block() as block:

        @block.gpsimd
        def _(gpsimd):
            # Const-AP memsets (ACT bias) are earlier in gpsimd's stream;
            # this inc publishes their completion to the scalar queue.
            gpsimd.sem_inc(boot_sem, 1)

        @block.sync
        def _(sync):
            for t in range(n_tiles):
                if head_dma_on_vector and t < h0:
                    continue  # issued from the vector queue (clears the
                              # NEFF entry barrier ~1.4us before sync)
                if t >= n_bufs:
                    # ring slot free once its previous occupant was subbed
                    sync.wait_ge(sub_sem, t - n_bufs + 1)
                sync.dma_start(
                    out=ring[:, (t % n_bufs) * 2 * m : (t % n_bufs + 1) * 2 * m]
                    .rearrange("p (w m) -> p w m", w=2),
                    in_=pt_v[t],
                ).then_inc(dma_sems[t % n_bufs], 16)
            # accumulator flush: big chunks early, last-group slivers at end
            sync.wait_ge(red_sem, 3 * (n_groups - 1))
            sync.dma_start(
                out=out[:, 0 : 3 * (n_groups - 1)],
                in_=acc_cr[:, 0 : 3 * (n_groups - 1)],
            ).then_inc(out_sem, 16)
            sync.wait_ge(act_sem, 3 * full_sq)
            sync.dma_start(
                out=out[:, ncr : ncr + 3 * full_sq],
                in_=acc_sq[:, 0 : 3 * full_sq],
            ).then_inc(out_sem, 16)
            sync.wait_ge(red_sem, 3 * n_groups)
            sync.dma_start(
                out=out[:, 3 * (n_groups - 1) : ncr],
                in_=acc_cr[:, 3 * (n_groups - 1) : ncr],
            ).then_inc(out_sem, 16)
            sync.wait_ge(act_sem, nsq)
            sync.dma_start(
                out=out[:, ncr + 3 * full_sq : ncr + nsq],
                in_=acc_sq[:, 3 * full_sq : nsq],
            ).then_inc(out_sem, 16)
            sync.wait_ge(out_sem, 64)

        @block.vector
        def _(vector):
            for t in range(n_tiles):
                s = t % n_bufs
                buf = ring[:, s * 2 * m : (s + 1) * 2 * m]
                vector.wait_ge(dma_sems[s], 16 * (t // n_bufs + 1))
                # d = pred - targ, downcast to bf16, scattered into
                # component planes (write AP [r, 3] w/ strides [1, r])
                vector.tensor_tensor(
                    out=d_all[:, t * m : (t + 1) * m].rearrange(
                        "p (c r) -> p r c", c=3
                    ),
                    in0=buf[:, 0:m],
                    in1=buf[:, m : 2 * m],
                    op=mybir.AluOpType.subtract,
                ).then_inc(sub_sem, 1)
                if t % group == group - 1:
                    g = t // group
                    for k, (i, j) in enumerate(_PAIRS):
                        vector.scalar_tensor_tensor(
                            out=cr_scr[:].rearrange("p (t r) -> p t r", t=group),
                            in0=dcomp(g * group, group, i),
                            scalar=1.0,
                            in1=dcomp(g * group, group, j),
                            op0=mybir.AluOpType.mult,
                            op1=mybir.AluOpType.mult,
                            accum_out=acc_cr[:, g * 3 + k : g * 3 + k + 1],
                        ).then_inc(red_sem, 1)

        @block.scalar
        def _(scalar):
            scalar.wait_ge(boot_sem, 1)
            for g in range(full_sq):
                scalar.wait_ge(sub_sem, group * (g + 1))
                for i in range(3):
                    scalar.activation(
                        out=sq_scr[:].rearrange("p (t r) -> p t r", t=group),
                        in_=dcomp(g * group, group, i),
                        func=mybir.ActivationFunctionType.Square,
                        accum_out=acc_sq[:, g * 3 + i : g * 3 + i + 1],
                    ).then_inc(act_sem, 1)
            for w, t in enumerate(range(tail0, n_tiles)):
                scalar.wait_ge(sub_sem, t + 1)
                for i in range(3):
                    c = 3 * full_sq + w * 3 + i
                    scalar.activation(
                        out=sq_scr[:, 0:r],
                        in_=dcomp(t, 1, i),
                        func=mybir.ActivationFunctionType.Square,
                        accum_out=acc_sq[:, c : c + 1],
                    ).then_inc(act_sem, 1)

    nc.compile()
    nc._v3_meta = (n_tiles, group)
    return nc


def build_gram_kernel_v4(n_rows: int, bulk_r: int = 512, n_bufs: int = 12,
                         group: int = 4, gp_stride: int = 0,
                         head_rs: tuple = (128, 256, 256, 384),
                         tail_rs: tuple = (256, 128, 64, 32, 32),
                         dpad: int = 3, head_dma_on_vector: bool = True,
                         strip_barriers: bool = True,
                         skip_exit_barrier: bool = True):
    """v4: interleaved-bf16 d, measured-cost engine mix, shrinking tail.

    Measured HW rates (ns per 128-wide column): DVE sub fp32->bf16 unit
    1.28; DVE stt reduce bf16 stride-3 1.32; ACT Square ~1.0-1.4 + 740
    fixed; GPSIMD sub ~3.5. Writes must be unit-stride (scatter = 4.6x);
    strided reads are cheap. So d stays row-interleaved bf16.

    - bulk tiles of r=bulk_r rows/partition; every gp_stride-th bulk tile's
      sub runs on GPSIMD to keep DVE under the DMA pace.
    - cross-products: DVE stt grouped over `group` consecutive bulk tiles.
    - squares: ACT, same grouping; tail tiles per-tile; r<=32 tails on DVE.
    - tail tiles shrink so the post-last-DMA dependency chain is tiny.
    """
    R = n_rows // P
    assert n_rows % P == 0
    bulk_n = (R - sum(head_rs) - sum(tail_rs)) // bulk_r
    assert sum(head_rs) + bulk_n * bulk_r + sum(tail_rs) == R
    rs = list(head_rs) + [bulk_r] * bulk_n + list(tail_rs)
    n_tiles = len(rs)
    h0 = len(head_rs)               # first bulk tile index
    t0_tail = h0 + bulk_n           # first tail tile index
    cum = [0]
    for r in rs:
        cum.append(cum[-1] + r)
    # bulk groups: chunks of `group` (absolute tile indices)
    groups = [list(range(s, min(s + group, t0_tail)))
              for s in range(h0, t0_tail, group)]
    group_last = {g[-1]: g for g in groups}
    # per-tile (ungrouped) reduce tiles: head + tail
    per_tile = set(range(0, h0)) | set(range(t0_tail, n_tiles))
    # every gp_stride-th bulk tile's sub runs on GPSIMD (0 = none)
    gp_tiles = (set(range(h0, t0_tail, gp_stride)) if gp_stride else set())
    f32, bf16 = mybir.dt.float32, mybir.dt.bfloat16

    nc = bacc.Bacc("TRN2", target_bir_lowering=False, debug=False)
    if strip_barriers:
        _strip_entry_barriers(nc)
    pt = nc.dram_tensor("pt", [2, n_rows, 3], f32, kind="ExternalInput")

    # per-tile engine assignment of the sub + cumulative sem targets
    dve_idx, gp_idx = {}, {}
    for t in range(n_tiles):
        if t in gp_tiles:
            gp_idx[t] = len(gp_idx)
        else:
            dve_idx[t] = len(dve_idx)

    def sub_waits(last_t):
        """(sub_sem target, gsub_sem target) covering tiles 0..last_t."""
        d = sum(1 for t, i in dve_idx.items() if t <= last_t)
        g = sum(1 for t, i in gp_idx.items() if t <= last_t)
        return d, g

    # reduce slot counts (order finalized at emission)
    n_dve = 3 * len(groups) + 3 * len(per_tile) + 3 * sum(
        1 for t in per_tile if rs[t] <= 32)
    plan_act = []
    for t in sorted(per_tile):
        if t < h0 and rs[t] > 32:
            for i in range(3):
                plan_act.append((i, i))
    for g_tiles in groups:
        for i in range(3):
            plan_act.append((i, i))
    for t in sorted(per_tile):
        if t >= t0_tail and rs[t] > 32:
            for i in range(3):
                plan_act.append((i, i))
    n_act = len(plan_act)
    out = nc.dram_tensor("partials", [P, n_dve + n_act], f32,
                         kind="ExternalOutput")

    m_bulk = 3 * bulk_r
    ring = nc.alloc_sbuf_tensor("ring", [P, n_bufs * 2 * m_bulk], f32).ap()
    d_all = nc.alloc_sbuf_tensor("d_all", [P, dpad * R], bf16).ap()
    acc_dve = nc.alloc_sbuf_tensor("acc_dve", [P, n_dve], f32).ap()
    acc_act = nc.alloc_sbuf_tensor("acc_act", [P, n_act], f32).ap()
    cr_scr = nc.alloc_sbuf_tensor("cr_scr", [P, group * bulk_r], bf16).ap()
    sq_scr = nc.alloc_sbuf_tensor("sq_scr", [P, group * bulk_r], bf16).ap()

    dma_sems = [nc.alloc_semaphore(f"dma{i}") for i in range(n_bufs)]
    sub_sem = nc.alloc_semaphore("sub_sem")
    gsub_sem = nc.alloc_semaphore("gsub_sem")
    red_sem = nc.alloc_semaphore("red_sem")
    act_sem = nc.alloc_semaphore("act_sem")
    out_sem = nc.alloc_semaphore("out_sem")
    boot_sem = nc.alloc_semaphore("boot_sem")

    def ring_slot(t):
        s = t % n_bufs
        return ring[:, s * 2 * m_bulk : s * 2 * m_bulk + 2 * 3 * rs[t]]

    def pt_tile(t):
        sl = pt[:, cum[t] * P : cum[t + 1] * P, :]
        return sl.rearrange("w (p r) c -> p w (r c)", p=P)

    def dseg(t0, nt, i):
        # component i of tiles t0..t0+nt-1 (equal r), stride-dpad reads
        v = d_all[:, dpad * cum[t0] : dpad * cum[t0 + nt]]
        return v.rearrange("p (t r c) -> p t c r", t=nt, c=dpad)[:, :, i, :]

    def dsub_out(t):
        # write view for the sub: rows of 3 packed comps, dpad-elem row pitch
        v = d_all[:, dpad * cum[t] : dpad * cum[t + 1]]
        if dpad == 3:
            return v
        return v.rearrange("p (r c) -> p r c", c=dpad)[:, :, 0:3]

    # early-flush boundaries: head reduces + all-but-last bulk group are
    # guaranteed emitted before any tail reduce; ACT head+bulk likewise
    red_early = 3 * h0 + 3 * (len(groups) - 1)
    act_bulk = 3 * h0 + 3 * len(groups)
    _red_pen = max(red_early, n_dve - 6)
    _act_pen = max(act_bulk, n_act - 3)
    n_flush_dve = sum(1 for lo, hi in ((0, red_early), (red_early, _red_pen),
                                       (_red_pen, n_dve)) if hi > lo)
    act_chunks = [(act_bulk, 0, act_bulk), (_act_pen, act_bulk, _act_pen),
                  (n_act, _act_pen, n_act)]
    act_chunks = [c for c in act_chunks if c[2] > c[1]]
    n_flush_act = len(act_chunks)

    import contextlib

    @contextlib.contextmanager
    def _block():
        with nc.Block(no_gpsimd_drain=True) as blk:
            try:
                yield blk
            finally:
                if skip_exit_barrier:
                    nc.all_engine_barrier = lambda **kw: None
        if skip_exit_barrier:
            del nc.all_engine_barrier

    with _block() as block:

        @block.sync
        def _(sync):
            for t in range(n_tiles):
                if head_dma_on_vector and t < h0:
                    continue  # issued from the vector queue (clears the
                              # NEFF entry barrier ~1.4us before sync)
                if t >= n_bufs:
                    u = t - n_bufs
                    if u in gp_idx:
                        sync.wait_ge(gsub_sem, gp_idx[u] + 1)
                    else:
                        sync.wait_ge(sub_sem, dve_idx[u] + 1)
                sync.dma_start(
                    out=ring_slot(t).rearrange("p (w m) -> p w m", w=2),
                    in_=pt_tile(t),
                ).then_inc(dma_sems[t % n_bufs], 16)
            # acc_dve flushes stay here; acc_act flushes issue from the
            # scalar queue (ACT finishes ~2.5us before DVE, so they then
            # complete during the DVE tail instead of serializing after it)
            red_pen = max(red_early, n_dve - 6)   # all but last tail tile
            for val, lo, hi in ((red_early, 0, red_early),
                                (red_pen, red_early, red_pen),
                                (n_dve, red_pen, n_dve)):
                if hi <= lo:
                    continue
                sync.wait_ge(red_sem, val)
                sync.dma_start(out=out[:, lo:hi],
                               in_=acc_dve[:, lo:hi]).then_inc(out_sem, 16)
            sync.wait_ge(out_sem, 16 * (n_flush_dve + n_flush_act))

        @block.vector
        def _(vector):
            red_c = 0
            plan_dyn = []
            pending = []  # deferred bulk-group reduces, drained 1-per-sub

            def emit_one(t0, nt, i, j):
                nonlocal red_c
                r = rs[t0]
                vector.scalar_tensor_tensor(
                    out=cr_scr[:, 0 : nt * r].rearrange(
                        "p (t r) -> p t r", t=nt),
                    in0=dseg(t0, nt, i),
                    scalar=1.0,
                    in1=dseg(t0, nt, j),
                    op0=mybir.AluOpType.mult,
                    op1=mybir.AluOpType.mult,
                    accum_out=acc_dve[:, red_c : red_c + 1],
                ).then_inc(red_sem, 1)
                plan_dyn.append((i, j))
                red_c += 1

            for t in range(n_tiles):
                if t not in gp_tiles:
                    buf = ring_slot(t)
                    mt = 3 * rs[t]
                    vector.wait_ge(dma_sems[t % n_bufs], 16 * (t // n_bufs + 1))
                    vector.tensor_tensor(
                        out=dsub_out(t),
                        in0=buf[:, 0:mt],
                        in1=buf[:, mt : 2 * mt],
                        op=mybir.AluOpType.subtract,
                    ).then_inc(sub_sem, 1)
                    # spread deferred group reduces between subs so the DMA
                    # ring (freed by subs) never stalls on a reduce burst
                    if pending:
                        emit_one(*pending.pop(0))
                if t in group_last:
                    g_tiles = group_last[t]
                    _, gw = sub_waits(g_tiles[-1])
                    if gw:
                        vector.wait_ge(gsub_sem, gw)
                    for (i, j) in _PAIRS:
                        pending.append((g_tiles[0], len(g_tiles), i, j))
                elif t in per_tile:
                    for (i, j) in _PAIRS:
                        emit_one(t, 1, i, j)
                    if rs[t] <= 32:
                        for i in range(3):
                            emit_one(t, 1, i, i)
            while pending:
                emit_one(*pending.pop(0))
            nc._v4_plan_dve = plan_dyn

        @block.scalar
        def _(scalar):
            if head_dma_on_vector:
                # Scalar clears the NEFF entry-barrier chain ~2us before
                # Sync; issuing the head-tile loads here starts the stream
                # (and so the first sub) earlier. Ring slots are fresh, no
                # waits needed; the const-AP fence only matters for the
                # activations below.
                for t in range(h0):
                    scalar.dma_start(
                        out=ring_slot(t).rearrange("p (w m) -> p w m", w=2),
                        in_=pt_tile(t),
                    ).then_inc(dma_sems[t % n_bufs], 16)
            scalar.wait_ge(boot_sem, 1)
            act_c = 0

            def emit_squares(t0, nt):
                nonlocal act_c
                r = rs[t0]
                for i in range(3):
                    scalar.activation(
                        out=sq_scr[:, 0 : nt * r].rearrange(
                            "p (t r) -> p t r", t=nt),
                        in_=dseg(t0, nt, i),
                        func=mybir.ActivationFunctionType.Square,
                        accum_out=acc_act[:, act_c : act_c + 1],
                    ).then_inc(act_sem, 1)
                    act_c += 1

            def sq_waits(last_t):
                d, g = sub_waits(last_t)
                scalar.wait_ge(sub_sem, d)
                if g:
                    scalar.wait_ge(gsub_sem, g)

            for t in sorted(per_tile):
                if t < h0 and rs[t] > 32:
                    sq_waits(t)
                    emit_squares(t, 1)
            for g_tiles in groups:
                sq_waits(g_tiles[-1])
                emit_squares(g_tiles[0], len(g_tiles))
            for t in sorted(per_tile):
                if t >= t0_tail and rs[t] > 32:
                    sq_waits(t)
                    emit_squares(t, 1)
            for val, lo, hi in act_chunks:
                scalar.wait_ge(act_sem, val)
                scalar.dma_start(out=out[:, n_dve + lo : n_dve + hi],
                                 in_=acc_act[:, lo:hi]).then_inc(out_sem, 16)

        @block.gpsimd
        def _(gpsimd):
            # Zero the accumulator tensors so a (never-observed, but cheap
            # to insure against) flush-before-drain race reads zeros - a
            # ~1e-4 relative error - instead of stale SBUF garbage.
            gpsimd.memset(acc_dve, 0.0)
            gpsimd.memset(acc_act, 0.0)
            gpsimd.sem_inc(boot_sem, 1)
            for t in sorted(gp_tiles):
                buf = ring_slot(t)
                mt = 3 * rs[t]
                gpsimd.wait_ge(dma_sems[t % n_bufs], 16 * (t // n_bufs + 1))
                gpsimd.tensor_tensor(
                    out=dsub_out(t),
                    in0=buf[:, 0:mt],
                    in1=buf[:, mt : 2 * mt],
                    op=mybir.AluOpType.subtract,
                ).then_inc(gsub_sem, 1)

    nc.compile()
    assert len(nc._v4_plan_dve) == n_dve
    nc._v4_plan = (nc._v4_plan_dve, plan_act)
    return nc


def build_diag_kernel_v5(n_rows: int, bulk_r: int = 512, n_bufs: int = 14,
                         n_dbufs: int = 8,
                         tail_rs: tuple = (256, 128, 64, 32, 32),
                         head_queues: tuple = ("gpsimd", "scalar"),
                         strip_barriers: bool = True,
                         skip_exit_barrier: bool = True):
    """v5: diagonal-sigma fast path — per-tile sum of squared differences.

    For sigma = c*I (the shipped input), the loss needs only
    S = sum_b ||p_b - t_b||^2; no cross products. Per tile: DMA both
    halves -> DVE sub (fp32 in, bf16 interleaved out, unit-stride write)
    -> ACT Square with accum_out (one fp32 partial column per tile).
    Both engines run at ~50% of the DMA pace, so the kernel is purely
    DMA-bound: the 16-engine pool sustains ~415 GB/s.

    Output: partials [128, n_tiles]; host sums everything (f64).
    """
    R = n_rows // P
    assert n_rows % P == 0
    bulk_n = (R - sum(tail_rs)) // bulk_r
    assert bulk_n * bulk_r + sum(tail_rs) == R
    rs = [bulk_r] * bulk_n + list(tail_rs)
    n_tiles = len(rs)
    cum = [0]
    for r in rs:
        cum.append(cum[-1] + r)
    m_bulk = 3 * bulk_r
    f32, bf16 = mybir.dt.float32, mybir.dt.bfloat16

    nc = bacc.Bacc("TRN2", target_bir_lowering=False, debug=False)
    if strip_barriers:
        _strip_entry_barriers(nc)
    pt = nc.dram_tensor("pt", [2, n_rows, 3], f32, kind="ExternalInput")
    out = nc.dram_tensor("partials", [P, n_tiles], f32, kind="ExternalOutput")

    ring = nc.alloc_sbuf_tensor("ring", [P, n_bufs * 2 * m_bulk], f32).ap()
    dbuf = nc.alloc_sbuf_tensor("dbuf", [P, n_dbufs * m_bulk], bf16).ap()
    acc = nc.alloc_sbuf_tensor("acc", [P, n_tiles], f32).ap()
    sq_scr = nc.alloc_sbuf_tensor("sq_scr", [P, m_bulk], bf16).ap()

    dma_sems = [nc.alloc_semaphore(f"dma{i}") for i in range(n_bufs)]
    sub_sem = nc.alloc_semaphore("sub_sem")
    act_sem = nc.alloc_semaphore("act_sem")
    out_sem = nc.alloc_semaphore("out_sem")
    boot_sem = nc.alloc_semaphore("boot_sem")

    def ring_slot(t):
        s = t % n_bufs
        return ring[:, s * 2 * m_bulk : s * 2 * m_bulk + 2 * 3 * rs[t]]

    def d_slot(t):
        s = t % n_dbufs
        return dbuf[:, s * m_bulk : s * m_bulk + 3 * rs[t]]

    def pt_tile(t):
        sl = pt[:, cum[t] * P : cum[t + 1] * P, :]
        return sl.rearrange("w (p r) c -> p w (r c)", p=P)

    n_head = len(head_queues)

    def issue_load(q, t):
        q.dma_start(
            out=ring_slot(t).rearrange("p (w m) -> p w m", w=2),
            in_=pt_tile(t),
        ).then_inc(dma_sems[t % n_bufs], 16)

    # early-flush boundary: everything but the last 3 tiles
    flush0 = n_tiles - 3

    import contextlib

    @contextlib.contextmanager
    def _block():
        with nc.Block(no_gpsimd_drain=True) as blk:
            try:
                yield blk
            finally:
                if skip_exit_barrier:
                    nc.all_engine_barrier = lambda **kw: None
        if skip_exit_barrier:
            del nc.all_engine_barrier

    with _block() as block:

        @block.sync
        def _(sync):
            for t in range(n_head, n_tiles):
                if t >= n_bufs:
                    # ring slot free once its previous occupant was subbed
                    sync.wait_ge(sub_sem, t - n_bufs + 1)
                issue_load(sync, t)
            sync.wait_ge(out_sem, 32)

        @block.vector
        def _(vector):
            for t in range(n_tiles):
                vector.wait_ge(dma_sems[t % n_bufs], 16 * (t // n_bufs + 1))
                if t >= n_dbufs:
                    # d slot free once its previous occupant was squared
                    vector.wait_ge(act_sem, t - n_dbufs + 1)
                buf = ring_slot(t)
                mt = 3 * rs[t]
                vector.tensor_tensor(
                    out=d_slot(t),
                    in0=buf[:, 0:mt],
                    in1=buf[:, mt : 2 * mt],
                    op=mybir.AluOpType.subtract,
                ).then_inc(sub_sem, 1)

        @block.scalar
        def _(scalar):
            if "scalar" in head_queues:
                issue_load(scalar, head_queues.index("scalar"))
            scalar.wait_ge(boot_sem, 1)
            for t in range(n_tiles):
                scalar.wait_ge(sub_sem, t + 1)
                scalar.activation(
                    out=sq_scr[:, 0 : 3 * rs[t]],
                    in_=d_slot(t),
                    func=mybir.ActivationFunctionType.Square,
                    accum_out=acc[:, t : t + 1],
                ).then_inc(act_sem, 1)
                if t == flush0 - 1:
                    scalar.wait_ge(act_sem, flush0)
                    scalar.dma_start(
                        out=out[:, 0:flush0], in_=acc[:, 0:flush0]
                    ).then_inc(out_sem, 16)
            scalar.wait_ge(act_sem, n_tiles)
            scalar.dma_start(
                out=out[:, flush0:n_tiles], in_=acc[:, flush0:n_tiles]
            ).then_inc(out_sem, 16)

        @block.gpsimd
        def _(gpsimd):
            if "gpsimd" in head_queues:
                issue_load(gpsimd, head_queues.index("gpsimd"))
            # Zero acc so a flush-before-drain race reads zeros, not garbage.
            gpsimd.memset(acc, 0.0)
            gpsimd.sem_inc(boot_sem, 1)

    nc.compile()
    return nc


def gram_from_partials_v4(partials: np.ndarray, plan) -> np.ndarray:
    plan_dve, plan_act = plan
    s = partials.astype(np.float64).reshape(-1, partials.shape[-1]).sum(axis=0)
    g = np.zeros((3, 3), dtype=np.float64)
    for c, (i, j) in enumerate(plan_dve + plan_act):
        if i == j:
            g[i, i] += s[c]
        else:
            g[i, j] += s[c]
            g[j, i] += s[c]
    return g


def gram_from_partials_v3(partials: np.ndarray, n_tiles: int, group: int) -> np.ndarray:
    n_groups = n_tiles // group
    ncr = 3 * n_groups
    s = partials.astype(np.float64).reshape(-1, partials.shape[-1]).sum(axis=0)
    cr = s[:ncr].reshape(-1, 3).sum(axis=0)
    sq = s[ncr:].reshape(-1, 3).sum(axis=0)
    g = np.empty((3, 3), dtype=np.float64)
    g[0, 0], g[1, 1], g[2, 2] = sq
    for k, (i, j) in enumerate(_PAIRS):
        g[i, j] = g[j, i] = cr[k]
    return g


_NC_CACHE: dict[tuple, object] = {}


def _get_nc(n_rows: int, n_tiles: int, use_act: bool, raw: bool = False,
            group: int = 4, version: int = 4, n_bufs: int = 12,
            strip_barriers: bool = True, gp_stride: int = 0,
            bulk_r: int = 512, tail_rs: tuple = (256, 128, 64, 32, 32),
            head_rs: tuple = (128, 256, 256, 384), dpad: int = 3,
            head_dma_on_vector: bool = True):
    key = (n_rows, n_tiles, use_act, raw, group, version, n_bufs,
           strip_barriers, gp_stride, bulk_r, tail_rs, head_rs, dpad,
           head_dma_on_vector)
    if key not in _NC_CACHE:
        if version == 5:
            _NC_CACHE[key] = build_diag_kernel_v5(
                n_rows, bulk_r=bulk_r, n_bufs=n_bufs, tail_rs=tail_rs,
                strip_barriers=strip_barriers)
        elif version == 4:
            _NC_CACHE[key] = build_gram_kernel_v4(
                n_rows, bulk_r=bulk_r, n_bufs=n_bufs, group=group,
                gp_stride=gp_stride, head_rs=head_rs, tail_rs=tail_rs,
                dpad=dpad, head_dma_on_vector=head_dma_on_vector,
                strip_barriers=strip_barriers)
        elif version == 3:
            _NC_CACHE[key] = build_gram_kernel_v3(
                n_rows, n_tiles, n_bufs=n_bufs, group=group,
                strip_barriers=strip_barriers)
        elif raw:
            _NC_CACHE[key] = build_gram_kernel_raw(n_rows, n_tiles, group=group)
        else:
            _NC_CACHE[key] = build_gram_kernel(n_rows, n_tiles, use_act)
    return _NC_CACHE[key]


def gram_from_partials(partials: np.ndarray, n_tiles: int | None = None) -> np.ndarray:
    """[..., 128, 6*slots] partials -> full 3x3 Gram matrix (float64)."""
    slots = partials.shape[-1] // 6
    s = partials.astype(np.float64).reshape(-1, 6 * slots).sum(axis=0)
    sq = s[: 3 * slots].reshape(slots, 3).sum(axis=0)
    cr = s[3 * slots :].reshape(slots, 3).sum(axis=0)
    g = np.empty((3, 3), dtype=np.float64)
    g[0, 0], g[1, 1], g[2, 2] = sq
    for k, (i, j) in enumerate(_PAIRS):
        g[i, j] = g[j, i] = cr[k]
    return g


def run_device_partials(predictions: np.ndarray, targets: np.ndarray,
                        n_tiles: int = 4, use_act: bool = True,
                        raw: bool = False, group: int = 4, version: int = 4,
                        n_bufs: int = 12, strip_barriers: bool = True,
                        gp_stride: int = 0, bulk_r: int = 512,
                        tail_rs: tuple = (256, 128, 64, 32, 32),
                        head_rs: tuple = (128, 256, 256, 384), dpad: int = 3,
                        head_dma_on_vector: bool = True,
                        **run_kwargs):
    """Shard over N_CORES, run on device, return per-core partials + results."""
    b = predictions.shape[0]
    assert b % N_CORES == 0
    n_rows = b // N_CORES
    nc = _get_nc(n_rows, n_tiles, use_act, raw, group, version, n_bufs,
                 strip_barriers, gp_stride, bulk_r, tail_rs, head_rs, dpad,
                 head_dma_on_vector)
    preds = np.ascontiguousarray(predictions, dtype=np.float32).reshape(
        N_CORES, n_rows, 3
    )
    targs = np.ascontiguousarray(targets, dtype=np.float32).reshape(
        N_CORES, n_rows, 3
    )
    in_maps = [
        {"pt": np.stack([preds[c], targs[c]])} for c in range(N_CORES)
    ]
    res = run_bass_kernel_spmd(nc, in_maps, list(range(N_CORES)), **run_kwargs)
    partials = np.stack([r["partials"] for r in res.results])
    return partials, res, nc


def _host_loss(predictions, targets, sigma_inv, logdet, lo=0, hi=None):
    """Exact (float64) loss over rows [lo, hi) on the host, chunked."""
    hi = predictions.shape[0] if hi is None else hi
    tot = 0.0
    for s in range(lo, hi, 1 << 20):
        e = min(s + (1 << 20), hi)
        d = predictions[s:e].astype(np.float64) - targets[s:e].astype(np.float64)
        tot += float(np.einsum("bi,ij,bj->", d, sigma_inv, d))
    return abs(logdet + tot / (hi - lo))


def _sigma_inv_is_scalar(sigma_inv: np.ndarray) -> bool:
    """True iff sigma_inv == c*I to fp64 precision (the shipped input)."""
    d = np.diag(sigma_inv)
    off = sigma_inv - np.diag(d)
    tol = 1e-9 * float(np.abs(d).min())
    return (float(np.abs(off).max()) <= tol
            and float(np.abs(d - d[0]).max()) <= 1e-9 * abs(float(d[0])))


def kernel(predictions: np.ndarray, targets: np.ndarray, sigma: np.ndarray) -> np.ndarray:
    predictions = np.asarray(predictions, dtype=np.float32)
    targets = np.asarray(targets, dtype=np.float32)
    sigma64 = np.asarray(sigma, dtype=np.float64)
    sigma_inv = np.linalg.inv(sigma64)
    _, logdet = np.linalg.slogdet(sigma64)

    # Cheap subsample estimate (~0.3% rel) to sanity-gate the device result.
    est = _host_loss(predictions, targets, sigma_inv, logdet,
                     0, min(1 << 16, predictions.shape[0]))

    use_v5 = _sigma_inv_is_scalar(sigma_inv)
    loss = None
    for _attempt in range(2):
        if use_v5:
            partials, _, _ = run_device_partials(predictions, targets, version=5)
            s = float(partials.astype(np.float64).sum())
            mean_mahal = float(sigma_inv[0, 0]) * s / predictions.shape[0]
        else:
            partials, _, nc = run_device_partials(predictions, targets, version=4)
            g = gram_from_partials_v4(partials, nc._v4_plan)
            mean_mahal = float((sigma_inv * g).sum()) / predictions.shape[0]
        loss = abs(logdet + mean_mahal)
        if np.isfinite(loss) and abs(loss - est) <= 0.05 * max(abs(est), 1e-9):
            return np.float32(loss)
    # Device result failed the sanity gate twice: fall back to exact host.
    return np.float32(_host_loss(predictions, targets, sigma_inv, logdet))



# revision 13
# speedup vs baseline: 1.2181x; 1.0440x over previous
"""Trainium2 Bass kernel for CustomLossWithCovariance.

loss = abs(logdet(sigma) + mean_b[(p_b - t_b)^T sigma^{-1} (p_b - t_b)])

Only the 3x3 Gram matrix G = sum_b d_b d_b^T (d = pred - targ) requires
touching the [B, 3] data; the device computes per-core partial pair-sums
of G, and the host finishes with the tiny 3x3 algebra:
    mean_mahalanobis = <sigma_inv, G> / B
    loss = |logdet(sigma) + mean_mahalanobis|

Sharding: data-parallel over the batch across 8 NeuronCores (each core
streams a contiguous [B/8, 3] shard; partial sums gathered on host).

Production path: build_gram_kernel_v4 (raw Bacc, manual semaphores).
Per tile: one dma_start brings pred|targ halves; DVE subtracts into a
row-interleaved bf16 d buffer (unit-stride write — scatter writes are
4.6x slower on DVE); DVE fused multiply-reduces (stride-3 component
reads, grouped across tiles) produce the cross sums and ACT Square
accumulate produces the diagonals. Tile sizes ramp up at the head (so
DVE starts ~5us earlier) and shrink at the tail (so the post-last-DMA
dependency chain is short). The fp32 ring slot is freed by the sub
alone, letting the DMA stream run n_bufs tiles ahead. Bass's two
__init__ all-engine barriers are stripped (saves ~1.5us; the one real
dependency — gpsimd const memsets before ACT bias reads — is re-fenced
with boot_sem). Accumulator tensors are pre-zeroed so any flush race
degrades to ~1e-4 error instead of garbage, and kernel() additionally
sanity-gates the device result against a host subsample estimate with
retry + exact-host fallback.

Older variants (build_gram_kernel, build_gram_kernel_raw,
build_gram_kernel_v3) are kept for reference only.
"""

import numpy as np

import concourse.bass as bass
import concourse.bacc as bacc
import concourse.mybir as mybir
from concourse import tile
from concourse.bass_utils import run_bass_kernel_spmd

N_CORES = 8
B_FULL = 8388608
P = 128

_PAIRS = [(0, 1), (0, 2), (1, 2)]


def build_gram_kernel(n_rows: int, n_tiles: int, use_act: bool = True):
    """Build the per-core Bass module.

    Input: pt [2, n_rows, 3] f32 (pred stacked with targ)
    Output: partials [128, 6 * n_tiles] f32
        col t*3+i            : sum over this tile/partition of d_i^2
        col 3*n_tiles + t*3+k: sum of d_i*d_j for pair k in _PAIRS
    """
    assert n_rows % (P * n_tiles) == 0
    r = n_rows // (P * n_tiles)  # rows per partition per tile
    m = 3 * r                    # flat f32 elements per partition per tile
    f32 = mybir.dt.float32

    # Bacc (not plain Bass): its compile() pass legalizes semaphore waits
    # (each TRN2 instruction holds at most one wait slot).
    nc = bacc.Bacc("TRN2", target_bir_lowering=False, debug=False)
    pt = nc.dram_tensor("pt", [2, n_rows, 3], f32, kind="ExternalInput")
    out = nc.dram_tensor("partials", [P, 6 * n_tiles], f32, kind="ExternalOutput")

    # [t][p][w(2), m] — per tile/partition: pred chunk and targ chunk, each
    # m contiguous f32 in DRAM.
    pt_v = pt[:].rearrange("w (t p r) c -> t p w (r c)", t=n_tiles, p=P)

    with tile.TileContext(nc) as tc:
        with (
            tc.tile_pool(name="io", bufs=3) as io_pool,
            tc.tile_pool(name="dve_scr", bufs=2) as dve_scr,
            tc.tile_pool(name="act_scr", bufs=2) as act_scr,
            tc.tile_pool(name="acc", bufs=1) as acc_pool,
        ):
            acc_sq = acc_pool.tile([P, 3 * n_tiles], f32)
            acc_cr = acc_pool.tile([P, 3 * n_tiles], f32)

            for t in range(n_tiles):
                buf = io_pool.tile([P, 2 * m], f32, tag="buf")
                nc.sync.dma_start(
                    out=buf[:].rearrange("p (w m) -> p w m", w=2),
                    in_=pt_v[t],
                )

                # In-place: d = pred - targ, overwriting the pred half.
                nc.vector.tensor_tensor(
                    out=buf[:, 0:m],
                    in0=buf[:, 0:m],
                    in1=buf[:, m : 2 * m],
                    op=mybir.AluOpType.subtract,
                )
                d3 = buf[:, 0:m].rearrange("p (r c) -> p c r", c=3)

                # Diagonal sums on the scalar engine (Square + accum_out),
                # overlapping with the DVE cross-products.
                if use_act:
                    for i in range(3):
                        sq = act_scr.tile([P, r], f32, tag="sq")
                        nc.scalar.activation(
                            out=sq[:],
                            in_=d3[:, i, :],
                            func=mybir.ActivationFunctionType.Square,
                            accum_out=acc_sq[:, t * 3 + i : t * 3 + i + 1],
                        )
                else:
                    for i in range(3):
                        sq = dve_scr.tile([P, r], f32, tag="pr")
                        nc.vector.scalar_tensor_tensor(
                            out=sq[:],
                            in0=d3[:, i, :],
                            scalar=1.0,
                            in1=d3[:, i, :],
                            op0=mybir.AluOpType.mult,
                            op1=mybir.AluOpType.mult,
                            accum_out=acc_sq[:, t * 3 + i : t * 3 + i + 1],
                        )
                # Cross sums: fused multiply+reduce on DVE
                # (scalar_tensor_tensor: out = (in0 * 1.0) * in1, accum = sum).
                for k, (i, j) in enumerate(_PAIRS):
                    pr = dve_scr.tile([P, r], f32, tag="pr")
                    nc.vector.scalar_tensor_tensor(
                        out=pr[:],
                        in0=d3[:, i, :],
                        scalar=1.0,
                        in1=d3[:, j, :],
                        op0=mybir.AluOpType.mult,
                        op1=mybir.AluOpType.mult,
                        accum_out=acc_cr[:, t * 3 + k : t * 3 + k + 1],
                    )

            nc.sync.dma_start(out=out[:, 0 : 3 * n_tiles], in_=acc_sq[:])
            nc.sync.dma_start(out=out[:, 3 * n_tiles : 6 * n_tiles], in_=acc_cr[:])

    nc.compile()
    return nc


def build_gram_kernel_raw(n_rows: int, n_tiles: int = 32, n_bufs: int = 24,
                          group: int = 4, skip_exit_barrier: bool = True):
    """Raw-Bacc variant: manual semaphores, no TileContext.

    Skips Tile's prologue/epilogue (drain + two all-engine EVSEM
    barriers, ~16 us) — the only sync needed is a three-semaphore chain:
    DMA loads (one HWDGE ring) -> DVE -> ACT.

    The ring of tile buffers lives in ONE SBUF tensor so the fused
    multiply-reduces can span `group` consecutive tiles with a single
    instruction (free-dim AP [group, r]) — amortizing the per-op fixed
    cost and the accumulator-drain, which keeps both compute engines
    well under the DMA pace.

    Input: pt [2, n_rows, 3] f32. Output: partials [128, 6 * n_groups]
    (same slot layout as build_gram_kernel, with n_groups slots).
    """
    assert n_tiles % group == 0 and n_bufs % group == 0
    assert n_rows % (P * n_tiles) == 0
    n_groups = n_tiles // group
    r = n_rows // (P * n_tiles)
    m = 3 * r
    f32 = mybir.dt.float32

    nc = bacc.Bacc("TRN2", target_bir_lowering=False, debug=False)
    pt = nc.dram_tensor("pt", [2, n_rows, 3], f32, kind="ExternalInput")
    out = nc.dram_tensor("partials", [P, 6 * n_groups], f32, kind="ExternalOutput")
    pt_v = pt[:].rearrange("w (t p r) c -> t p w (r c)", t=n_tiles, p=P)

    ring = nc.alloc_sbuf_tensor("ring", [P, n_bufs * 2 * m], f32).ap()

    def buf(t):
        s = t % n_bufs
        return ring[:, s * 2 * m : (s + 1) * 2 * m]

    def dgroup(g, i):
        # component i of the diff halves of tiles 4g..4g+3: [128, group, r]
        s0 = (g * group) % n_bufs
        w = ring[:, s0 * 2 * m : (s0 + group) * 2 * m]
        return w.rearrange("p (t w r c) -> p t w c r", t=group, w=2, c=3)[:, :, 0, i, :]

    acc_sq = nc.alloc_sbuf_tensor("acc_sq", [P, 3 * n_groups], f32).ap()
    acc_cr = nc.alloc_sbuf_tensor("acc_cr", [P, 3 * n_groups], f32).ap()
    # Rotated scratch (dead stores of the fused ops), 2 groups deep so each
    # group's single stale semaphore wait also covers the scratch WAW from
    # two groups back.
    pr_scrs = [
        nc.alloc_sbuf_tensor(f"pr_scr{k}", [P, group * r], f32).ap() for k in range(6)
    ]
    sq_scrs = [
        nc.alloc_sbuf_tensor(f"sq_scr{k}", [P, group * r], f32).ap() for k in range(6)
    ]

    # One DMA-completion semaphore per ring buffer: a single shared sem
    # would be unsound — each dma_start is split across 16 SDMA engines
    # whose sub-completions interleave across in-flight DMAs.
    dma_sems = [nc.alloc_semaphore(f"dma_sem{i}") for i in range(n_bufs)]
    out_sem = nc.alloc_semaphore("out_sem")
    dve_sem = nc.alloc_semaphore("dve_sem")
    act_sem = nc.alloc_semaphore("act_sem")

    # DVE emission order: subs run ahead; the grouped multiply-reduces for
    # group g are emitted after sub(4g+4) so their drain-wait on the last
    # sub of the group is already satisfied when it executes (DVE writes
    # drain asynchronously). Only the last group trails the final sub.
    dve_order = []
    for t in range(n_tiles):
        dve_order.append(("sub", t))
        if t % group == 0 and t >= group:
            # one sub of stagger after the group's last sub
            dve_order.append(("stt", t // group - 1))
    dve_order.append(("stt", n_groups - 1))
    sub_done, sttg_done = {}, {}
    v = 0
    for kind, x in dve_order:
        if kind == "sub":
            v += 1
            sub_done[x] = v
        else:
            v += 3
            sttg_done[x] = v

    # Output chunks: flush finished accumulator columns while later tiles
    # still stream, so the tail only waits on the last small chunk.
    chunk = max(1, n_groups // 2)
    chunks = [(c, min(c + chunk, n_groups)) for c in range(0, n_groups, chunk)]

    import contextlib

    @contextlib.contextmanager
    def _block():
        # no_gpsimd_drain=True emits per-engine drains explicitly and then a
        # sem-only all-engine butterfly. The butterfly only delays NEFF end
        # (outputs are already fenced by the sequencer's out_sem wait), so
        # optionally no-op it during Block.__exit__.
        with nc.Block(no_gpsimd_drain=True) as blk:
            try:
                yield blk
            finally:
                if skip_exit_barrier:
                    nc.all_engine_barrier = lambda **kw: None
        if skip_exit_barrier:
            del nc.all_engine_barrier  # restore class method

    with _block() as block:

        @block.sync
        def _(sync):
            for t in range(n_tiles):
                if head_dma_on_vector and t < h0:
                    continue  # issued from the vector queue (clears the
                              # NEFF entry barrier ~1.4us before sync)
                if t >= n_bufs:
                    # ring reuse: all consumers of the buffer's previous
                    # occupant (tile t - n_bufs) must be done
                    prev = t - n_bufs
                    sync.wait_ge(dve_sem, sttg_done[prev // group])
                    sync.wait_ge(act_sem, 3 * (prev // group + 1))
                sync.dma_start(
                    out=buf(t).rearrange("p (w m) -> p w m", w=2),
                    in_=pt_v[t],
                ).then_inc(dma_sems[t % n_bufs], 16)
            n_out = 0
            for lo, hi in chunks:
                sync.wait_ge(act_sem, 3 * hi)
                sync.dma_start(
                    out=out[:, 3 * lo : 3 * hi], in_=acc_sq[:, 3 * lo : 3 * hi]
                ).then_inc(out_sem, 16)
                sync.wait_ge(dve_sem, sttg_done[hi - 1])
                sync.dma_start(
                    out=out[:, 3 * (n_groups + lo) : 3 * (n_groups + hi)],
                    in_=acc_cr[:, 3 * lo : 3 * hi],
                ).then_inc(out_sem, 16)
                n_out += 32
            sync.wait_ge(out_sem, n_out)

        @block.vector
        def _(vector):
            for kind, x in dve_order:
                if kind == "sub":
                    b = buf(x)
                    vector.wait_ge(dma_sems[x % n_bufs], 16 * (x // n_bufs + 1))
                    vector.tensor_tensor(
                        out=b[:, 0:m],
                        in0=b[:, 0:m],
                        in1=b[:, m : 2 * m],
                        op=mybir.AluOpType.subtract,
                    ).then_inc(dve_sem, 1)
                else:
                    vector.wait_ge(dve_sem, sub_done[(x + 1) * group - 1])
                    for k, (i, j) in enumerate(_PAIRS):
                        vector.scalar_tensor_tensor(
                            out=pr_scrs[(x % 2) * 3 + k][:].rearrange(
                                "p (t r) -> p t r", t=group
                            ),
                            in0=dgroup(x, i),
                            scalar=1.0,
                            in1=dgroup(x, j),
                            op0=mybir.AluOpType.mult,
                            op1=mybir.AluOpType.mult,
                            accum_out=acc_cr[:, x * 3 + k : x * 3 + k + 1],
                        ).then_inc(dve_sem, 1)

        @block.scalar
        def _(scalar):
            for g in range(n_groups):
                scalar.wait_ge(dve_sem, sub_done[(g + 1) * group - 1])
                if g >= 2:
                    # scratch slot reuse from two groups back
                    scalar.wait_ge(act_sem, 3 * (g - 1))
                for i in range(3):
                    scalar.activation(
                        out=sq_scrs[(g % 2) * 3 + i][:].rearrange(
                            "p (t r) -> p t r", t=group
                        ),
                        in_=dgroup(g, i),
                        func=mybir.ActivationFunctionType.Square,
                        accum_out=acc_sq[:, g * 3 + i : g * 3 + i + 1],
                    ).then_inc(act_sem, 1)

    nc.compile()
    return nc

def _strip_entry_barriers(nc):
    """Remove the two all-engine entry barriers Bass.__init__ emits.

    They serialize ~4us of semaphore round-trips before the first DMA can
    issue. The only cross-engine ordering they provide that this kernel
    needs is gpsimd-const-AP-memset -> ACT-bias-read, which is re-fenced
    explicitly with boot_sem in build_gram_kernel_v3.
    """
    bar = set(nc.barrier_sems)
    blk = nc.main_func.blocks[0]
    drop = []
    for ins in blk.instructions:
        si = getattr(ins, "sync_info", None)
        if si is None:
            continue
        sems = {w.id for w in si.on_wait or []}
        sems |= {u.id for u in si.on_update or []}
        if sems & bar:
            drop.append(ins)
    for ins in drop:
        blk.instructions.remove(ins)
    return len(drop)


def build_gram_kernel_v3(n_rows: int, n_tiles: int = 16, n_bufs: int = 8,
                         group: int = 4, strip_barriers: bool = True,
                         skip_exit_barrier: bool = True):
    """v3: planar-bf16 d + 2x DVE reduces + ACT squares.

    Per tile: DMA both halves -> DVE sub (fp32 in, planar bf16 out:
    component planes x|y|z so reduce operands are unit-stride 2-byte,
    unlocking the DVE 2x perf mode) -> DVE cross-product reduces (grouped
    `group` tiles per instr) + ACT Square reduces (grouped; per-tile for
    the last group so the post-DMA tail stays short).

    The fp32 ring slot is freed by the sub alone (d lives in its own
    full-size buffer), so the DMA stream runs ~n_bufs tiles ahead of
    compute and never stalls on the reduce bursts.

    Output layout [128, 3*n_groups + 3*(n_groups-1) + 3*group]:
      cols 0 .. 3*n_groups-1: cross sums (group g, pair k at 3g+k)
      then squares: full groups 0..n_groups-2 (3 each), then the last
      group's tiles individually (3 each).
    """
    assert n_rows % (P * n_tiles) == 0 and n_tiles % group == 0
    r = n_rows // (P * n_tiles)
    m = 3 * r
    n_groups = n_tiles // group
    full_sq = n_groups - 1           # square-groups emitted grouped
    tail0 = full_sq * group          # first per-tile-squares tile
    ncr = 3 * n_groups
    nsq = 3 * full_sq + 3 * group
    f32, bf16 = mybir.dt.float32, mybir.dt.bfloat16

    nc = bacc.Bacc("TRN2", target_bir_lowering=False, debug=False)
    if strip_barriers:
        _strip_entry_barriers(nc)
    pt = nc.dram_tensor("pt", [2, n_rows, 3], f32, kind="ExternalInput")
    out = nc.dram_tensor("partials", [P, ncr + nsq], f32, kind="ExternalOutput")
    pt_v = pt[:].rearrange("w (t p r) c -> t p w (r c)", t=n_tiles, p=P)

    ring = nc.alloc_sbuf_tensor("ring", [P, n_bufs * 2 * m], f32).ap()
    d_all = nc.alloc_sbuf_tensor("d_all", [P, n_tiles * m], bf16).ap()
    d_t = d_all.rearrange("p (t c r) -> p t c r", t=n_tiles, c=3)
    acc_cr = nc.alloc_sbuf_tensor("acc_cr", [P, ncr], f32).ap()
    acc_sq = nc.alloc_sbuf_tensor("acc_sq", [P, nsq], f32).ap()
    # Dead stores of the fused reduces; single slot per engine (each
    # engine executes its own stream in order, so WAW is safe).
    cr_scr = nc.alloc_sbuf_tensor("cr_scr", [P, group * r], bf16).ap()
    sq_scr = nc.alloc_sbuf_tensor("sq_scr", [P, group * r], bf16).ap()

    dma_sems = [nc.alloc_semaphore(f"dma{i}") for i in range(n_bufs)]
    sub_sem = nc.alloc_semaphore("sub_sem")
    red_sem = nc.alloc_semaphore("red_sem")
    act_sem = nc.alloc_semaphore("act_sem")
    out_sem = nc.alloc_semaphore("out_sem")
    boot_sem = nc.alloc_semaphore("boot_sem")

    def dcomp(t0, nt, i):
        # component i of tiles t0..t0+nt-1: [128, nt, r] unit-stride bf16
        v = d_t[:, t0 : t0 + nt, i, :]
        return v

    import contextlib

    @contextlib.contextmanager
    def _block():
        with nc.Block(no_gpsimd_drain=True) as blk:
            try:
                yield blk
            finally:
                if skip_exit_barrier:
                    nc.all_engine_barrier = lambda **kw: None
        if skip_exit_barrier:
            del nc.all_engine_barrier  # restore class method

    with _block() as block:

        @block.gpsimd
        def _(gpsimd):
            # Const-AP memsets (ACT bias) are earlier in gpsimd's stream;
            # this inc publishes their completion to the scalar queue.
            gpsimd.sem_inc(boot_sem, 1)

        @block.sync
        def _(sync):
            for t in range(n_tiles):
                if head_dma_on_vector and t < h0:
                    continue  # issued from the vector queue (clears the
                              # NEFF entry barrier ~1.4us before sync)
                if t >= n_bufs:
                    # ring slot free once its previous occupant was subbed
                    sync.wait_ge(sub_sem, t - n_bufs + 1)
                sync.dma_start(
                    out=ring[:, (t % n_bufs) * 2 * m : (t % n_bufs + 1) * 2 * m]
                    .rearrange("p (w m) -> p w m", w=2),
                    in_=pt_v[t],
                ).then_inc(dma_sems[t % n_bufs], 16)
            # accumulator flush: big chunks early, last-group slivers at end
            sync.wait_ge(red_sem, 3 * (n_groups - 1))
            sync.dma_start(
                out=out[:, 0 : 3 * (n_groups - 1)],
                in_=acc_cr[:, 0 : 3 * (n_groups - 1)],
            ).then_inc(out_sem, 16)
            sync.wait_ge(act_sem, 3 * full_sq)
            sync.dma_start(
                out=out[:, ncr : ncr + 3 * full_sq],
                in_=acc_sq[:, 0 : 3 * full_sq],
            ).then_inc(out_sem, 16)
            sync.wait_ge(red_sem, 3 * n_groups)
            sync.dma_start(
                out=out[:, 3 * (n_groups - 1) : ncr],
                in_=acc_cr[:, 3 * (n_groups - 1) : ncr],
            ).then_inc(out_sem, 16)
            sync.wait_ge(act_sem, nsq)
            sync.dma_start(
                out=out[:, ncr + 3 * full_sq : ncr + nsq],
                in_=acc_sq[:, 3 * full_sq : nsq],
            ).then_inc(out_sem, 16)
            sync.wait_ge(out_sem, 64)

        @block.vector
        def _(vector):
            for t in range(n_tiles):
                s = t % n_bufs
                buf = ring[:, s * 2 * m : (s + 1) * 2 * m]
                vector.wait_ge(dma_sems[s], 16 * (t // n_bufs + 1))
                # d = pred - targ, downcast to bf16, scattered into
                # component planes (write AP [r, 3] w/ strides [1, r])
                vector.tensor_tensor(
                    out=d_all[:, t * m : (t + 1) * m].rearrange(
                        "p (c r) -> p r c", c=3
                    ),
                    in0=buf[:, 0:m],
                    in1=buf[:, m : 2 * m],
                    op=mybir.AluOpType.subtract,
                ).then_inc(sub_sem, 1)
                if t % group == group - 1:
                    g = t // group
                    for k, (i, j) in enumerate(_PAIRS):
                        vector.scalar_tensor_tensor(
                            out=cr_scr[:].rearrange("p (t r) -> p t r", t=group),
                            in0=dcomp(g * group, group, i),
                            scalar=1.0,
                            in1=dcomp(g * group, group, j),
                            op0=mybir.AluOpType.mult,
                            op1=mybir.AluOpType.mult,
                            accum_out=acc_cr[:, g * 3 + k : g * 3 + k + 1],
                        ).then_inc(red_sem, 1)

        @block.scalar
        def _(scalar):
            scalar.wait_ge(boot_sem, 1)
            for g in range(full_sq):
                scalar.wait_ge(sub_sem, group * (g + 1))
                for i in range(3):
                    scalar.activation(
                        out=sq_scr[:].rearrange("p (t r) -> p t r", t=group),
                        in_=dcomp(g * group, group, i),
                        func=mybir.ActivationFunctionType.Square,
                        accum_out=acc_sq[:, g * 3 + i : g * 3 + i + 1],
                    ).then_inc(act_sem, 1)
            for w, t in enumerate(range(tail0, n_tiles)):
                scalar.wait_ge(sub_sem, t + 1)
                for i in range(3):
                    c = 3 * full_sq + w * 3 + i
                    scalar.activation(
                        out=sq_scr[:, 0:r],
                        in_=dcomp(t, 1, i),
                        func=mybir.ActivationFunctionType.Square,
                        accum_out=acc_sq[:, c : c + 1],
                    ).then_inc(act_sem, 1)

    nc.compile()
    nc._v3_meta = (n_tiles, group)
    return nc


def build_gram_kernel_v4(n_rows: int, bulk_r: int = 512, n_bufs: int = 12,
                         group: int = 4, gp_stride: int = 0,
                         head_rs: tuple = (128, 256, 256, 384),
                         tail_rs: tuple = (256, 128, 64, 32, 32),
                         dpad: int = 3, head_dma_on_vector: bool = True,
                         strip_barriers: bool = True,
                         skip_exit_barrier: bool = True):
    """v4: interleaved-bf16 d, measured-cost engine mix, shrinking tail.

    Measured HW rates (ns per 128-wide column): DVE sub fp32->bf16 unit
    1.28; DVE stt reduce bf16 stride-3 1.32; ACT Square ~1.0-1.4 + 740
    fixed; GPSIMD sub ~3.5. Writes must be unit-stride (scatter = 4.6x);
    strided reads are cheap. So d stays row-interleaved bf16.

    - bulk tiles of r=bulk_r rows/partition; every gp_stride-th bulk tile's
      sub runs on GPSIMD to keep DVE under the DMA pace.
    - cross-products: DVE stt grouped over `group` consecutive bulk tiles.
    - squares: ACT, same grouping; tail tiles per-tile; r<=32 tails on DVE.
    - tail tiles shrink so the post-last-DMA dependency chain is tiny.
    """
    R = n_rows // P
    assert n_rows % P == 0
    bulk_n = (R - sum(head_rs) - sum(tail_rs)) // bulk_r
    assert sum(head_rs) + bulk_n * bulk_r + sum(tail_rs) == R
    rs = list(head_rs) + [bulk_r] * bulk_n + list(tail_rs)
    n_tiles = len(rs)
    h0 = len(head_rs)               # first bulk tile index
    t0_tail = h0 + bulk_n           # first tail tile index
    cum = [0]
    for r in rs:
        cum.append(cum[-1] + r)
    # bulk groups: chunks of `group` (absolute tile indices)
    groups = [list(range(s, min(s + group, t0_tail)))
              for s in range(h0, t0_tail, group)]
    group_last = {g[-1]: g for g in groups}
    # per-tile (ungrouped) reduce tiles: head + tail
    per_tile = set(range(0, h0)) | set(range(t0_tail, n_tiles))
    # every gp_stride-th bulk tile's sub runs on GPSIMD (0 = none)
    gp_tiles = (set(range(h0, t0_tail, gp_stride)) if gp_stride else set())
    f32, bf16 = mybir.dt.float32, mybir.dt.bfloat16

    nc = bacc.Bacc("TRN2", target_bir_lowering=False, debug=False)
    if strip_barriers:
        _strip_entry_barriers(nc)
    pt = nc.dram_tensor("pt", [2, n_rows, 3], f32, kind="ExternalInput")

    # per-tile engine assignment of the sub + cumulative sem targets
    dve_idx, gp_idx = {}, {}
    for t in range(n_tiles):
        if t in gp_tiles:
            gp_idx[t] = len(gp_idx)
        else:
            dve_idx[t] = len(dve_idx)

    def sub_waits(last_t):
        """(sub_sem target, gsub_sem target) covering tiles 0..last_t."""
        d = sum(1 for t, i in dve_idx.items() if t <= last_t)
        g = sum(1 for t, i in gp_idx.items() if t <= last_t)
        return d, g

    # reduce slot counts (order finalized at emission)
    n_dve = 3 * len(groups) + 3 * len(per_tile) + 3 * sum(
        1 for t in per_tile if rs[t] <= 32)
    plan_act = []
    for t in sorted(per_tile):
        if t < h0 and rs[t] > 32:
            for i in range(3):
                plan_act.append((i, i))
    for g_tiles in groups:
        for i in range(3):
            plan_act.append((i, i))
    for t in sorted(per_tile):
        if t >= t0_tail and rs[t] > 32:
            for i in range(3):
                plan_act.append((i, i))
    n_act = len(plan_act)
    out = nc.dram_tensor("partials", [P, n_dve + n_act], f32,
                         kind="ExternalOutput")

    m_bulk = 3 * bulk_r
    ring = nc.alloc_sbuf_tensor("ring", [P, n_bufs * 2 * m_bulk], f32).ap()
    d_all = nc.alloc_sbuf_tensor("d_all", [P, dpad * R], bf16).ap()
    acc_dve = nc.alloc_sbuf_tensor("acc_dve", [P, n_dve], f32).ap()
    acc_act = nc.alloc_sbuf_tensor("acc_act", [P, n_act], f32).ap()
    cr_scr = nc.alloc_sbuf_tensor("cr_scr", [P, group * bulk_r], bf16).ap()
    sq_scr = nc.alloc_sbuf_tensor("sq_scr", [P, group * bulk_r], bf16).ap()

    dma_sems = [nc.alloc_semaphore(f"dma{i}") for i in range(n_bufs)]
    sub_sem = nc.alloc_semaphore("sub_sem")
    gsub_sem = nc.alloc_semaphore("gsub_sem")
    red_sem = nc.alloc_semaphore("red_sem")
    act_sem = nc.alloc_semaphore("act_sem")
    out_sem = nc.alloc_semaphore("out_sem")
    boot_sem = nc.alloc_semaphore("boot_sem")

    def ring_slot(t):
        s = t % n_bufs
        return ring[:, s * 2 * m_bulk : s * 2 * m_bulk + 2 * 3 * rs[t]]

    def pt_tile(t):
        sl = pt[:, cum[t] * P : cum[t + 1] * P, :]
        return sl.rearrange("w (p r) c -> p w (r c)", p=P)

    def dseg(t0, nt, i):
        # component i of tiles t0..t0+nt-1 (equal r), stride-dpad reads
        v = d_all[:, dpad * cum[t0] : dpad * cum[t0 + nt]]
        return v.rearrange("p (t r c) -> p t c r", t=nt, c=dpad)[:, :, i, :]

    def dsub_out(t):
        # write view for the sub: rows of 3 packed comps, dpad-elem row pitch
        v = d_all[:, dpad * cum[t] : dpad * cum[t + 1]]
        if dpad == 3:
            return v
        return v.rearrange("p (r c) -> p r c", c=dpad)[:, :, 0:3]

    # early-flush boundaries: head reduces + all-but-last bulk group are
    # guaranteed emitted before any tail reduce; ACT head+bulk likewise
    red_early = 3 * h0 + 3 * (len(groups) - 1)
    act_bulk = 3 * h0 + 3 * len(groups)
    _red_pen = max(red_early, n_dve - 6)
    _act_pen = max(act_bulk, n_act - 3)
    n_flush_dve = sum(1 for lo, hi in ((0, red_early), (red_early, _red_pen),
                                       (_red_pen, n_dve)) if hi > lo)
    act_chunks = [(act_bulk, 0, act_bulk), (_act_pen, act_bulk, _act_pen),
                  (n_act, _act_pen, n_act)]
    act_chunks = [c for c in act_chunks if c[2] > c[1]]
    n_flush_act = len(act_chunks)

    import contextlib

    @contextlib.contextmanager
    def _block():
        with nc.Block(no_gpsimd_drain=True) as blk:
            try:
                yield blk
            finally:
                if skip_exit_barrier:
                    nc.all_engine_barrier = lambda **kw: None
        if skip_exit_barrier:
            del nc.all_engine_barrier

    with _block() as block:

        @block.sync
        def _(sync):
            for t in range(n_tiles):
                if head_dma_on_vector and t < h0:
                    continue  # issued from the vector queue (clears the
                              # NEFF entry barrier ~1.4us before sync)
                if t >= n_bufs:
                    u = t - n_bufs
                    if u in gp_idx:
                        sync.wait_ge(gsub_sem, gp_idx[u] + 1)
                    else:
                        sync.wait_ge(sub_sem, dve_idx[u] + 1)
                sync.dma_start(
                    out=ring_slot(t).rearrange("p (w m) -> p w m", w=2),
                    in_=pt_tile(t),
                ).then_inc(dma_sems[t % n_bufs], 16)
            # acc_dve flushes stay here; acc_act flushes issue from the
            # scalar queue (ACT finishes ~2.5us before DVE, so they then
            # complete during the DVE tail instead of serializing after it)
            red_pen = max(red_early, n_dve - 6)   # all but last tail tile
            for val, lo, hi in ((red_early, 0, red_early),
                                (red_pen, red_early, red_pen),
                                (n_dve, red_pen, n_dve)):
                if hi <= lo:
                    continue
                sync.wait_ge(red_sem, val)
                sync.dma_start(out=out[:, lo:hi],
                               in_=acc_dve[:, lo:hi]).then_inc(out_sem, 16)
            sync.wait_ge(out_sem, 16 * (n_flush_dve + n_flush_act))

        @block.vector
        def _(vector):
            red_c = 0
            plan_dyn = []
            pending = []  # deferred bulk-group reduces, drained 1-per-sub

            def emit_one(t0, nt, i, j):
                nonlocal red_c
                r = rs[t0]
                vector.scalar_tensor_tensor(
                    out=cr_scr[:, 0 : nt * r].rearrange(
                        "p (t r) -> p t r", t=nt),
                    in0=dseg(t0, nt, i),
                    scalar=1.0,
                    in1=dseg(t0, nt, j),
                    op0=mybir.AluOpType.mult,
                    op1=mybir.AluOpType.mult,
                    accum_out=acc_dve[:, red_c : red_c + 1],
                ).then_inc(red_sem, 1)
                plan_dyn.append((i, j))
                red_c += 1

            for t in range(n_tiles):
                if t not in gp_tiles:
                    buf = ring_slot(t)
                    mt = 3 * rs[t]
                    vector.wait_ge(dma_sems[t % n_bufs], 16 * (t // n_bufs + 1))
                    vector.tensor_tensor(
                        out=dsub_out(t),
                        in0=buf[:, 0:mt],
                        in1=buf[:, mt : 2 * mt],
                        op=mybir.AluOpType.subtract,
                    ).then_inc(sub_sem, 1)
                    # spread deferred group reduces between subs so the DMA
                    # ring (freed by subs) never stalls on a reduce burst
                    if pending:
                        emit_one(*pending.pop(0))
                if t in group_last:
                    g_tiles = group_last[t]
                    _, gw = sub_waits(g_tiles[-1])
                    if gw:
                        vector.wait_ge(gsub_sem, gw)
                    for (i, j) in _PAIRS:
                        pending.append((g_tiles[0], len(g_tiles), i, j))
                elif t in per_tile:
                    for (i, j) in _PAIRS:
                        emit_one(t, 1, i, j)
                    if rs[t] <= 32:
                        for i in range(3):
                            emit_one(t, 1, i, i)
            while pending:
                emit_one(*pending.pop(0))
            nc._v4_plan_dve = plan_dyn

        @block.scalar
        def _(scalar):
            if head_dma_on_vector:
                # Scalar clears the NEFF entry-barrier chain ~2us before
                # Sync; issuing the head-tile loads here starts the stream
                # (and so the first sub) earlier. Ring slots are fresh, no
                # waits needed; the const-AP fence only matters for the
                # activations below.
                for t in range(h0):
                    scalar.dma_start(
                        out=ring_slot(t).rearrange("p (w m) -> p w m", w=2),
                        in_=pt_tile(t),
                    ).then_inc(dma_sems[t % n_bufs], 16)
            scalar.wait_ge(boot_sem, 1)
            act_c = 0

            def emit_squares(t0, nt):
                nonlocal act_c
                r = rs[t0]
                for i in range(3):
                    scalar.activation(
                        out=sq_scr[:, 0 : nt * r].rearrange(
                            "p (t r) -> p t r", t=nt),
                        in_=dseg(t0, nt, i),
                        func=mybir.ActivationFunctionType.Square,
                        accum_out=acc_act[:, act_c : act_c + 1],
                    ).then_inc(act_sem, 1)
                    act_c += 1

            def sq_waits(last_t):
                d, g = sub_waits(last_t)
                scalar.wait_ge(sub_sem, d)
                if g:
                    scalar.wait_ge(gsub_sem, g)

            for t in sorted(per_tile):
                if t < h0 and rs[t] > 32:
                    sq_waits(t)
                    emit_squares(t, 1)
            for g_tiles in groups:
                sq_waits(g_tiles[-1])
                emit_squares(g_tiles[0], len(g_tiles))
            for t in sorted(per_tile):
                if t >= t0_tail and rs[t] > 32:
                    sq_waits(t)
                    emit_squares(t, 1)
            for val, lo, hi in act_chunks:
                scalar.wait_ge(act_sem, val)
                scalar.dma_start(out=out[:, n_dve + lo : n_dve + hi],
                                 in_=acc_act[:, lo:hi]).then_inc(out_sem, 16)

        @block.gpsimd
        def _(gpsimd):
            # Zero the accumulator tensors so a (never-observed, but cheap
            # to insure against) flush-before-drain race reads zeros - a
            # ~1e-4 relative error - instead of stale SBUF garbage.
            gpsimd.memset(acc_dve, 0.0)
            gpsimd.memset(acc_act, 0.0)
            gpsimd.sem_inc(boot_sem, 1)
            for t in sorted(gp_tiles):
                buf = ring_slot(t)
                mt = 3 * rs[t]
                gpsimd.wait_ge(dma_sems[t % n_bufs], 16 * (t // n_bufs + 1))
                gpsimd.tensor_tensor(
                    out=dsub_out(t),
                    in0=buf[:, 0:mt],
                    in1=buf[:, mt : 2 * mt],
                    op=mybir.AluOpType.subtract,
                ).then_inc(gsub_sem, 1)

    nc.compile()
    assert len(nc._v4_plan_dve) == n_dve
    nc._v4_plan = (nc._v4_plan_dve, plan_act)
    return nc


def build_diag_kernel_v5(n_rows: int, bulk_r: int = 512, n_bufs: int = 14,
                         n_dbufs: int = 8,
                         head_rs: tuple = (128, 128, 256),
                         tail_rs: tuple = (256, 256, 192, 128, 64, 64, 32, 32),
                         n_sq_dve: int = 4,
                         strip_barriers: bool = True,
                         skip_exit_barrier: bool = True):
    """v5: diagonal-sigma fast path — per-tile sum of squared differences.

    For sigma = c*I (the shipped input), the loss needs only
    S = sum_b ||p_b - t_b||^2; no cross products. Per tile: DMA both
    halves -> DVE sub (fp32 in, bf16 interleaved out, unit-stride write)
    -> Square with accum_out (one fp32 partial column per tile), on ACT
    for most tiles. Both engines run at ~50% of the DMA pace, so the
    kernel is purely DMA-bound: the 16-engine pool sustains ~415 GB/s.

    Head: small ramp tiles issued from the gpsimd/scalar queues (they
    clear the NEFF boot chain before sync) — small, so the brief
    3-queue pool contention costs little. Tail: ramp-down sizes, with
    the squares of the last 2*n_sq_dve tiles alternating DVE/ACT so the
    post-last-load catch-up runs on both engines in parallel.

    Output: partials [128, n_tiles] (ACT tiles then DVE tiles, by the
    _v5_order attr); host sums everything (f64).
    """
    R = n_rows // P
    assert n_rows % P == 0
    bulk_n = (R - sum(head_rs) - sum(tail_rs)) // bulk_r
    assert sum(head_rs) + bulk_n * bulk_r + sum(tail_rs) == R
    rs = list(head_rs) + [bulk_r] * bulk_n + list(tail_rs)
    n_tiles = len(rs)
    cum = [0]
    for r in rs:
        cum.append(cum[-1] + r)
    m_bulk = 3 * max(rs)
    f32, bf16 = mybir.dt.float32, mybir.dt.bfloat16

    # squares of the last 2*n_sq_dve tiles alternate DVE/ACT (DVE takes
    # the even offsets from the end: ..., t-4, t-2, last)
    sq_dve = {n_tiles - 1 - 2 * k for k in range(n_sq_dve)}
    act_tiles = [t for t in range(n_tiles) if t not in sq_dve]
    dve_tiles = sorted(sq_dve)
    n_act, n_dve = len(act_tiles), len(dve_tiles)
    acol = {t: i for i, t in enumerate(act_tiles)}
    vcol = {t: i for i, t in enumerate(dve_tiles)}
    # act_sem value after the square of tile u (ACT tiles only)
    act_done = {t: i + 1 for i, t in enumerate(act_tiles)}
    # sub_sem value after DVE finished tile t (sub, plus square if DVE tile)
    sub_done = {}
    _v = 0
    for _t in range(n_tiles):
        _v += 2 if _t in sq_dve else 1
        sub_done[_t] = _v
    n_flush = (1 if n_act > 1 else 0) + 1 + (1 if n_dve else 0)

    nc = bacc.Bacc("TRN2", target_bir_lowering=False, debug=False)
    if strip_barriers:
        _strip_entry_barriers(nc)
    pt = nc.dram_tensor("pt", [2, n_rows, 3], f32, kind="ExternalInput")
    out = nc.dram_tensor("partials", [P, n_tiles], f32, kind="ExternalOutput")

    ring = nc.alloc_sbuf_tensor("ring", [P, n_bufs * 2 * m_bulk], f32).ap()
    dbuf = nc.alloc_sbuf_tensor("dbuf", [P, n_dbufs * m_bulk], bf16).ap()
    acc_a = nc.alloc_sbuf_tensor("acc_a", [P, max(n_act, 1)], f32).ap()
    acc_v = nc.alloc_sbuf_tensor("acc_v", [P, max(n_dve, 1)], f32).ap()
    sq_scr = nc.alloc_sbuf_tensor("sq_scr", [P, m_bulk], bf16).ap()
    vq_scr = nc.alloc_sbuf_tensor("vq_scr", [P, m_bulk], bf16).ap()

    dma_sems = [nc.alloc_semaphore(f"dma{i}") for i in range(n_bufs)]
    sub_sem = nc.alloc_semaphore("sub_sem")
    act_sem = nc.alloc_semaphore("act_sem")
    out_sem = nc.alloc_semaphore("out_sem")
    boot_sem = nc.alloc_semaphore("boot_sem")

    def ring_slot(t):
        s = t % n_bufs
        return ring[:, s * 2 * m_bulk : s * 2 * m_bulk + 2 * 3 * rs[t]]

    def d_slot(t):
        s = t % n_dbufs
        return dbuf[:, s * m_bulk : s * m_bulk + 3 * rs[t]]

    def pt_tile(t):
        sl = pt[:, cum[t] * P : cum[t + 1] * P, :]
        return sl.rearrange("w (p r) c -> p w (r c)", p=P)

    n_head = len(head_rs)

    def issue_load(q, t):
        q.dma_start(
            out=ring_slot(t).rearrange("p (w m) -> p w m", w=2),
            in_=pt_tile(t),
        ).then_inc(dma_sems[t % n_bufs], 16)

    import contextlib

    @contextlib.contextmanager
    def _block():
        with nc.Block(no_gpsimd_drain=True) as blk:
            try:
                yield blk
            finally:
                if skip_exit_barrier:
                    nc.all_engine_barrier = lambda **kw: None
        if skip_exit_barrier:
            del nc.all_engine_barrier

    with _block() as block:

        @block.sync
        def _(sync):
            for t in range(n_head, n_tiles):
                if t >= n_bufs:
                    # ring slot free once its previous occupant was subbed
                    u = t - n_bufs
                    sync.wait_ge(sub_sem, sub_done[u] - (1 if u in sq_dve else 0))
                issue_load(sync, t)
            sync.wait_ge(out_sem, 16 * n_flush)

        @block.vector
        def _(vector):
            for t in range(n_tiles):
                vector.wait_ge(dma_sems[t % n_bufs], 16 * (t // n_bufs + 1))
                u = t - n_dbufs
                if u >= 0 and u not in sq_dve:
                    # d slot free once its previous occupant was squared
                    vector.wait_ge(act_sem, act_done[u])
                buf = ring_slot(t)
                mt = 3 * rs[t]
                vector.tensor_tensor(
                    out=d_slot(t),
                    in0=buf[:, 0:mt],
                    in1=buf[:, mt : 2 * mt],
                    op=mybir.AluOpType.subtract,
                ).then_inc(sub_sem, 1)
                if t in sq_dve:
                    c = vcol[t]
                    vector.scalar_tensor_tensor(
                        out=vq_scr[:, 0 : 3 * rs[t]],
                        in0=d_slot(t),
                        scalar=1.0,
                        in1=d_slot(t),
                        op0=mybir.AluOpType.mult,
                        op1=mybir.AluOpType.mult,
                        accum_out=acc_v[:, c : c + 1],
                    ).then_inc(sub_sem, 1)

        @block.scalar
        def _(scalar):
            for t in range(n_head):
                if t > 0:
                    issue_load(scalar, t)
            scalar.wait_ge(boot_sem, 1)
            flush0 = act_tiles[-2] if n_act > 1 else None
            for t in act_tiles:
                scalar.wait_ge(sub_sem, sub_done[t])
                c = acol[t]
                scalar.activation(
                    out=sq_scr[:, 0 : 3 * rs[t]],
                    in_=d_slot(t),
                    func=mybir.ActivationFunctionType.Square,
                    accum_out=acc_a[:, c : c + 1],
                ).then_inc(act_sem, 1)
                if t == flush0:
                    scalar.wait_ge(act_sem, n_act - 1)
                    scalar.dma_start(
                        out=out[:, 0 : n_act - 1], in_=acc_a[:, 0 : n_act - 1]
                    ).then_inc(out_sem, 16)
            scalar.wait_ge(act_sem, n_act)
            lo = max(n_act - 2, 0) if flush0 is not None else 0
            scalar.dma_start(
                out=out[:, lo:n_act], in_=acc_a[:, lo:n_act]
            ).then_inc(out_sem, 16)
            if n_dve:
                # DVE squares: all done once the last DVE tile's pair ran
                scalar.wait_ge(sub_sem, sub_done[dve_tiles[-1]])
                scalar.dma_start(
                    out=out[:, n_act : n_act + n_dve], in_=acc_v[:, 0:n_dve]
                ).then_inc(out_sem, 16)

        @block.gpsimd
        def _(gpsimd):
            issue_load(gpsimd, 0)
            # Zero accs so a flush-before-drain race reads zeros, not garbage.
            gpsimd.memset(acc_a, 0.0)
            gpsimd.memset(acc_v, 0.0)
            gpsimd.sem_inc(boot_sem, 1)

    nc.compile()
    nc._v5_order = (act_tiles, dve_tiles)
    return nc


def gram_from_partials_v4(partials: np.ndarray, plan) -> np.ndarray:
    plan_dve, plan_act = plan
    s = partials.astype(np.float64).reshape(-1, partials.shape[-1]).sum(axis=0)
    g = np.zeros((3, 3), dtype=np.float64)
    for c, (i, j) in enumerate(plan_dve + plan_act):
        if i == j:
            g[i, i] += s[c]
        else:
            g[i, j] += s[c]
            g[j, i] += s[c]
    return g


def gram_from_partials_v3(partials: np.ndarray, n_tiles: int, group: int) -> np.ndarray:
    n_groups = n_tiles // group
    ncr = 3 * n_groups
    s = partials.astype(np.float64).reshape(-1, partials.shape[-1]).sum(axis=0)
    cr = s[:ncr].reshape(-1, 3).sum(axis=0)
    sq = s[ncr:].reshape(-1, 3).sum(axis=0)
    g = np.empty((3, 3), dtype=np.float64)
    g[0, 0], g[1, 1], g[2, 2] = sq
    for k, (i, j) in enumerate(_PAIRS):
        g[i, j] = g[j, i] = cr[k]
    return g


_NC_CACHE: dict[tuple, object] = {}


def _get_nc(n_rows: int, n_tiles: int, use_act: bool, raw: bool = False,
            group: int = 4, version: int = 4, n_bufs: int = 12,
            strip_barriers: bool = True, gp_stride: int = 0,
            bulk_r: int = 512, tail_rs: tuple = (256, 128, 64, 32, 32),
            head_rs: tuple = (128, 256, 256, 384), dpad: int = 3,
            head_dma_on_vector: bool = True):
    key = (n_rows, n_tiles, use_act, raw, group, version, n_bufs,
           strip_barriers, gp_stride, bulk_r, tail_rs, head_rs, dpad,
           head_dma_on_vector)
    if key not in _NC_CACHE:
        if version == 5:
            _NC_CACHE[key] = build_diag_kernel_v5(
                n_rows, strip_barriers=strip_barriers)
        elif version == 4:
            _NC_CACHE[key] = build_gram_kernel_v4(
                n_rows, bulk_r=bulk_r, n_bufs=n_bufs, group=group,
                gp_stride=gp_stride, head_rs=head_rs, tail_rs=tail_rs,
                dpad=dpad, head_dma_on_vector=head_dma_on_vector,
                strip_barriers=strip_barriers)
        elif version == 3:
            _NC_CACHE[key] = build_gram_kernel_v3(
                n_rows, n_tiles, n_bufs=n_bufs, group=group,
                strip_barriers=strip_barriers)
        elif raw:
            _NC_CACHE[key] = build_gram_kernel_raw(n_rows, n_tiles, group=group)
        else:
            _NC_CACHE[key] = build_gram_kernel(n_rows, n_tiles, use_act)
    return _NC_CACHE[key]


def gram_from_partials(partials: np.ndarray, n_tiles: int | None = None) -> np.ndarray:
    """[..., 128, 6*slots] partials -> full 3x3 Gram matrix (float64)."""
    slots = partials.shape[-1] // 6
    s = partials.astype(np.float64).reshape(-1, 6 * slots).sum(axis=0)
    sq = s[: 3 * slots].reshape(slots, 3).sum(axis=0)
    cr = s[3 * slots :].reshape(slots, 3).sum(axis=0)
    g = np.empty((3, 3), dtype=np.float64)
    g[0, 0], g[1, 1], g[2, 2] = sq
    for k, (i, j) in enumerate(_PAIRS):
        g[i, j] = g[j, i] = cr[k]
    return g


def run_device_partials(predictions: np.ndarray, targets: np.ndarray,
                        n_tiles: int = 4, use_act: bool = True,
                        raw: bool = False, group: int = 4, version: int = 4,
                        n_bufs: int = 12, strip_barriers: bool = True,
                        gp_stride: int = 0, bulk_r: int = 512,
                        tail_rs: tuple = (256, 128, 64, 32, 32),
                        head_rs: tuple = (128, 256, 256, 384), dpad: int = 3,
                        head_dma_on_vector: bool = True,
                        **run_kwargs):
    """Shard over N_CORES, run on device, return per-core partials + results."""
    b = predictions.shape[0]
    assert b % N_CORES == 0
    n_rows = b // N_CORES
    nc = _get_nc(n_rows, n_tiles, use_act, raw, group, version, n_bufs,
                 strip_barriers, gp_stride, bulk_r, tail_rs, head_rs, dpad,
                 head_dma_on_vector)
    preds = np.ascontiguousarray(predictions, dtype=np.float32).reshape(
        N_CORES, n_rows, 3
    )
    targs = np.ascontiguousarray(targets, dtype=np.float32).reshape(
        N_CORES, n_rows, 3
    )
    in_maps = [
        {"pt": np.stack([preds[c], targs[c]])} for c in range(N_CORES)
    ]
    res = run_bass_kernel_spmd(nc, in_maps, list(range(N_CORES)), **run_kwargs)
    partials = np.stack([r["partials"] for r in res.results])
    return partials, res, nc


def _host_loss(predictions, targets, sigma_inv, logdet, lo=0, hi=None):
    """Exact (float64) loss over rows [lo, hi) on the host, chunked."""
    hi = predictions.shape[0] if hi is None else hi
    tot = 0.0
    for s in range(lo, hi, 1 << 20):
        e = min(s + (1 << 20), hi)
        d = predictions[s:e].astype(np.float64) - targets[s:e].astype(np.float64)
        tot += float(np.einsum("bi,ij,bj->", d, sigma_inv, d))
    return abs(logdet + tot / (hi - lo))


def _sigma_inv_is_scalar(sigma_inv: np.ndarray) -> bool:
    """True iff sigma_inv == c*I to fp64 precision (the shipped input)."""
    d = np.diag(sigma_inv)
    off = sigma_inv - np.diag(d)
    tol = 1e-9 * float(np.abs(d).min())
    return (float(np.abs(off).max()) <= tol
            and float(np.abs(d - d[0]).max()) <= 1e-9 * abs(float(d[0])))


def kernel(predictions: np.ndarray, targets: np.ndarray, sigma: np.ndarray) -> np.ndarray:
    predictions = np.asarray(predictions, dtype=np.float32)
    targets = np.asarray(targets, dtype=np.float32)
    sigma64 = np.asarray(sigma, dtype=np.float64)
    sigma_inv = np.linalg.inv(sigma64)
    _, logdet = np.linalg.slogdet(sigma64)

    # Cheap subsample estimate (~0.3% rel) to sanity-gate the device result.
    est = _host_loss(predictions, targets, sigma_inv, logdet,
                     0, min(1 << 16, predictions.shape[0]))

    use_v5 = _sigma_inv_is_scalar(sigma_inv)
    loss = None
    for _attempt in range(2):
        if use_v5:
            partials, _, _ = run_device_partials(predictions, targets, version=5)
            s = float(partials.astype(np.float64).sum())
            mean_mahal = float(sigma_inv[0, 0]) * s / predictions.shape[0]
        else:
            partials, _, nc = run_device_partials(predictions, targets, version=4)
            g = gram_from_partials_v4(partials, nc._v4_plan)
            mean_mahal = float((sigma_inv * g).sum()) / predictions.shape[0]
        loss = abs(logdet + mean_mahal)
        if np.isfinite(loss) and abs(loss - est) <= 0.05 * max(abs(est), 1e-9):
            return np.float32(loss)
    # Device result failed the sanity gate twice: fall back to exact host.
    return np.float32(_host_loss(predictions, targets, sigma_inv, logdet))



# revision 17
# speedup vs baseline: 1.2350x; 1.0139x over previous
"""Trainium2 Bass kernel for CustomLossWithCovariance.

loss = abs(logdet(sigma) + mean_b[(p_b - t_b)^T sigma^{-1} (p_b - t_b)])

Only the 3x3 Gram matrix G = sum_b d_b d_b^T (d = pred - targ) requires
touching the [B, 3] data; the device computes per-core partial pair-sums
of G, and the host finishes with the tiny 3x3 algebra:
    mean_mahalanobis = <sigma_inv, G> / B
    loss = |logdet(sigma) + mean_mahalanobis|

Sharding: data-parallel over the batch across 8 NeuronCores (each core
streams a contiguous [B/8, 3] shard; partial sums gathered on host).

Production path: build_gram_kernel_v4 (raw Bacc, manual semaphores).
Per tile: one dma_start brings pred|targ halves; DVE subtracts into a
row-interleaved bf16 d buffer (unit-stride write — scatter writes are
4.6x slower on DVE); DVE fused multiply-reduces (stride-3 component
reads, grouped across tiles) produce the cross sums and ACT Square
accumulate produces the diagonals. Tile sizes ramp up at the head (so
DVE starts ~5us earlier) and shrink at the tail (so the post-last-DMA
dependency chain is short). The fp32 ring slot is freed by the sub
alone, letting the DMA stream run n_bufs tiles ahead. Bass's two
__init__ all-engine barriers are stripped (saves ~1.5us; the one real
dependency — gpsimd const memsets before ACT bias reads — is re-fenced
with boot_sem). Accumulator tensors are pre-zeroed so any flush race
degrades to ~1e-4 error instead of garbage, and kernel() additionally
sanity-gates the device result against a host subsample estimate with
retry + exact-host fallback.

Older variants (build_gram_kernel, build_gram_kernel_raw,
build_gram_kernel_v3) are kept for reference only.
"""

import numpy as np

import concourse.bass as bass
import concourse.bacc as bacc
import concourse.mybir as mybir
from concourse import tile
from concourse.bass_utils import run_bass_kernel_spmd

N_CORES = 8
B_FULL = 8388608
P = 128

_PAIRS = [(0, 1), (0, 2), (1, 2)]


def build_gram_kernel(n_rows: int, n_tiles: int, use_act: bool = True):
    """Build the per-core Bass module.

    Input: pt [2, n_rows, 3] f32 (pred stacked with targ)
    Output: partials [128, 6 * n_tiles] f32
        col t*3+i            : sum over this tile/partition of d_i^2
        col 3*n_tiles + t*3+k: sum of d_i*d_j for pair k in _PAIRS
    """
    assert n_rows % (P * n_tiles) == 0
    r = n_rows // (P * n_tiles)  # rows per partition per tile
    m = 3 * r                    # flat f32 elements per partition per tile
    f32 = mybir.dt.float32

    # Bacc (not plain Bass): its compile() pass legalizes semaphore waits
    # (each TRN2 instruction holds at most one wait slot).
    nc = bacc.Bacc("TRN2", target_bir_lowering=False, debug=False)
    pt = nc.dram_tensor("pt", [2, n_rows, 3], f32, kind="ExternalInput")
    out = nc.dram_tensor("partials", [P, 6 * n_tiles], f32, kind="ExternalOutput")

    # [t][p][w(2), m] — per tile/partition: pred chunk and targ chunk, each
    # m contiguous f32 in DRAM.
    pt_v = pt[:].rearrange("w (t p r) c -> t p w (r c)", t=n_tiles, p=P)

    with tile.TileContext(nc) as tc:
        with (
            tc.tile_pool(name="io", bufs=3) as io_pool,
            tc.tile_pool(name="dve_scr", bufs=2) as dve_scr,
            tc.tile_pool(name="act_scr", bufs=2) as act_scr,
            tc.tile_pool(name="acc", bufs=1) as acc_pool,
        ):
            acc_sq = acc_pool.tile([P, 3 * n_tiles], f32)
            acc_cr = acc_pool.tile([P, 3 * n_tiles], f32)

            for t in range(n_tiles):
                buf = io_pool.tile([P, 2 * m], f32, tag="buf")
                nc.sync.dma_start(
                    out=buf[:].rearrange("p (w m) -> p w m", w=2),
                    in_=pt_v[t],
                )

                # In-place: d = pred - targ, overwriting the pred half.
                nc.vector.tensor_tensor(
                    out=buf[:, 0:m],
                    in0=buf[:, 0:m],
                    in1=buf[:, m : 2 * m],
                    op=mybir.AluOpType.subtract,
                )
                d3 = buf[:, 0:m].rearrange("p (r c) -> p c r", c=3)

                # Diagonal sums on the scalar engine (Square + accum_out),
                # overlapping with the DVE cross-products.
                if use_act:
                    for i in range(3):
                        sq = act_scr.tile([P, r], f32, tag="sq")
                        nc.scalar.activation(
                            out=sq[:],
                            in_=d3[:, i, :],
                            func=mybir.ActivationFunctionType.Square,
                            accum_out=acc_sq[:, t * 3 + i : t * 3 + i + 1],
                        )
                else:
                    for i in range(3):
                        sq = dve_scr.tile([P, r], f32, tag="pr")
                        nc.vector.scalar_tensor_tensor(
                            out=sq[:],
                            in0=d3[:, i, :],
                            scalar=1.0,
                            in1=d3[:, i, :],
                            op0=mybir.AluOpType.mult,
                            op1=mybir.AluOpType.mult,
                            accum_out=acc_sq[:, t * 3 + i : t * 3 + i + 1],
                        )
                # Cross sums: fused multiply+reduce on DVE
                # (scalar_tensor_tensor: out = (in0 * 1.0) * in1, accum = sum).
                for k, (i, j) in enumerate(_PAIRS):
                    pr = dve_scr.tile([P, r], f32, tag="pr")
                    nc.vector.scalar_tensor_tensor(
                        out=pr[:],
                        in0=d3[:, i, :],
                        scalar=1.0,
                        in1=d3[:, j, :],
                        op0=mybir.AluOpType.mult,
                        op1=mybir.AluOpType.mult,
                        accum_out=acc_cr[:, t * 3 + k : t * 3 + k + 1],
                    )

            nc.sync.dma_start(out=out[:, 0 : 3 * n_tiles], in_=acc_sq[:])
            nc.sync.dma_start(out=out[:, 3 * n_tiles : 6 * n_tiles], in_=acc_cr[:])

    nc.compile()
    return nc


def build_gram_kernel_raw(n_rows: int, n_tiles: int = 32, n_bufs: int = 24,
                          group: int = 4, skip_exit_barrier: bool = True):
    """Raw-Bacc variant: manual semaphores, no TileContext.

    Skips Tile's prologue/epilogue (drain + two all-engine EVSEM
    barriers, ~16 us) — the only sync needed is a three-semaphore chain:
    DMA loads (one HWDGE ring) -> DVE -> ACT.

    The ring of tile buffers lives in ONE SBUF tensor so the fused
    multiply-reduces can span `group` consecutive tiles with a single
    instruction (free-dim AP [group, r]) — amortizing the per-op fixed
    cost and the accumulator-drain, which keeps both compute engines
    well under the DMA pace.

    Input: pt [2, n_rows, 3] f32. Output: partials [128, 6 * n_groups]
    (same slot layout as build_gram_kernel, with n_groups slots).
    """
    assert n_tiles % group == 0 and n_bufs % group == 0
    assert n_rows % (P * n_tiles) == 0
    n_groups = n_tiles // group
    r = n_rows // (P * n_tiles)
    m = 3 * r
    f32 = mybir.dt.float32

    nc = bacc.Bacc("TRN2", target_bir_lowering=False, debug=False)
    pt = nc.dram_tensor("pt", [2, n_rows, 3], f32, kind="ExternalInput")
    out = nc.dram_tensor("partials", [P, 6 * n_groups], f32, kind="ExternalOutput")
    pt_v = pt[:].rearrange("w (t p r) c -> t p w (r c)", t=n_tiles, p=P)

    ring = nc.alloc_sbuf_tensor("ring", [P, n_bufs * 2 * m], f32).ap()

    def buf(t):
        s = t % n_bufs
        return ring[:, s * 2 * m : (s + 1) * 2 * m]

    def dgroup(g, i):
        # component i of the diff halves of tiles 4g..4g+3: [128, group, r]
        s0 = (g * group) % n_bufs
        w = ring[:, s0 * 2 * m : (s0 + group) * 2 * m]
        return w.rearrange("p (t w r c) -> p t w c r", t=group, w=2, c=3)[:, :, 0, i, :]

    acc_sq = nc.alloc_sbuf_tensor("acc_sq", [P, 3 * n_groups], f32).ap()
    acc_cr = nc.alloc_sbuf_tensor("acc_cr", [P, 3 * n_groups], f32).ap()
    # Rotated scratch (dead stores of the fused ops), 2 groups deep so each
    # group's single stale semaphore wait also covers the scratch WAW from
    # two groups back.
    pr_scrs = [
        nc.alloc_sbuf_tensor(f"pr_scr{k}", [P, group * r], f32).ap() for k in range(6)
    ]
    sq_scrs = [
        nc.alloc_sbuf_tensor(f"sq_scr{k}", [P, group * r], f32).ap() for k in range(6)
    ]

    # One DMA-completion semaphore per ring buffer: a single shared sem
    # would be unsound — each dma_start is split across 16 SDMA engines
    # whose sub-completions interleave across in-flight DMAs.
    dma_sems = [nc.alloc_semaphore(f"dma_sem{i}") for i in range(n_bufs)]
    out_sem = nc.alloc_semaphore("out_sem")
    dve_sem = nc.alloc_semaphore("dve_sem")
    act_sem = nc.alloc_semaphore("act_sem")

    # DVE emission order: subs run ahead; the grouped multiply-reduces for
    # group g are emitted after sub(4g+4) so their drain-wait on the last
    # sub of the group is already satisfied when it executes (DVE writes
    # drain asynchronously). Only the last group trails the final sub.
    dve_order = []
    for t in range(n_tiles):
        dve_order.append(("sub", t))
        if t % group == 0 and t >= group:
            # one sub of stagger after the group's last sub
            dve_order.append(("stt", t // group - 1))
    dve_order.append(("stt", n_groups - 1))
    sub_done, sttg_done = {}, {}
    v = 0
    for kind, x in dve_order:
        if kind == "sub":
            v += 1
            sub_done[x] = v
        else:
            v += 3
            sttg_done[x] = v

    # Output chunks: flush finished accumulator columns while later tiles
    # still stream, so the tail only waits on the last small chunk.
    chunk = max(1, n_groups // 2)
    chunks = [(c, min(c + chunk, n_groups)) for c in range(0, n_groups, chunk)]

    import contextlib

    @contextlib.contextmanager
    def _block():
        # no_gpsimd_drain=True emits per-engine drains explicitly and then a
        # sem-only all-engine butterfly. The butterfly only delays NEFF end
        # (outputs are already fenced by the sequencer's out_sem wait), so
        # optionally no-op it during Block.__exit__.
        with nc.Block(no_gpsimd_drain=True) as blk:
            try:
                yield blk
            finally:
                if skip_exit_barrier:
                    nc.all_engine_barrier = lambda **kw: None
        if skip_exit_barrier:
            del nc.all_engine_barrier  # restore class method

    with _block() as block:

        @block.sync
        def _(sync):
            for t in range(n_tiles):
                if head_dma_on_vector and t < h0:
                    continue  # issued from the vector queue (clears the
                              # NEFF entry barrier ~1.4us before sync)
                if t >= n_bufs:
                    # ring reuse: all consumers of the buffer's previous
                    # occupant (tile t - n_bufs) must be done
                    prev = t - n_bufs
                    sync.wait_ge(dve_sem, sttg_done[prev // group])
                    sync.wait_ge(act_sem, 3 * (prev // group + 1))
                sync.dma_start(
                    out=buf(t).rearrange("p (w m) -> p w m", w=2),
                    in_=pt_v[t],
                ).then_inc(dma_sems[t % n_bufs], 16)
            n_out = 0
            for lo, hi in chunks:
                sync.wait_ge(act_sem, 3 * hi)
                sync.dma_start(
                    out=out[:, 3 * lo : 3 * hi], in_=acc_sq[:, 3 * lo : 3 * hi]
                ).then_inc(out_sem, 16)
                sync.wait_ge(dve_sem, sttg_done[hi - 1])
                sync.dma_start(
                    out=out[:, 3 * (n_groups + lo) : 3 * (n_groups + hi)],
                    in_=acc_cr[:, 3 * lo : 3 * hi],
                ).then_inc(out_sem, 16)
                n_out += 32
            sync.wait_ge(out_sem, n_out)

        @block.vector
        def _(vector):
            for kind, x in dve_order:
                if kind == "sub":
                    b = buf(x)
                    vector.wait_ge(dma_sems[x % n_bufs], 16 * (x // n_bufs + 1))
                    vector.tensor_tensor(
                        out=b[:, 0:m],
                        in0=b[:, 0:m],
                        in1=b[:, m : 2 * m],
                        op=mybir.AluOpType.subtract,
                    ).then_inc(dve_sem, 1)
                else:
                    vector.wait_ge(dve_sem, sub_done[(x + 1) * group - 1])
                    for k, (i, j) in enumerate(_PAIRS):
                        vector.scalar_tensor_tensor(
                            out=pr_scrs[(x % 2) * 3 + k][:].rearrange(
                                "p (t r) -> p t r", t=group
                            ),
                            in0=dgroup(x, i),
                            scalar=1.0,
                            in1=dgroup(x, j),
                            op0=mybir.AluOpType.mult,
                            op1=mybir.AluOpType.mult,
                            accum_out=acc_cr[:, x * 3 + k : x * 3 + k + 1],
                        ).then_inc(dve_sem, 1)

        @block.scalar
        def _(scalar):
            for g in range(n_groups):
                scalar.wait_ge(dve_sem, sub_done[(g + 1) * group - 1])
                if g >= 2:
                    # scratch slot reuse from two groups back
                    scalar.wait_ge(act_sem, 3 * (g - 1))
                for i in range(3):
                    scalar.activation(
                        out=sq_scrs[(g % 2) * 3 + i][:].rearrange(
                            "p (t r) -> p t r", t=group
                        ),
                        in_=dgroup(g, i),
                        func=mybir.ActivationFunctionType.Square,
                        accum_out=acc_sq[:, g * 3 + i : g * 3 + i + 1],
                    ).then_inc(act_sem, 1)

    nc.compile()
    return nc

def _strip_entry_barriers(nc):
    """Remove the two all-engine entry barriers Bass.__init__ emits.

    They serialize ~4us of semaphore round-trips before the first DMA can
    issue. The only cross-engine ordering they provide that this kernel
    needs is gpsimd-const-AP-memset -> ACT-bias-read, which is re-fenced
    explicitly with boot_sem in build_gram_kernel_v3.
    """
    bar = set(nc.barrier_sems)
    blk = nc.main_func.blocks[0]
    drop = []
    for ins in blk.instructions:
        si = getattr(ins, "sync_info", None)
        if si is None:
            continue
        sems = {w.id for w in si.on_wait or []}
        sems |= {u.id for u in si.on_update or []}
        if sems & bar:
            drop.append(ins)
    for ins in drop:
        blk.instructions.remove(ins)
    return len(drop)


def build_gram_kernel_v3(n_rows: int, n_tiles: int = 16, n_bufs: int = 8,
                         group: int = 4, strip_barriers: bool = True,
                         skip_exit_barrier: bool = True):
    """v3: planar-bf16 d + 2x DVE reduces + ACT squares.

    Per tile: DMA both halves -> DVE sub (fp32 in, planar bf16 out:
    component planes x|y|z so reduce operands are unit-stride 2-byte,
    unlocking the DVE 2x perf mode) -> DVE cross-product reduces (grouped
    `group` tiles per instr) + ACT Square reduces (grouped; per-tile for
    the last group so the post-DMA tail stays short).

    The fp32 ring slot is freed by the sub alone (d lives in its own
    full-size buffer), so the DMA stream runs ~n_bufs tiles ahead of
    compute and never stalls on the reduce bursts.

    Output layout [128, 3*n_groups + 3*(n_groups-1) + 3*group]:
      cols 0 .. 3*n_groups-1: cross sums (group g, pair k at 3g+k)
      then squares: full groups 0..n_groups-2 (3 each), then the last
      group's tiles individually (3 each).
    """
    assert n_rows % (P * n_tiles) == 0 and n_tiles % group == 0
    r = n_rows // (P * n_tiles)
    m = 3 * r
    n_groups = n_tiles // group
    full_sq = n_groups - 1           # square-groups emitted grouped
    tail0 = full_sq * group          # first per-tile-squares tile
    ncr = 3 * n_groups
    nsq = 3 * full_sq + 3 * group
    f32, bf16 = mybir.dt.float32, mybir.dt.bfloat16

    nc = bacc.Bacc("TRN2", target_bir_lowering=False, debug=False)
    if strip_barriers:
        _strip_entry_barriers(nc)
    pt = nc.dram_tensor("pt", [2, n_rows, 3], f32, kind="ExternalInput")
    out = nc.dram_tensor("partials", [P, ncr + nsq], f32, kind="ExternalOutput")
    pt_v = pt[:].rearrange("w (t p r) c -> t p w (r c)", t=n_tiles, p=P)

    ring = nc.alloc_sbuf_tensor("ring", [P, n_bufs * 2 * m], f32).ap()
    d_all = nc.alloc_sbuf_tensor("d_all", [P, n_tiles * m], bf16).ap()
    d_t = d_all.rearrange("p (t c r) -> p t c r", t=n_tiles, c=3)
    acc_cr = nc.alloc_sbuf_tensor("acc_cr", [P, ncr], f32).ap()
    acc_sq = nc.alloc_sbuf_tensor("acc_sq", [P, nsq], f32).ap()
    # Dead stores of the fused reduces; single slot per engine (each
    # engine executes its own stream in order, so WAW is safe).
    cr_scr = nc.alloc_sbuf_tensor("cr_scr", [P, group * r], bf16).ap()
    sq_scr = nc.alloc_sbuf_tensor("sq_scr", [P, group * r], bf16).ap()

    dma_sems = [nc.alloc_semaphore(f"dma{i}") for i in range(n_bufs)]
    sub_sem = nc.alloc_semaphore("sub_sem")
    red_sem = nc.alloc_semaphore("red_sem")
    act_sem = nc.alloc_semaphore("act_sem")
    out_sem = nc.alloc_semaphore("out_sem")
    boot_sem = nc.alloc_semaphore("boot_sem")

    def dcomp(t0, nt, i):
        # component i of tiles t0..t0+nt-1: [128, nt, r] unit-stride bf16
        v = d_t[:, t0 : t0 + nt, i, :]
        return v

    import contextlib

    @contextlib.contextmanager
    def _block():
        with nc.Block(no_gpsimd_drain=True) as blk:
            try:
                yield blk
            finally:
                if skip_exit_barrier:
                    nc.all_engine_barrier = lambda **kw: None
        if skip_exit_barrier:
            del nc.all_engine_barrier  # restore class method

    with _block() as block:

        @block.gpsimd
        def _(gpsimd):
            # Const-AP memsets (ACT bias) are earlier in gpsimd's stream;
            # this inc publishes their completion to the scalar queue.
            gpsimd.sem_inc(boot_sem, 1)

        @block.sync
        def _(sync):
            for t in range(n_tiles):
                if head_dma_on_vector and t < h0:
                    continue  # issued from the vector queue (clears the
                              # NEFF entry barrier ~1.4us before sync)
                if t >= n_bufs:
                    # ring slot free once its previous occupant was subbed
                    sync.wait_ge(sub_sem, t - n_bufs + 1)
                sync.dma_start(
                    out=ring[:, (t % n_bufs) * 2 * m : (t % n_bufs + 1) * 2 * m]
                    .rearrange("p (w m) -> p w m", w=2),
                    in_=pt_v[t],
                ).then_inc(dma_sems[t % n_bufs], 16)
            # accumulator flush: big chunks early, last-group slivers at end
            sync.wait_ge(red_sem, 3 * (n_groups - 1))
            sync.dma_start(
                out=out[:, 0 : 3 * (n_groups - 1)],
                in_=acc_cr[:, 0 : 3 * (n_groups - 1)],
            ).then_inc(out_sem, 16)
            sync.wait_ge(act_sem, 3 * full_sq)
            sync.dma_start(
                out=out[:, ncr : ncr + 3 * full_sq],
                in_=acc_sq[:, 0 : 3 * full_sq],
            ).then_inc(out_sem, 16)
            sync.wait_ge(red_sem, 3 * n_groups)
            sync.dma_start(
                out=out[:, 3 * (n_groups - 1) : ncr],
                in_=acc_cr[:, 3 * (n_groups - 1) : ncr],
            ).then_inc(out_sem, 16)
            sync.wait_ge(act_sem, nsq)
            sync.dma_start(
                out=out[:, ncr + 3 * full_sq : ncr + nsq],
                in_=acc_sq[:, 3 * full_sq : nsq],
            ).then_inc(out_sem, 16)
            sync.wait_ge(out_sem, 64)

        @block.vector
        def _(vector):
            for t in range(n_tiles):
                s = t % n_bufs
                buf = ring[:, s * 2 * m : (s + 1) * 2 * m]
                vector.wait_ge(dma_sems[s], 16 * (t // n_bufs + 1))
                # d = pred - targ, downcast to bf16, scattered into
                # component planes (write AP [r, 3] w/ strides [1, r])
                vector.tensor_tensor(
                    out=d_all[:, t * m : (t + 1) * m].rearrange(
                        "p (c r) -> p r c", c=3
                    ),
                    in0=buf[:, 0:m],
                    in1=buf[:, m : 2 * m],
                    op=mybir.AluOpType.subtract,
                ).then_inc(sub_sem, 1)
                if t % group == group - 1:
                    g = t // group
                    for k, (i, j) in enumerate(_PAIRS):
                        vector.scalar_tensor_tensor(
                            out=cr_scr[:].rearrange("p (t r) -> p t r", t=group),
                            in0=dcomp(g * group, group, i),
                            scalar=1.0,
                            in1=dcomp(g * group, group, j),
                            op0=mybir.AluOpType.mult,
                            op1=mybir.AluOpType.mult,
                            accum_out=acc_cr[:, g * 3 + k : g * 3 + k + 1],
                        ).then_inc(red_sem, 1)

        @block.scalar
        def _(scalar):
            scalar.wait_ge(boot_sem, 1)
            for g in range(full_sq):
                scalar.wait_ge(sub_sem, group * (g + 1))
                for i in range(3):
                    scalar.activation(
                        out=sq_scr[:].rearrange("p (t r) -> p t r", t=group),
                        in_=dcomp(g * group, group, i),
                        func=mybir.ActivationFunctionType.Square,
                        accum_out=acc_sq[:, g * 3 + i : g * 3 + i + 1],
                    ).then_inc(act_sem, 1)
            for w, t in enumerate(range(tail0, n_tiles)):
                scalar.wait_ge(sub_sem, t + 1)
                for i in range(3):
                    c = 3 * full_sq + w * 3 + i
                    scalar.activation(
                        out=sq_scr[:, 0:r],
                        in_=dcomp(t, 1, i),
                        func=mybir.ActivationFunctionType.Square,
                        accum_out=acc_sq[:, c : c + 1],
                    ).then_inc(act_sem, 1)

    nc.compile()
    nc._v3_meta = (n_tiles, group)
    return nc


def build_gram_kernel_v4(n_rows: int, bulk_r: int = 512, n_bufs: int = 12,
                         group: int = 4, gp_stride: int = 0,
                         head_rs: tuple = (128, 256, 256, 384),
                         tail_rs: tuple = (256, 128, 64, 32, 32),
                         dpad: int = 3, head_dma_on_vector: bool = True,
                         strip_barriers: bool = True,
                         skip_exit_barrier: bool = True):
    """v4: interleaved-bf16 d, measured-cost engine mix, shrinking tail.

    Measured HW rates (ns per 128-wide column): DVE sub fp32->bf16 unit
    1.28; DVE stt reduce bf16 stride-3 1.32; ACT Square ~1.0-1.4 + 740
    fixed; GPSIMD sub ~3.5. Writes must be unit-stride (scatter = 4.6x);
    strided reads are cheap. So d stays row-interleaved bf16.

    - bulk tiles of r=bulk_r rows/partition; every gp_stride-th bulk tile's
      sub runs on GPSIMD to keep DVE under the DMA pace.
    - cross-products: DVE stt grouped over `group` consecutive bulk tiles.
    - squares: ACT, same grouping; tail tiles per-tile; r<=32 tails on DVE.
    - tail tiles shrink so the post-last-DMA dependency chain is tiny.
    """
    R = n_rows // P
    assert n_rows % P == 0
    bulk_n = (R - sum(head_rs) - sum(tail_rs)) // bulk_r
    assert sum(head_rs) + bulk_n * bulk_r + sum(tail_rs) == R
    rs = list(head_rs) + [bulk_r] * bulk_n + list(tail_rs)
    n_tiles = len(rs)
    h0 = len(head_rs)               # first bulk tile index
    t0_tail = h0 + bulk_n           # first tail tile index
    cum = [0]
    for r in rs:
        cum.append(cum[-1] + r)
    # bulk groups: chunks of `group` (absolute tile indices)
    groups = [list(range(s, min(s + group, t0_tail)))
              for s in range(h0, t0_tail, group)]
    group_last = {g[-1]: g for g in groups}
    # per-tile (ungrouped) reduce tiles: head + tail
    per_tile = set(range(0, h0)) | set(range(t0_tail, n_tiles))
    # every gp_stride-th bulk tile's sub runs on GPSIMD (0 = none)
    gp_tiles = (set(range(h0, t0_tail, gp_stride)) if gp_stride else set())
    f32, bf16 = mybir.dt.float32, mybir.dt.bfloat16

    nc = bacc.Bacc("TRN2", target_bir_lowering=False, debug=False)
    if strip_barriers:
        _strip_entry_barriers(nc)
    pt = nc.dram_tensor("pt", [2, n_rows, 3], f32, kind="ExternalInput")

    # per-tile engine assignment of the sub + cumulative sem targets
    dve_idx, gp_idx = {}, {}
    for t in range(n_tiles):
        if t in gp_tiles:
            gp_idx[t] = len(gp_idx)
        else:
            dve_idx[t] = len(dve_idx)

    def sub_waits(last_t):
        """(sub_sem target, gsub_sem target) covering tiles 0..last_t."""
        d = sum(1 for t, i in dve_idx.items() if t <= last_t)
        g = sum(1 for t, i in gp_idx.items() if t <= last_t)
        return d, g

    # reduce slot counts (order finalized at emission)
    n_dve = 3 * len(groups) + 3 * len(per_tile) + 3 * sum(
        1 for t in per_tile if rs[t] <= 32)
    plan_act = []
    for t in sorted(per_tile):
        if t < h0 and rs[t] > 32:
            for i in range(3):
                plan_act.append((i, i))
    for g_tiles in groups:
        for i in range(3):
            plan_act.append((i, i))
    for t in sorted(per_tile):
        if t >= t0_tail and rs[t] > 32:
            for i in range(3):
                plan_act.append((i, i))
    n_act = len(plan_act)
    out = nc.dram_tensor("partials", [P, n_dve + n_act], f32,
                         kind="ExternalOutput")

    m_bulk = 3 * bulk_r
    ring = nc.alloc_sbuf_tensor("ring", [P, n_bufs * 2 * m_bulk], f32).ap()
    d_all = nc.alloc_sbuf_tensor("d_all", [P, dpad * R], bf16).ap()
    acc_dve = nc.alloc_sbuf_tensor("acc_dve", [P, n_dve], f32).ap()
    acc_act = nc.alloc_sbuf_tensor("acc_act", [P, n_act], f32).ap()
    cr_scr = nc.alloc_sbuf_tensor("cr_scr", [P, group * bulk_r], bf16).ap()
    sq_scr = nc.alloc_sbuf_tensor("sq_scr", [P, group * bulk_r], bf16).ap()

    dma_sems = [nc.alloc_semaphore(f"dma{i}") for i in range(n_bufs)]
    sub_sem = nc.alloc_semaphore("sub_sem")
    gsub_sem = nc.alloc_semaphore("gsub_sem")
    red_sem = nc.alloc_semaphore("red_sem")
    act_sem = nc.alloc_semaphore("act_sem")
    out_sem = nc.alloc_semaphore("out_sem")
    boot_sem = nc.alloc_semaphore("boot_sem")

    def ring_slot(t):
        s = t % n_bufs
        return ring[:, s * 2 * m_bulk : s * 2 * m_bulk + 2 * 3 * rs[t]]

    def pt_tile(t):
        sl = pt[:, cum[t] * P : cum[t + 1] * P, :]
        return sl.rearrange("w (p r) c -> p w (r c)", p=P)

    def dseg(t0, nt, i):
        # component i of tiles t0..t0+nt-1 (equal r), stride-dpad reads
        v = d_all[:, dpad * cum[t0] : dpad * cum[t0 + nt]]
        return v.rearrange("p (t r c) -> p t c r", t=nt, c=dpad)[:, :, i, :]

    def dsub_out(t):
        # write view for the sub: rows of 3 packed comps, dpad-elem row pitch
        v = d_all[:, dpad * cum[t] : dpad * cum[t + 1]]
        if dpad == 3:
            return v
        return v.rearrange("p (r c) -> p r c", c=dpad)[:, :, 0:3]

    # early-flush boundaries: head reduces + all-but-last bulk group are
    # guaranteed emitted before any tail reduce; ACT head+bulk likewise
    red_early = 3 * h0 + 3 * (len(groups) - 1)
    act_bulk = 3 * h0 + 3 * len(groups)
    _red_pen = max(red_early, n_dve - 6)
    _act_pen = max(act_bulk, n_act - 3)
    n_flush_dve = sum(1 for lo, hi in ((0, red_early), (red_early, _red_pen),
                                       (_red_pen, n_dve)) if hi > lo)
    act_chunks = [(act_bulk, 0, act_bulk), (_act_pen, act_bulk, _act_pen),
                  (n_act, _act_pen, n_act)]
    act_chunks = [c for c in act_chunks if c[2] > c[1]]
    n_flush_act = len(act_chunks)

    import contextlib

    @contextlib.contextmanager
    def _block():
        with nc.Block(no_gpsimd_drain=True) as blk:
            try:
                yield blk
            finally:
                if skip_exit_barrier:
                    nc.all_engine_barrier = lambda **kw: None
        if skip_exit_barrier:
            del nc.all_engine_barrier

    with _block() as block:

        @block.sync
        def _(sync):
            for t in range(n_tiles):
                if head_dma_on_vector and t < h0:
                    continue  # issued from the vector queue (clears the
                              # NEFF entry barrier ~1.4us before sync)
                if t >= n_bufs:
                    u = t - n_bufs
                    if u in gp_idx:
                        sync.wait_ge(gsub_sem, gp_idx[u] + 1)
                    else:
                        sync.wait_ge(sub_sem, dve_idx[u] + 1)
                sync.dma_start(
                    out=ring_slot(t).rearrange("p (w m) -> p w m", w=2),
                    in_=pt_tile(t),
                ).then_inc(dma_sems[t % n_bufs], 16)
            # acc_dve flushes stay here; acc_act flushes issue from the
            # scalar queue (ACT finishes ~2.5us before DVE, so they then
            # complete during the DVE tail instead of serializing after it)
            red_pen = max(red_early, n_dve - 6)   # all but last tail tile
            for val, lo, hi in ((red_early, 0, red_early),
                                (red_pen, red_early, red_pen),
                                (n_dve, red_pen, n_dve)):
                if hi <= lo:
                    continue
                sync.wait_ge(red_sem, val)
                sync.dma_start(out=out[:, lo:hi],
                               in_=acc_dve[:, lo:hi]).then_inc(out_sem, 16)
            sync.wait_ge(out_sem, 16 * (n_flush_dve + n_flush_act))

        @block.vector
        def _(vector):
            red_c = 0
            plan_dyn = []
            pending = []  # deferred bulk-group reduces, drained 1-per-sub

            def emit_one(t0, nt, i, j):
                nonlocal red_c
                r = rs[t0]
                vector.scalar_tensor_tensor(
                    out=cr_scr[:, 0 : nt * r].rearrange(
                        "p (t r) -> p t r", t=nt),
                    in0=dseg(t0, nt, i),
                    scalar=1.0,
                    in1=dseg(t0, nt, j),
                    op0=mybir.AluOpType.mult,
                    op1=mybir.AluOpType.mult,
                    accum_out=acc_dve[:, red_c : red_c + 1],
                ).then_inc(red_sem, 1)
                plan_dyn.append((i, j))
                red_c += 1

            for t in range(n_tiles):
                if t not in gp_tiles:
                    buf = ring_slot(t)
                    mt = 3 * rs[t]
                    vector.wait_ge(dma_sems[t % n_bufs], 16 * (t // n_bufs + 1))
                    vector.tensor_tensor(
                        out=dsub_out(t),
                        in0=buf[:, 0:mt],
                        in1=buf[:, mt : 2 * mt],
                        op=mybir.AluOpType.subtract,
                    ).then_inc(sub_sem, 1)
                    # spread deferred group reduces between subs so the DMA
                    # ring (freed by subs) never stalls on a reduce burst
                    if pending:
                        emit_one(*pending.pop(0))
                if t in group_last:
                    g_tiles = group_last[t]
                    _, gw = sub_waits(g_tiles[-1])
                    if gw:
                        vector.wait_ge(gsub_sem, gw)
                    for (i, j) in _PAIRS:
                        pending.append((g_tiles[0], len(g_tiles), i, j))
                elif t in per_tile:
                    for (i, j) in _PAIRS:
                        emit_one(t, 1, i, j)
                    if rs[t] <= 32:
                        for i in range(3):
                            emit_one(t, 1, i, i)
            while pending:
                emit_one(*pending.pop(0))
            nc._v4_plan_dve = plan_dyn

        @block.scalar
        def _(scalar):
            if head_dma_on_vector:
                # Scalar clears the NEFF entry-barrier chain ~2us before
                # Sync; issuing the head-tile loads here starts the stream
                # (and so the first sub) earlier. Ring slots are fresh, no
                # waits needed; the const-AP fence only matters for the
                # activations below.
                for t in range(h0):
                    scalar.dma_start(
                        out=ring_slot(t).rearrange("p (w m) -> p w m", w=2),
                        in_=pt_tile(t),
                    ).then_inc(dma_sems[t % n_bufs], 16)
            scalar.wait_ge(boot_sem, 1)
            act_c = 0

            def emit_squares(t0, nt):
                nonlocal act_c
                r = rs[t0]
                for i in range(3):
                    scalar.activation(
                        out=sq_scr[:, 0 : nt * r].rearrange(
                            "p (t r) -> p t r", t=nt),
                        in_=dseg(t0, nt, i),
                        func=mybir.ActivationFunctionType.Square,
                        accum_out=acc_act[:, act_c : act_c + 1],
                    ).then_inc(act_sem, 1)
                    act_c += 1

            def sq_waits(last_t):
                d, g = sub_waits(last_t)
                scalar.wait_ge(sub_sem, d)
                if g:
                    scalar.wait_ge(gsub_sem, g)

            for t in sorted(per_tile):
                if t < h0 and rs[t] > 32:
                    sq_waits(t)
                    emit_squares(t, 1)
            for g_tiles in groups:
                sq_waits(g_tiles[-1])
                emit_squares(g_tiles[0], len(g_tiles))
            for t in sorted(per_tile):
                if t >= t0_tail and rs[t] > 32:
                    sq_waits(t)
                    emit_squares(t, 1)
            for val, lo, hi in act_chunks:
                scalar.wait_ge(act_sem, val)
                scalar.dma_start(out=out[:, n_dve + lo : n_dve + hi],
                                 in_=acc_act[:, lo:hi]).then_inc(out_sem, 16)

        @block.gpsimd
        def _(gpsimd):
            # Zero the accumulator tensors so a (never-observed, but cheap
            # to insure against) flush-before-drain race reads zeros - a
            # ~1e-4 relative error - instead of stale SBUF garbage.
            gpsimd.memset(acc_dve, 0.0)
            gpsimd.memset(acc_act, 0.0)
            gpsimd.sem_inc(boot_sem, 1)
            for t in sorted(gp_tiles):
                buf = ring_slot(t)
                mt = 3 * rs[t]
                gpsimd.wait_ge(dma_sems[t % n_bufs], 16 * (t // n_bufs + 1))
                gpsimd.tensor_tensor(
                    out=dsub_out(t),
                    in0=buf[:, 0:mt],
                    in1=buf[:, mt : 2 * mt],
                    op=mybir.AluOpType.subtract,
                ).then_inc(gsub_sem, 1)

    nc.compile()
    assert len(nc._v4_plan_dve) == n_dve
    nc._v4_plan = (nc._v4_plan_dve, plan_act)
    return nc


def build_diag_kernel_v5(n_rows: int, bulk_r: int = 512, n_bufs: int = 14,
                         n_dbufs: int = 8,
                         head_rs: tuple = (128, 128, 256),
                         tail_rs: tuple = (256, 256, 192, 128, 64, 64, 32, 32),
                         n_sq_dve: int = 4,
                         strip_barriers: bool = True,
                         skip_exit_barrier: bool = True):
    """v5: diagonal-sigma fast path — per-tile sum of squared differences.

    For sigma = c*I (the shipped input), the loss needs only
    S = sum_b ||p_b - t_b||^2; no cross products. Per tile: DMA both
    halves -> DVE sub (fp32 in, bf16 interleaved out, unit-stride write)
    -> Square with accum_out (one fp32 partial column per tile), on ACT
    for most tiles. Both engines run at ~50% of the DMA pace, so the
    kernel is purely DMA-bound: the 16-engine pool sustains ~415 GB/s.

    Head: small ramp tiles issued from the gpsimd/scalar queues (they
    clear the NEFF boot chain before sync) — small, so the brief
    3-queue pool contention costs little. Tail: ramp-down sizes, with
    the squares of the last 2*n_sq_dve tiles alternating DVE/ACT so the
    post-last-load catch-up runs on both engines in parallel.

    Output: partials [128, n_tiles] (ACT tiles then DVE tiles, by the
    _v5_order attr); host sums everything (f64).
    """
    R = n_rows // P
    assert n_rows % P == 0
    bulk_n = (R - sum(head_rs) - sum(tail_rs)) // bulk_r
    assert sum(head_rs) + bulk_n * bulk_r + sum(tail_rs) == R
    rs = list(head_rs) + [bulk_r] * bulk_n + list(tail_rs)
    n_tiles = len(rs)
    cum = [0]
    for r in rs:
        cum.append(cum[-1] + r)
    m_bulk = 3 * max(rs)
    f32, bf16 = mybir.dt.float32, mybir.dt.bfloat16

    # squares of the last 2*n_sq_dve tiles alternate DVE/ACT (DVE takes
    # the even offsets from the end: ..., t-4, t-2, last)
    sq_dve = {n_tiles - 1 - 2 * k for k in range(n_sq_dve)}
    act_tiles = [t for t in range(n_tiles) if t not in sq_dve]
    dve_tiles = sorted(sq_dve)
    n_act, n_dve = len(act_tiles), len(dve_tiles)
    acol = {t: i for i, t in enumerate(act_tiles)}
    vcol = {t: i for i, t in enumerate(dve_tiles)}
    # act_sem value after the square of tile u (ACT tiles only)
    act_done = {t: i + 1 for i, t in enumerate(act_tiles)}
    # sub_sem value after DVE finished tile t (sub, plus square if DVE tile)
    sub_done = {}
    _v = 0
    for _t in range(n_tiles):
        _v += 2 if _t in sq_dve else 1
        sub_done[_t] = _v
    n_flush = (1 if n_act > 1 else 0) + 1 + (1 if n_dve else 0)

    nc = bacc.Bacc("TRN2", target_bir_lowering=False, debug=False)
    if strip_barriers:
        _strip_entry_barriers(nc)
    pt = nc.dram_tensor("pt", [2, n_rows, 3], f32, kind="ExternalInput")
    out = nc.dram_tensor("partials", [P, n_tiles], f32, kind="ExternalOutput")

    ring = nc.alloc_sbuf_tensor("ring", [P, n_bufs * 2 * m_bulk], f32).ap()
    dbuf = nc.alloc_sbuf_tensor("dbuf", [P, n_dbufs * m_bulk], bf16).ap()
    acc_a = nc.alloc_sbuf_tensor("acc_a", [P, max(n_act, 1)], f32).ap()
    acc_v = nc.alloc_sbuf_tensor("acc_v", [P, max(n_dve, 1)], f32).ap()
    sq_scr = nc.alloc_sbuf_tensor("sq_scr", [P, m_bulk], bf16).ap()
    vq_scr = nc.alloc_sbuf_tensor("vq_scr", [P, m_bulk], bf16).ap()

    dma_sems = [nc.alloc_semaphore(f"dma{i}") for i in range(n_bufs)]
    sub_sem = nc.alloc_semaphore("sub_sem")
    act_sem = nc.alloc_semaphore("act_sem")
    out_sem = nc.alloc_semaphore("out_sem")
    boot_sem = nc.alloc_semaphore("boot_sem")

    def ring_slot(t):
        s = t % n_bufs
        return ring[:, s * 2 * m_bulk : s * 2 * m_bulk + 2 * 3 * rs[t]]

    def d_slot(t):
        s = t % n_dbufs
        return dbuf[:, s * m_bulk : s * m_bulk + 3 * rs[t]]

    def pt_tile(t):
        sl = pt[:, cum[t] * P : cum[t + 1] * P, :]
        return sl.rearrange("w (p r) c -> p w (r c)", p=P)

    n_head = len(head_rs)

    def issue_load(q, t):
        q.dma_start(
            out=ring_slot(t).rearrange("p (w m) -> p w m", w=2),
            in_=pt_tile(t),
        ).then_inc(dma_sems[t % n_bufs], 16)

    import contextlib

    @contextlib.contextmanager
    def _block():
        with nc.Block(no_gpsimd_drain=True) as blk:
            try:
                yield blk
            finally:
                if skip_exit_barrier:
                    nc.all_engine_barrier = lambda **kw: None
        if skip_exit_barrier:
            del nc.all_engine_barrier

    with _block() as block:

        @block.sync
        def _(sync):
            for t in range(2, n_tiles):
                if t >= n_bufs:
                    # ring slot free once its previous occupant was subbed
                    u = t - n_bufs
                    sync.wait_ge(sub_sem, sub_done[u] - (1 if u in sq_dve else 0))
                issue_load(sync, t)
            if n_dve:
                # DVE squares all drained once the last DVE tile's pair ran;
                # this flush overlaps scalar's final acc_a sliver flush.
                sync.wait_ge(sub_sem, sub_done[dve_tiles[-1]])
                sync.dma_start(
                    out=out[:, n_act : n_act + n_dve], in_=acc_v[:, 0:n_dve]
                ).then_inc(out_sem, 16)
            sync.wait_ge(out_sem, 16 * n_flush)

        @block.vector
        def _(vector):
            for t in range(n_tiles):
                vector.wait_ge(dma_sems[t % n_bufs], 16 * (t // n_bufs + 1))
                u = t - n_dbufs
                if u >= 0 and u not in sq_dve:
                    # d slot free once its previous occupant was squared
                    vector.wait_ge(act_sem, act_done[u])
                buf = ring_slot(t)
                mt = 3 * rs[t]
                vector.tensor_tensor(
                    out=d_slot(t),
                    in0=buf[:, 0:mt],
                    in1=buf[:, mt : 2 * mt],
                    op=mybir.AluOpType.subtract,
                ).then_inc(sub_sem, 1)
                if t in sq_dve:
                    c = vcol[t]
                    vector.scalar_tensor_tensor(
                        out=vq_scr[:, 0 : 3 * rs[t]],
                        in0=d_slot(t),
                        scalar=1.0,
                        in1=d_slot(t),
                        op0=mybir.AluOpType.mult,
                        op1=mybir.AluOpType.mult,
                        accum_out=acc_v[:, c : c + 1],
                    ).then_inc(sub_sem, 1)

        @block.scalar
        def _(scalar):
            # One small head tile on this queue: it drains before the sync
            # stream builds up, buying the 0.6us the sync engine's boot lags.
            issue_load(scalar, 1)
            scalar.wait_ge(boot_sem, 1)
            flush0 = act_tiles[-2] if n_act > 1 else None
            for t in act_tiles:
                scalar.wait_ge(sub_sem, sub_done[t])
                c = acol[t]
                scalar.activation(
                    out=sq_scr[:, 0 : 3 * rs[t]],
                    in_=d_slot(t),
                    func=mybir.ActivationFunctionType.Square,
                    accum_out=acc_a[:, c : c + 1],
                ).then_inc(act_sem, 1)
                if t == flush0:
                    scalar.wait_ge(act_sem, n_act - 1)
                    scalar.dma_start(
                        out=out[:, 0 : n_act - 1], in_=acc_a[:, 0 : n_act - 1]
                    ).then_inc(out_sem, 16)
            scalar.wait_ge(act_sem, n_act)
            lo = max(n_act - 2, 0) if flush0 is not None else 0
            scalar.dma_start(
                out=out[:, lo:n_act], in_=acc_a[:, lo:n_act]
            ).then_inc(out_sem, 16)

        @block.gpsimd
        def _(gpsimd):
            issue_load(gpsimd, 0)
            # Zero accs so a flush-before-drain race reads zeros, not garbage.
            gpsimd.memset(acc_a, 0.0)
            gpsimd.memset(acc_v, 0.0)
            gpsimd.sem_inc(boot_sem, 1)

    nc.compile()
    nc._v5_order = (act_tiles, dve_tiles)
    return nc


def gram_from_partials_v4(partials: np.ndarray, plan) -> np.ndarray:
    plan_dve, plan_act = plan
    s = partials.astype(np.float64).reshape(-1, partials.shape[-1]).sum(axis=0)
    g = np.zeros((3, 3), dtype=np.float64)
    for c, (i, j) in enumerate(plan_dve + plan_act):
        if i == j:
            g[i, i] += s[c]
        else:
            g[i, j] += s[c]
            g[j, i] += s[c]
    return g


def gram_from_partials_v3(partials: np.ndarray, n_tiles: int, group: int) -> np.ndarray:
    n_groups = n_tiles // group
    ncr = 3 * n_groups
    s = partials.astype(np.float64).reshape(-1, partials.shape[-1]).sum(axis=0)
    cr = s[:ncr].reshape(-1, 3).sum(axis=0)
    sq = s[ncr:].reshape(-1, 3).sum(axis=0)
    g = np.empty((3, 3), dtype=np.float64)
    g[0, 0], g[1, 1], g[2, 2] = sq
    for k, (i, j) in enumerate(_PAIRS):
        g[i, j] = g[j, i] = cr[k]
    return g


_NC_CACHE: dict[tuple, object] = {}


def _get_nc(n_rows: int, n_tiles: int, use_act: bool, raw: bool = False,
            group: int = 4, version: int = 4, n_bufs: int = 12,
            strip_barriers: bool = True, gp_stride: int = 0,
            bulk_r: int = 512, tail_rs: tuple = (256, 128, 64, 32, 32),
            head_rs: tuple = (128, 256, 256, 384), dpad: int = 3,
            head_dma_on_vector: bool = True):
    key = (n_rows, n_tiles, use_act, raw, group, version, n_bufs,
           strip_barriers, gp_stride, bulk_r, tail_rs, head_rs, dpad,
           head_dma_on_vector)
    if key not in _NC_CACHE:
        if version == 5:
            _NC_CACHE[key] = build_diag_kernel_v5(
                n_rows, strip_barriers=strip_barriers)
        elif version == 4:
            _NC_CACHE[key] = build_gram_kernel_v4(
                n_rows, bulk_r=bulk_r, n_bufs=n_bufs, group=group,
                gp_stride=gp_stride, head_rs=head_rs, tail_rs=tail_rs,
                dpad=dpad, head_dma_on_vector=head_dma_on_vector,
                strip_barriers=strip_barriers)
        elif version == 3:
            _NC_CACHE[key] = build_gram_kernel_v3(
                n_rows, n_tiles, n_bufs=n_bufs, group=group,
                strip_barriers=strip_barriers)
        elif raw:
            _NC_CACHE[key] = build_gram_kernel_raw(n_rows, n_tiles, group=group)
        else:
            _NC_CACHE[key] = build_gram_kernel(n_rows, n_tiles, use_act)
    return _NC_CACHE[key]


def gram_from_partials(partials: np.ndarray, n_tiles: int | None = None) -> np.ndarray:
    """[..., 128, 6*slots] partials -> full 3x3 Gram matrix (float64)."""
    slots = partials.shape[-1] // 6
    s = partials.astype(np.float64).reshape(-1, 6 * slots).sum(axis=0)
    sq = s[: 3 * slots].reshape(slots, 3).sum(axis=0)
    cr = s[3 * slots :].reshape(slots, 3).sum(axis=0)
    g = np.empty((3, 3), dtype=np.float64)
    g[0, 0], g[1, 1], g[2, 2] = sq
    for k, (i, j) in enumerate(_PAIRS):
        g[i, j] = g[j, i] = cr[k]
    return g


def run_device_partials(predictions: np.ndarray, targets: np.ndarray,
                        n_tiles: int = 4, use_act: bool = True,
                        raw: bool = False, group: int = 4, version: int = 4,
                        n_bufs: int = 12, strip_barriers: bool = True,
                        gp_stride: int = 0, bulk_r: int = 512,
                        tail_rs: tuple = (256, 128, 64, 32, 32),
                        head_rs: tuple = (128, 256, 256, 384), dpad: int = 3,
                        head_dma_on_vector: bool = True,
                        **run_kwargs):
    """Shard over N_CORES, run on device, return per-core partials + results."""
    b = predictions.shape[0]
    assert b % N_CORES == 0
    n_rows = b // N_CORES
    nc = _get_nc(n_rows, n_tiles, use_act, raw, group, version, n_bufs,
                 strip_barriers, gp_stride, bulk_r, tail_rs, head_rs, dpad,
                 head_dma_on_vector)
    preds = np.ascontiguousarray(predictions, dtype=np.float32).reshape(
        N_CORES, n_rows, 3
    )
    targs = np.ascontiguousarray(targets, dtype=np.float32).reshape(
        N_CORES, n_rows, 3
    )
    in_maps = [
        {"pt": np.stack([preds[c], targs[c]])} for c in range(N_CORES)
    ]
    res = run_bass_kernel_spmd(nc, in_maps, list(range(N_CORES)), **run_kwargs)
    partials = np.stack([r["partials"] for r in res.results])
    return partials, res, nc


def _host_loss(predictions, targets, sigma_inv, logdet, lo=0, hi=None):
    """Exact (float64) loss over rows [lo, hi) on the host, chunked."""
    hi = predictions.shape[0] if hi is None else hi
    tot = 0.0
    for s in range(lo, hi, 1 << 20):
        e = min(s + (1 << 20), hi)
        d = predictions[s:e].astype(np.float64) - targets[s:e].astype(np.float64)
        tot += float(np.einsum("bi,ij,bj->", d, sigma_inv, d))
    return abs(logdet + tot / (hi - lo))


def _sigma_inv_is_scalar(sigma_inv: np.ndarray) -> bool:
    """True iff sigma_inv == c*I to fp64 precision (the shipped input)."""
    d = np.diag(sigma_inv)
    off = sigma_inv - np.diag(d)
    tol = 1e-9 * float(np.abs(d).min())
    return (float(np.abs(off).max()) <= tol
            and float(np.abs(d - d[0]).max()) <= 1e-9 * abs(float(d[0])))


def kernel(predictions: np.ndarray, targets: np.ndarray, sigma: np.ndarray) -> np.ndarray:
    predictions = np.asarray(predictions, dtype=np.float32)
    targets = np.asarray(targets, dtype=np.float32)
    sigma64 = np.asarray(sigma, dtype=np.float64)
    sigma_inv = np.linalg.inv(sigma64)
    _, logdet = np.linalg.slogdet(sigma64)

    # Cheap subsample estimate (~0.3% rel) to sanity-gate the device result.
    est = _host_loss(predictions, targets, sigma_inv, logdet,
                     0, min(1 << 16, predictions.shape[0]))

    use_v5 = _sigma_inv_is_scalar(sigma_inv)
    loss = None
    for _attempt in range(2):
        if use_v5:
            partials, _, _ = run_device_partials(predictions, targets, version=5)
            s = float(partials.astype(np.float64).sum())
            mean_mahal = float(sigma_inv[0, 0]) * s / predictions.shape[0]
        else:
            partials, _, nc = run_device_partials(predictions, targets, version=4)
            g = gram_from_partials_v4(partials, nc._v4_plan)
            mean_mahal = float((sigma_inv * g).sum()) / predictions.shape[0]
        loss = abs(logdet + mean_mahal)
        if np.isfinite(loss) and abs(loss - est) <= 0.05 * max(abs(est), 1e-9):
            return np.float32(loss)
    # Device result failed the sanity gate twice: fall back to exact host.
    return np.float32(_host_loss(predictions, targets, sigma_inv, logdet))



# revision 18
# speedup vs baseline: 1.2374x; 1.0019x over previous
"""Trainium2 Bass kernel for CustomLossWithCovariance.

loss = abs(logdet(sigma) + mean_b[(p_b - t_b)^T sigma^{-1} (p_b - t_b)])

Only the 3x3 Gram matrix G = sum_b d_b d_b^T (d = pred - targ) requires
touching the [B, 3] data; the device computes per-core partial pair-sums
of G, and the host finishes with the tiny 3x3 algebra:
    mean_mahalanobis = <sigma_inv, G> / B
    loss = |logdet(sigma) + mean_mahalanobis|

Sharding: data-parallel over the batch across 8 NeuronCores (each core
streams a contiguous [B/8, 3] shard; partial sums gathered on host).

Production path: build_gram_kernel_v4 (raw Bacc, manual semaphores).
Per tile: one dma_start brings pred|targ halves; DVE subtracts into a
row-interleaved bf16 d buffer (unit-stride write — scatter writes are
4.6x slower on DVE); DVE fused multiply-reduces (stride-3 component
reads, grouped across tiles) produce the cross sums and ACT Square
accumulate produces the diagonals. Tile sizes ramp up at the head (so
DVE starts ~5us earlier) and shrink at the tail (so the post-last-DMA
dependency chain is short). The fp32 ring slot is freed by the sub
alone, letting the DMA stream run n_bufs tiles ahead. Bass's two
__init__ all-engine barriers are stripped (saves ~1.5us; the one real
dependency — gpsimd const memsets before ACT bias reads — is re-fenced
with boot_sem). Accumulator tensors are pre-zeroed so any flush race
degrades to ~1e-4 error instead of garbage, and kernel() additionally
sanity-gates the device result against a host subsample estimate with
retry + exact-host fallback.

Older variants (build_gram_kernel, build_gram_kernel_raw,
build_gram_kernel_v3) are kept for reference only.
"""

import numpy as np

import concourse.bass as bass
import concourse.bacc as bacc
import concourse.mybir as mybir
from concourse import tile
from concourse.bass_utils import run_bass_kernel_spmd

N_CORES = 8
B_FULL = 8388608
P = 128

_PAIRS = [(0, 1), (0, 2), (1, 2)]


def build_gram_kernel(n_rows: int, n_tiles: int, use_act: bool = True):
    """Build the per-core Bass module.

    Input: pt [2, n_rows, 3] f32 (pred stacked with targ)
    Output: partials [128, 6 * n_tiles] f32
        col t*3+i            : sum over this tile/partition of d_i^2
        col 3*n_tiles + t*3+k: sum of d_i*d_j for pair k in _PAIRS
    """
    assert n_rows % (P * n_tiles) == 0
    r = n_rows // (P * n_tiles)  # rows per partition per tile
    m = 3 * r                    # flat f32 elements per partition per tile
    f32 = mybir.dt.float32

    # Bacc (not plain Bass): its compile() pass legalizes semaphore waits
    # (each TRN2 instruction holds at most one wait slot).
    nc = bacc.Bacc("TRN2", target_bir_lowering=False, debug=False)
    pt = nc.dram_tensor("pt", [2, n_rows, 3], f32, kind="ExternalInput")
    out = nc.dram_tensor("partials", [P, 6 * n_tiles], f32, kind="ExternalOutput")

    # [t][p][w(2), m] — per tile/partition: pred chunk and targ chunk, each
    # m contiguous f32 in DRAM.
    pt_v = pt[:].rearrange("w (t p r) c -> t p w (r c)", t=n_tiles, p=P)

    with tile.TileContext(nc) as tc:
        with (
            tc.tile_pool(name="io", bufs=3) as io_pool,
            tc.tile_pool(name="dve_scr", bufs=2) as dve_scr,
            tc.tile_pool(name="act_scr", bufs=2) as act_scr,
            tc.tile_pool(name="acc", bufs=1) as acc_pool,
        ):
            acc_sq = acc_pool.tile([P, 3 * n_tiles], f32)
            acc_cr = acc_pool.tile([P, 3 * n_tiles], f32)

            for t in range(n_tiles):
                buf = io_pool.tile([P, 2 * m], f32, tag="buf")
                nc.sync.dma_start(
                    out=buf[:].rearrange("p (w m) -> p w m", w=2),
                    in_=pt_v[t],
                )

                # In-place: d = pred - targ, overwriting the pred half.
                nc.vector.tensor_tensor(
                    out=buf[:, 0:m],
                    in0=buf[:, 0:m],
                    in1=buf[:, m : 2 * m],
                    op=mybir.AluOpType.subtract,
                )
                d3 = buf[:, 0:m].rearrange("p (r c) -> p c r", c=3)

                # Diagonal sums on the scalar engine (Square + accum_out),
                # overlapping with the DVE cross-products.
                if use_act:
                    for i in range(3):
                        sq = act_scr.tile([P, r], f32, tag="sq")
                        nc.scalar.activation(
                            out=sq[:],
                            in_=d3[:, i, :],
                            func=mybir.ActivationFunctionType.Square,
                            accum_out=acc_sq[:, t * 3 + i : t * 3 + i + 1],
                        )
                else:
                    for i in range(3):
                        sq = dve_scr.tile([P, r], f32, tag="pr")
                        nc.vector.scalar_tensor_tensor(
                            out=sq[:],
                            in0=d3[:, i, :],
                            scalar=1.0,
                            in1=d3[:, i, :],
                            op0=mybir.AluOpType.mult,
                            op1=mybir.AluOpType.mult,
                            accum_out=acc_sq[:, t * 3 + i : t * 3 + i + 1],
                        )
                # Cross sums: fused multiply+reduce on DVE
                # (scalar_tensor_tensor: out = (in0 * 1.0) * in1, accum = sum).
                for k, (i, j) in enumerate(_PAIRS):
                    pr = dve_scr.tile([P, r], f32, tag="pr")
                    nc.vector.scalar_tensor_tensor(
                        out=pr[:],
                        in0=d3[:, i, :],
                        scalar=1.0,
                        in1=d3[:, j, :],
                        op0=mybir.AluOpType.mult,
                        op1=mybir.AluOpType.mult,
                        accum_out=acc_cr[:, t * 3 + k : t * 3 + k + 1],
                    )

            nc.sync.dma_start(out=out[:, 0 : 3 * n_tiles], in_=acc_sq[:])
            nc.sync.dma_start(out=out[:, 3 * n_tiles : 6 * n_tiles], in_=acc_cr[:])

    nc.compile()
    return nc


def build_gram_kernel_raw(n_rows: int, n_tiles: int = 32, n_bufs: int = 24,
                          group: int = 4, skip_exit_barrier: bool = True):
    """Raw-Bacc variant: manual semaphores, no TileContext.

    Skips Tile's prologue/epilogue (drain + two all-engine EVSEM
    barriers, ~16 us) — the only sync needed is a three-semaphore chain:
    DMA loads (one HWDGE ring) -> DVE -> ACT.

    The ring of tile buffers lives in ONE SBUF tensor so the fused
    multiply-reduces can span `group` consecutive tiles with a single
    instruction (free-dim AP [group, r]) — amortizing the per-op fixed
    cost and the accumulator-drain, which keeps both compute engines
    well under the DMA pace.

    Input: pt [2, n_rows, 3] f32. Output: partials [128, 6 * n_groups]
    (same slot layout as build_gram_kernel, with n_groups slots).
    """
    assert n_tiles % group == 0 and n_bufs % group == 0
    assert n_rows % (P * n_tiles) == 0
    n_groups = n_tiles // group
    r = n_rows // (P * n_tiles)
    m = 3 * r
    f32 = mybir.dt.float32

    nc = bacc.Bacc("TRN2", target_bir_lowering=False, debug=False)
    pt = nc.dram_tensor("pt", [2, n_rows, 3], f32, kind="ExternalInput")
    out = nc.dram_tensor("partials", [P, 6 * n_groups], f32, kind="ExternalOutput")
    pt_v = pt[:].rearrange("w (t p r) c -> t p w (r c)", t=n_tiles, p=P)

    ring = nc.alloc_sbuf_tensor("ring", [P, n_bufs * 2 * m], f32).ap()

    def buf(t):
        s = t % n_bufs
        return ring[:, s * 2 * m : (s + 1) * 2 * m]

    def dgroup(g, i):
        # component i of the diff halves of tiles 4g..4g+3: [128, group, r]
        s0 = (g * group) % n_bufs
        w = ring[:, s0 * 2 * m : (s0 + group) * 2 * m]
        return w.rearrange("p (t w r c) -> p t w c r", t=group, w=2, c=3)[:, :, 0, i, :]

    acc_sq = nc.alloc_sbuf_tensor("acc_sq", [P, 3 * n_groups], f32).ap()
    acc_cr = nc.alloc_sbuf_tensor("acc_cr", [P, 3 * n_groups], f32).ap()
    # Rotated scratch (dead stores of the fused ops), 2 groups deep so each
    # group's single stale semaphore wait also covers the scratch WAW from
    # two groups back.
    pr_scrs = [
        nc.alloc_sbuf_tensor(f"pr_scr{k}", [P, group * r], f32).ap() for k in range(6)
    ]
    sq_scrs = [
        nc.alloc_sbuf_tensor(f"sq_scr{k}", [P, group * r], f32).ap() for k in range(6)
    ]

    # One DMA-completion semaphore per ring buffer: a single shared sem
    # would be unsound — each dma_start is split across 16 SDMA engines
    # whose sub-completions interleave across in-flight DMAs.
    dma_sems = [nc.alloc_semaphore(f"dma_sem{i}") for i in range(n_bufs)]
    out_sem = nc.alloc_semaphore("out_sem")
    dve_sem = nc.alloc_semaphore("dve_sem")
    act_sem = nc.alloc_semaphore("act_sem")

    # DVE emission order: subs run ahead; the grouped multiply-reduces for
    # group g are emitted after sub(4g+4) so their drain-wait on the last
    # sub of the group is already satisfied when it executes (DVE writes
    # drain asynchronously). Only the last group trails the final sub.
    dve_order = []
    for t in range(n_tiles):
        dve_order.append(("sub", t))
        if t % group == 0 and t >= group:
            # one sub of stagger after the group's last sub
            dve_order.append(("stt", t // group - 1))
    dve_order.append(("stt", n_groups - 1))
    sub_done, sttg_done = {}, {}
    v = 0
    for kind, x in dve_order:
        if kind == "sub":
            v += 1
            sub_done[x] = v
        else:
            v += 3
            sttg_done[x] = v

    # Output chunks: flush finished accumulator columns while later tiles
    # still stream, so the tail only waits on the last small chunk.
    chunk = max(1, n_groups // 2)
    chunks = [(c, min(c + chunk, n_groups)) for c in range(0, n_groups, chunk)]

    import contextlib

    @contextlib.contextmanager
    def _block():
        # no_gpsimd_drain=True emits per-engine drains explicitly and then a
        # sem-only all-engine butterfly. The butterfly only delays NEFF end
        # (outputs are already fenced by the sequencer's out_sem wait), so
        # optionally no-op it during Block.__exit__.
        with nc.Block(no_gpsimd_drain=True) as blk:
            try:
                yield blk
            finally:
                if skip_exit_barrier:
                    nc.all_engine_barrier = lambda **kw: None
        if skip_exit_barrier:
            del nc.all_engine_barrier  # restore class method

    with _block() as block:

        @block.sync
        def _(sync):
            for t in range(n_tiles):
                if head_dma_on_vector and t < h0:
                    continue  # issued from the vector queue (clears the
                              # NEFF entry barrier ~1.4us before sync)
                if t >= n_bufs:
                    # ring reuse: all consumers of the buffer's previous
                    # occupant (tile t - n_bufs) must be done
                    prev = t - n_bufs
                    sync.wait_ge(dve_sem, sttg_done[prev // group])
                    sync.wait_ge(act_sem, 3 * (prev // group + 1))
                sync.dma_start(
                    out=buf(t).rearrange("p (w m) -> p w m", w=2),
                    in_=pt_v[t],
                ).then_inc(dma_sems[t % n_bufs], 16)
            n_out = 0
            for lo, hi in chunks:
                sync.wait_ge(act_sem, 3 * hi)
                sync.dma_start(
                    out=out[:, 3 * lo : 3 * hi], in_=acc_sq[:, 3 * lo : 3 * hi]
                ).then_inc(out_sem, 16)
                sync.wait_ge(dve_sem, sttg_done[hi - 1])
                sync.dma_start(
                    out=out[:, 3 * (n_groups + lo) : 3 * (n_groups + hi)],
                    in_=acc_cr[:, 3 * lo : 3 * hi],
                ).then_inc(out_sem, 16)
                n_out += 32
            sync.wait_ge(out_sem, n_out)

        @block.vector
        def _(vector):
            for kind, x in dve_order:
                if kind == "sub":
                    b = buf(x)
                    vector.wait_ge(dma_sems[x % n_bufs], 16 * (x // n_bufs + 1))
                    vector.tensor_tensor(
                        out=b[:, 0:m],
                        in0=b[:, 0:m],
                        in1=b[:, m : 2 * m],
                        op=mybir.AluOpType.subtract,
                    ).then_inc(dve_sem, 1)
                else:
                    vector.wait_ge(dve_sem, sub_done[(x + 1) * group - 1])
                    for k, (i, j) in enumerate(_PAIRS):
                        vector.scalar_tensor_tensor(
                            out=pr_scrs[(x % 2) * 3 + k][:].rearrange(
                                "p (t r) -> p t r", t=group
                            ),
                            in0=dgroup(x, i),
                            scalar=1.0,
                            in1=dgroup(x, j),
                            op0=mybir.AluOpType.mult,
                            op1=mybir.AluOpType.mult,
                            accum_out=acc_cr[:, x * 3 + k : x * 3 + k + 1],
                        ).then_inc(dve_sem, 1)

        @block.scalar
        def _(scalar):
            for g in range(n_groups):
                scalar.wait_ge(dve_sem, sub_done[(g + 1) * group - 1])
                if g >= 2:
                    # scratch slot reuse from two groups back
                    scalar.wait_ge(act_sem, 3 * (g - 1))
                for i in range(3):
                    scalar.activation(
                        out=sq_scrs[(g % 2) * 3 + i][:].rearrange(
                            "p (t r) -> p t r", t=group
                        ),
                        in_=dgroup(g, i),
                        func=mybir.ActivationFunctionType.Square,
                        accum_out=acc_sq[:, g * 3 + i : g * 3 + i + 1],
                    ).then_inc(act_sem, 1)

    nc.compile()
    return nc

def _strip_entry_barriers(nc):
    """Remove the two all-engine entry barriers Bass.__init__ emits.

    They serialize ~4us of semaphore round-trips before the first DMA can
    issue. The only cross-engine ordering they provide that this kernel
    needs is gpsimd-const-AP-memset -> ACT-bias-read, which is re-fenced
    explicitly with boot_sem in build_gram_kernel_v3.
    """
    bar = set(nc.barrier_sems)
    blk = nc.main_func.blocks[0]
    drop = []
    for ins in blk.instructions:
        si = getattr(ins, "sync_info", None)
        if si is None:
            continue
        sems = {w.id for w in si.on_wait or []}
        sems |= {u.id for u in si.on_update or []}
        if sems & bar:
            drop.append(ins)
    for ins in drop:
        blk.instructions.remove(ins)
    return len(drop)


def build_gram_kernel_v3(n_rows: int, n_tiles: int = 16, n_bufs: int = 8,
                         group: int = 4, strip_barriers: bool = True,
                         skip_exit_barrier: bool = True):
    """v3: planar-bf16 d + 2x DVE reduces + ACT squares.

    Per tile: DMA both halves -> DVE sub (fp32 in, planar bf16 out:
    component planes x|y|z so reduce operands are unit-stride 2-byte,
    unlocking the DVE 2x perf mode) -> DVE cross-product reduces (grouped
    `group` tiles per instr) + ACT Square reduces (grouped; per-tile for
    the last group so the post-DMA tail stays short).

    The fp32 ring slot is freed by the sub alone (d lives in its own
    full-size buffer), so the DMA stream runs ~n_bufs tiles ahead of
    compute and never stalls on the reduce bursts.

    Output layout [128, 3*n_groups + 3*(n_groups-1) + 3*group]:
      cols 0 .. 3*n_groups-1: cross sums (group g, pair k at 3g+k)
      then squares: full groups 0..n_groups-2 (3 each), then the last
      group's tiles individually (3 each).
    """
    assert n_rows % (P * n_tiles) == 0 and n_tiles % group == 0
    r = n_rows // (P * n_tiles)
    m = 3 * r
    n_groups = n_tiles // group
    full_sq = n_groups - 1           # square-groups emitted grouped
    tail0 = full_sq * group          # first per-tile-squares tile
    ncr = 3 * n_groups
    nsq = 3 * full_sq + 3 * group
    f32, bf16 = mybir.dt.float32, mybir.dt.bfloat16

    nc = bacc.Bacc("TRN2", target_bir_lowering=False, debug=False)
    if strip_barriers:
        _strip_entry_barriers(nc)
    pt = nc.dram_tensor("pt", [2, n_rows, 3], f32, kind="ExternalInput")
    out = nc.dram_tensor("partials", [P, ncr + nsq], f32, kind="ExternalOutput")
    pt_v = pt[:].rearrange("w (t p r) c -> t p w (r c)", t=n_tiles, p=P)

    ring = nc.alloc_sbuf_tensor("ring", [P, n_bufs * 2 * m], f32).ap()
    d_all = nc.alloc_sbuf_tensor("d_all", [P, n_tiles * m], bf16).ap()
    d_t = d_all.rearrange("p (t c r) -> p t c r", t=n_tiles, c=3)
    acc_cr = nc.alloc_sbuf_tensor("acc_cr", [P, ncr], f32).ap()
    acc_sq = nc.alloc_sbuf_tensor("acc_sq", [P, nsq], f32).ap()
    # Dead stores of the fused reduces; single slot per engine (each
    # engine executes its own stream in order, so WAW is safe).
    cr_scr = nc.alloc_sbuf_tensor("cr_scr", [P, group * r], bf16).ap()
    sq_scr = nc.alloc_sbuf_tensor("sq_scr", [P, group * r], bf16).ap()

    dma_sems = [nc.alloc_semaphore(f"dma{i}") for i in range(n_bufs)]
    sub_sem = nc.alloc_semaphore("sub_sem")
    red_sem = nc.alloc_semaphore("red_sem")
    act_sem = nc.alloc_semaphore("act_sem")
    out_sem = nc.alloc_semaphore("out_sem")
    boot_sem = nc.alloc_semaphore("boot_sem")

    def dcomp(t0, nt, i):
        # component i of tiles t0..t0+nt-1: [128, nt, r] unit-stride bf16
        v = d_t[:, t0 : t0 + nt, i, :]
        return v

    import contextlib

    @contextlib.contextmanager
    def _block():
        with nc.Block(no_gpsimd_drain=True) as blk:
            try:
                yield blk
            finally:
                if skip_exit_barrier:
                    nc.all_engine_barrier = lambda **kw: None
        if skip_exit_barrier:
            del nc.all_engine_barrier  # restore class method

    with _block() as block:

        @block.gpsimd
        def _(gpsimd):
            # Const-AP memsets (ACT bias) are earlier in gpsimd's stream;
            # this inc publishes their completion to the scalar queue.
            gpsimd.sem_inc(boot_sem, 1)

        @block.sync
        def _(sync):
            for t in range(n_tiles):
                if head_dma_on_vector and t < h0:
                    continue  # issued from the vector queue (clears the
                              # NEFF entry barrier ~1.4us before sync)
                if t >= n_bufs:
                    # ring slot free once its previous occupant was subbed
                    sync.wait_ge(sub_sem, t - n_bufs + 1)
                sync.dma_start(
                    out=ring[:, (t % n_bufs) * 2 * m : (t % n_bufs + 1) * 2 * m]
                    .rearrange("p (w m) -> p w m", w=2),
                    in_=pt_v[t],
                ).then_inc(dma_sems[t % n_bufs], 16)
            # accumulator flush: big chunks early, last-group slivers at end
            sync.wait_ge(red_sem, 3 * (n_groups - 1))
            sync.dma_start(
                out=out[:, 0 : 3 * (n_groups - 1)],
                in_=acc_cr[:, 0 : 3 * (n_groups - 1)],
            ).then_inc(out_sem, 16)
            sync.wait_ge(act_sem, 3 * full_sq)
            sync.dma_start(
                out=out[:, ncr : ncr + 3 * full_sq],
                in_=acc_sq[:, 0 : 3 * full_sq],
            ).then_inc(out_sem, 16)
            sync.wait_ge(red_sem, 3 * n_groups)
            sync.dma_start(
                out=out[:, 3 * (n_groups - 1) : ncr],
                in_=acc_cr[:, 3 * (n_groups - 1) : ncr],
            ).then_inc(out_sem, 16)
            sync.wait_ge(act_sem, nsq)
            sync.dma_start(
                out=out[:, ncr + 3 * full_sq : ncr + nsq],
                in_=acc_sq[:, 3 * full_sq : nsq],
            ).then_inc(out_sem, 16)
            sync.wait_ge(out_sem, 64)

        @block.vector
        def _(vector):
            for t in range(n_tiles):
                s = t % n_bufs
                buf = ring[:, s * 2 * m : (s + 1) * 2 * m]
                vector.wait_ge(dma_sems[s], 16 * (t // n_bufs + 1))
                # d = pred - targ, downcast to bf16, scattered into
                # component planes (write AP [r, 3] w/ strides [1, r])
                vector.tensor_tensor(
                    out=d_all[:, t * m : (t + 1) * m].rearrange(
                        "p (c r) -> p r c", c=3
                    ),
                    in0=buf[:, 0:m],
                    in1=buf[:, m : 2 * m],
                    op=mybir.AluOpType.subtract,
                ).then_inc(sub_sem, 1)
                if t % group == group - 1:
                    g = t // group
                    for k, (i, j) in enumerate(_PAIRS):
                        vector.scalar_tensor_tensor(
                            out=cr_scr[:].rearrange("p (t r) -> p t r", t=group),
                            in0=dcomp(g * group, group, i),
                            scalar=1.0,
                            in1=dcomp(g * group, group, j),
                            op0=mybir.AluOpType.mult,
                            op1=mybir.AluOpType.mult,
                            accum_out=acc_cr[:, g * 3 + k : g * 3 + k + 1],
                        ).then_inc(red_sem, 1)

        @block.scalar
        def _(scalar):
            scalar.wait_ge(boot_sem, 1)
            for g in range(full_sq):
                scalar.wait_ge(sub_sem, group * (g + 1))
                for i in range(3):
                    scalar.activation(
                        out=sq_scr[:].rearrange("p (t r) -> p t r", t=group),
                        in_=dcomp(g * group, group, i),
                        func=mybir.ActivationFunctionType.Square,
                        accum_out=acc_sq[:, g * 3 + i : g * 3 + i + 1],
                    ).then_inc(act_sem, 1)
            for w, t in enumerate(range(tail0, n_tiles)):
                scalar.wait_ge(sub_sem, t + 1)
                for i in range(3):
                    c = 3 * full_sq + w * 3 + i
                    scalar.activation(
                        out=sq_scr[:, 0:r],
                        in_=dcomp(t, 1, i),
                        func=mybir.ActivationFunctionType.Square,
                        accum_out=acc_sq[:, c : c + 1],
                    ).then_inc(act_sem, 1)

    nc.compile()
    nc._v3_meta = (n_tiles, group)
    return nc


def build_gram_kernel_v4(n_rows: int, bulk_r: int = 512, n_bufs: int = 12,
                         group: int = 4, gp_stride: int = 0,
                         head_rs: tuple = (128, 256, 256, 384),
                         tail_rs: tuple = (256, 128, 64, 32, 32),
                         dpad: int = 3, head_dma_on_vector: bool = True,
                         strip_barriers: bool = True,
                         skip_exit_barrier: bool = True):
    """v4: interleaved-bf16 d, measured-cost engine mix, shrinking tail.

    Measured HW rates (ns per 128-wide column): DVE sub fp32->bf16 unit
    1.28; DVE stt reduce bf16 stride-3 1.32; ACT Square ~1.0-1.4 + 740
    fixed; GPSIMD sub ~3.5. Writes must be unit-stride (scatter = 4.6x);
    strided reads are cheap. So d stays row-interleaved bf16.

    - bulk tiles of r=bulk_r rows/partition; every gp_stride-th bulk tile's
      sub runs on GPSIMD to keep DVE under the DMA pace.
    - cross-products: DVE stt grouped over `group` consecutive bulk tiles.
    - squares: ACT, same grouping; tail tiles per-tile; r<=32 tails on DVE.
    - tail tiles shrink so the post-last-DMA dependency chain is tiny.
    """
    R = n_rows // P
    assert n_rows % P == 0
    bulk_n = (R - sum(head_rs) - sum(tail_rs)) // bulk_r
    assert sum(head_rs) + bulk_n * bulk_r + sum(tail_rs) == R
    rs = list(head_rs) + [bulk_r] * bulk_n + list(tail_rs)
    n_tiles = len(rs)
    h0 = len(head_rs)               # first bulk tile index
    t0_tail = h0 + bulk_n           # first tail tile index
    cum = [0]
    for r in rs:
        cum.append(cum[-1] + r)
    # bulk groups: chunks of `group` (absolute tile indices)
    groups = [list(range(s, min(s + group, t0_tail)))
              for s in range(h0, t0_tail, group)]
    group_last = {g[-1]: g for g in groups}
    # per-tile (ungrouped) reduce tiles: head + tail
    per_tile = set(range(0, h0)) | set(range(t0_tail, n_tiles))
    # every gp_stride-th bulk tile's sub runs on GPSIMD (0 = none)
    gp_tiles = (set(range(h0, t0_tail, gp_stride)) if gp_stride else set())
    f32, bf16 = mybir.dt.float32, mybir.dt.bfloat16

    nc = bacc.Bacc("TRN2", target_bir_lowering=False, debug=False)
    if strip_barriers:
        _strip_entry_barriers(nc)
    pt = nc.dram_tensor("pt", [2, n_rows, 3], f32, kind="ExternalInput")

    # per-tile engine assignment of the sub + cumulative sem targets
    dve_idx, gp_idx = {}, {}
    for t in range(n_tiles):
        if t in gp_tiles:
            gp_idx[t] = len(gp_idx)
        else:
            dve_idx[t] = len(dve_idx)

    def sub_waits(last_t):
        """(sub_sem target, gsub_sem target) covering tiles 0..last_t."""
        d = sum(1 for t, i in dve_idx.items() if t <= last_t)
        g = sum(1 for t, i in gp_idx.items() if t <= last_t)
        return d, g

    # reduce slot counts (order finalized at emission)
    n_dve = 3 * len(groups) + 3 * len(per_tile) + 3 * sum(
        1 for t in per_tile if rs[t] <= 32)
    plan_act = []
    for t in sorted(per_tile):
        if t < h0 and rs[t] > 32:
            for i in range(3):
                plan_act.append((i, i))
    for g_tiles in groups:
        for i in range(3):
            plan_act.append((i, i))
    for t in sorted(per_tile):
        if t >= t0_tail and rs[t] > 32:
            for i in range(3):
                plan_act.append((i, i))
    n_act = len(plan_act)
    out = nc.dram_tensor("partials", [P, n_dve + n_act], f32,
                         kind="ExternalOutput")

    m_bulk = 3 * bulk_r
    ring = nc.alloc_sbuf_tensor("ring", [P, n_bufs * 2 * m_bulk], f32).ap()
    d_all = nc.alloc_sbuf_tensor("d_all", [P, dpad * R], bf16).ap()
    acc_dve = nc.alloc_sbuf_tensor("acc_dve", [P, n_dve], f32).ap()
    acc_act = nc.alloc_sbuf_tensor("acc_act", [P, n_act], f32).ap()
    cr_scr = nc.alloc_sbuf_tensor("cr_scr", [P, group * bulk_r], bf16).ap()
    sq_scr = nc.alloc_sbuf_tensor("sq_scr", [P, group * bulk_r], bf16).ap()

    dma_sems = [nc.alloc_semaphore(f"dma{i}") for i in range(n_bufs)]
    sub_sem = nc.alloc_semaphore("sub_sem")
    gsub_sem = nc.alloc_semaphore("gsub_sem")
    red_sem = nc.alloc_semaphore("red_sem")
    act_sem = nc.alloc_semaphore("act_sem")
    out_sem = nc.alloc_semaphore("out_sem")
    boot_sem = nc.alloc_semaphore("boot_sem")

    def ring_slot(t):
        s = t % n_bufs
        return ring[:, s * 2 * m_bulk : s * 2 * m_bulk + 2 * 3 * rs[t]]

    def pt_tile(t):
        sl = pt[:, cum[t] * P : cum[t + 1] * P, :]
        return sl.rearrange("w (p r) c -> p w (r c)", p=P)

    def dseg(t0, nt, i):
        # component i of tiles t0..t0+nt-1 (equal r), stride-dpad reads
        v = d_all[:, dpad * cum[t0] : dpad * cum[t0 + nt]]
        return v.rearrange("p (t r c) -> p t c r", t=nt, c=dpad)[:, :, i, :]

    def dsub_out(t):
        # write view for the sub: rows of 3 packed comps, dpad-elem row pitch
        v = d_all[:, dpad * cum[t] : dpad * cum[t + 1]]
        if dpad == 3:
            return v
        return v.rearrange("p (r c) -> p r c", c=dpad)[:, :, 0:3]

    # early-flush boundaries: head reduces + all-but-last bulk group are
    # guaranteed emitted before any tail reduce; ACT head+bulk likewise
    red_early = 3 * h0 + 3 * (len(groups) - 1)
    act_bulk = 3 * h0 + 3 * len(groups)
    _red_pen = max(red_early, n_dve - 6)
    _act_pen = max(act_bulk, n_act - 3)
    n_flush_dve = sum(1 for lo, hi in ((0, red_early), (red_early, _red_pen),
                                       (_red_pen, n_dve)) if hi > lo)
    act_chunks = [(act_bulk, 0, act_bulk), (_act_pen, act_bulk, _act_pen),
                  (n_act, _act_pen, n_act)]
    act_chunks = [c for c in act_chunks if c[2] > c[1]]
    n_flush_act = len(act_chunks)

    import contextlib

    @contextlib.contextmanager
    def _block():
        with nc.Block(no_gpsimd_drain=True) as blk:
            try:
                yield blk
            finally:
                if skip_exit_barrier:
                    nc.all_engine_barrier = lambda **kw: None
        if skip_exit_barrier:
            del nc.all_engine_barrier

    with _block() as block:

        @block.sync
        def _(sync):
            for t in range(n_tiles):
                if head_dma_on_vector and t < h0:
                    continue  # issued from the vector queue (clears the
                              # NEFF entry barrier ~1.4us before sync)
                if t >= n_bufs:
                    u = t - n_bufs
                    if u in gp_idx:
                        sync.wait_ge(gsub_sem, gp_idx[u] + 1)
                    else:
                        sync.wait_ge(sub_sem, dve_idx[u] + 1)
                sync.dma_start(
                    out=ring_slot(t).rearrange("p (w m) -> p w m", w=2),
                    in_=pt_tile(t),
                ).then_inc(dma_sems[t % n_bufs], 16)
            # acc_dve flushes stay here; acc_act flushes issue from the
            # scalar queue (ACT finishes ~2.5us before DVE, so they then
            # complete during the DVE tail instead of serializing after it)
            red_pen = max(red_early, n_dve - 6)   # all but last tail tile
            for val, lo, hi in ((red_early, 0, red_early),
                                (red_pen, red_early, red_pen),
                                (n_dve, red_pen, n_dve)):
                if hi <= lo:
                    continue
                sync.wait_ge(red_sem, val)
                sync.dma_start(out=out[:, lo:hi],
                               in_=acc_dve[:, lo:hi]).then_inc(out_sem, 16)
            sync.wait_ge(out_sem, 16 * (n_flush_dve + n_flush_act))

        @block.vector
        def _(vector):
            red_c = 0
            plan_dyn = []
            pending = []  # deferred bulk-group reduces, drained 1-per-sub

            def emit_one(t0, nt, i, j):
                nonlocal red_c
                r = rs[t0]
                vector.scalar_tensor_tensor(
                    out=cr_scr[:, 0 : nt * r].rearrange(
                        "p (t r) -> p t r", t=nt),
                    in0=dseg(t0, nt, i),
                    scalar=1.0,
                    in1=dseg(t0, nt, j),
                    op0=mybir.AluOpType.mult,
                    op1=mybir.AluOpType.mult,
                    accum_out=acc_dve[:, red_c : red_c + 1],
                ).then_inc(red_sem, 1)
                plan_dyn.append((i, j))
                red_c += 1

            for t in range(n_tiles):
                if t not in gp_tiles:
                    buf = ring_slot(t)
                    mt = 3 * rs[t]
                    vector.wait_ge(dma_sems[t % n_bufs], 16 * (t // n_bufs + 1))
                    vector.tensor_tensor(
                        out=dsub_out(t),
                        in0=buf[:, 0:mt],
                        in1=buf[:, mt : 2 * mt],
                        op=mybir.AluOpType.subtract,
                    ).then_inc(sub_sem, 1)
                    # spread deferred group reduces between subs so the DMA
                    # ring (freed by subs) never stalls on a reduce burst
                    if pending:
                        emit_one(*pending.pop(0))
                if t in group_last:
                    g_tiles = group_last[t]
                    _, gw = sub_waits(g_tiles[-1])
                    if gw:
                        vector.wait_ge(gsub_sem, gw)
                    for (i, j) in _PAIRS:
                        pending.append((g_tiles[0], len(g_tiles), i, j))
                elif t in per_tile:
                    for (i, j) in _PAIRS:
                        emit_one(t, 1, i, j)
                    if rs[t] <= 32:
                        for i in range(3):
                            emit_one(t, 1, i, i)
            while pending:
                emit_one(*pending.pop(0))
            nc._v4_plan_dve = plan_dyn

        @block.scalar
        def _(scalar):
            if head_dma_on_vector:
                # Scalar clears the NEFF entry-barrier chain ~2us before
                # Sync; issuing the head-tile loads here starts the stream
                # (and so the first sub) earlier. Ring slots are fresh, no
                # waits needed; the const-AP fence only matters for the
                # activations below.
                for t in range(h0):
                    scalar.dma_start(
                        out=ring_slot(t).rearrange("p (w m) -> p w m", w=2),
                        in_=pt_tile(t),
                    ).then_inc(dma_sems[t % n_bufs], 16)
            scalar.wait_ge(boot_sem, 1)
            act_c = 0

            def emit_squares(t0, nt):
                nonlocal act_c
                r = rs[t0]
                for i in range(3):
                    scalar.activation(
                        out=sq_scr[:, 0 : nt * r].rearrange(
                            "p (t r) -> p t r", t=nt),
                        in_=dseg(t0, nt, i),
                        func=mybir.ActivationFunctionType.Square,
                        accum_out=acc_act[:, act_c : act_c + 1],
                    ).then_inc(act_sem, 1)
                    act_c += 1

            def sq_waits(last_t):
                d, g = sub_waits(last_t)
                scalar.wait_ge(sub_sem, d)
                if g:
                    scalar.wait_ge(gsub_sem, g)

            for t in sorted(per_tile):
                if t < h0 and rs[t] > 32:
                    sq_waits(t)
                    emit_squares(t, 1)
            for g_tiles in groups:
                sq_waits(g_tiles[-1])
                emit_squares(g_tiles[0], len(g_tiles))
            for t in sorted(per_tile):
                if t >= t0_tail and rs[t] > 32:
                    sq_waits(t)
                    emit_squares(t, 1)
            for val, lo, hi in act_chunks:
                scalar.wait_ge(act_sem, val)
                scalar.dma_start(out=out[:, n_dve + lo : n_dve + hi],
                                 in_=acc_act[:, lo:hi]).then_inc(out_sem, 16)

        @block.gpsimd
        def _(gpsimd):
            # Zero the accumulator tensors so a (never-observed, but cheap
            # to insure against) flush-before-drain race reads zeros - a
            # ~1e-4 relative error - instead of stale SBUF garbage.
            gpsimd.memset(acc_dve, 0.0)
            gpsimd.memset(acc_act, 0.0)
            gpsimd.sem_inc(boot_sem, 1)
            for t in sorted(gp_tiles):
                buf = ring_slot(t)
                mt = 3 * rs[t]
                gpsimd.wait_ge(dma_sems[t % n_bufs], 16 * (t // n_bufs + 1))
                gpsimd.tensor_tensor(
                    out=dsub_out(t),
                    in0=buf[:, 0:mt],
                    in1=buf[:, mt : 2 * mt],
                    op=mybir.AluOpType.subtract,
                ).then_inc(gsub_sem, 1)

    nc.compile()
    assert len(nc._v4_plan_dve) == n_dve
    nc._v4_plan = (nc._v4_plan_dve, plan_act)
    return nc


def build_diag_kernel_v5(n_rows: int, bulk_r: int = 512, n_bufs: int = 14,
                         n_dbufs: int = 8,
                         head_rs: tuple = (128, 128, 256),
                         tail_rs: tuple = (384, 256, 192, 128, 64),
                         n_sq_dve: int = 2,
                         strip_barriers: bool = True,
                         skip_exit_barrier: bool = True):
    """v5: diagonal-sigma fast path — per-tile sum of squared differences.

    For sigma = c*I (the shipped input), the loss needs only
    S = sum_b ||p_b - t_b||^2; no cross products. Per tile: DMA both
    halves -> DVE sub (fp32 in, bf16 interleaved out, unit-stride write)
    -> Square with accum_out (one fp32 partial column per tile), on ACT
    for most tiles. Both engines run at ~50% of the DMA pace, so the
    kernel is purely DMA-bound: the 16-engine pool sustains ~415 GB/s.

    Head: small ramp tiles issued from the gpsimd/scalar queues (they
    clear the NEFF boot chain before sync) — small, so the brief
    3-queue pool contention costs little. Tail: ramp-down sizes, with
    the squares of the last 2*n_sq_dve tiles alternating DVE/ACT so the
    post-last-load catch-up runs on both engines in parallel.

    Output: partials [128, n_tiles] (ACT tiles then DVE tiles, by the
    _v5_order attr); host sums everything (f64).
    """
    R = n_rows // P
    assert n_rows % P == 0
    bulk_n = (R - sum(head_rs) - sum(tail_rs)) // bulk_r
    assert sum(head_rs) + bulk_n * bulk_r + sum(tail_rs) == R
    rs = list(head_rs) + [bulk_r] * bulk_n + list(tail_rs)
    n_tiles = len(rs)
    cum = [0]
    for r in rs:
        cum.append(cum[-1] + r)
    m_bulk = 3 * max(rs)
    f32, bf16 = mybir.dt.float32, mybir.dt.bfloat16

    # squares of the last 2*n_sq_dve tiles alternate DVE/ACT (DVE takes
    # the even offsets from the end: ..., t-4, t-2, last)
    sq_dve = {n_tiles - 1 - 2 * k for k in range(n_sq_dve)}
    act_tiles = [t for t in range(n_tiles) if t not in sq_dve]
    dve_tiles = sorted(sq_dve)
    n_act, n_dve = len(act_tiles), len(dve_tiles)
    acol = {t: i for i, t in enumerate(act_tiles)}
    vcol = {t: i for i, t in enumerate(dve_tiles)}
    # act_sem value after the square of tile u (ACT tiles only)
    act_done = {t: i + 1 for i, t in enumerate(act_tiles)}
    # sub_sem value after DVE finished tile t (sub, plus square if DVE tile)
    sub_done = {}
    _v = 0
    for _t in range(n_tiles):
        _v += 2 if _t in sq_dve else 1
        sub_done[_t] = _v
    n_flush = (1 if n_act > 1 else 0) + 1 + (1 if n_dve else 0)

    nc = bacc.Bacc("TRN2", target_bir_lowering=False, debug=False)
    if strip_barriers:
        _strip_entry_barriers(nc)
    pt = nc.dram_tensor("pt", [2, n_rows, 3], f32, kind="ExternalInput")
    out = nc.dram_tensor("partials", [P, n_tiles], f32, kind="ExternalOutput")

    ring = nc.alloc_sbuf_tensor("ring", [P, n_bufs * 2 * m_bulk], f32).ap()
    dbuf = nc.alloc_sbuf_tensor("dbuf", [P, n_dbufs * m_bulk], bf16).ap()
    acc_a = nc.alloc_sbuf_tensor("acc_a", [P, max(n_act, 1)], f32).ap()
    acc_v = nc.alloc_sbuf_tensor("acc_v", [P, max(n_dve, 1)], f32).ap()
    sq_scr = nc.alloc_sbuf_tensor("sq_scr", [P, m_bulk], bf16).ap()
    vq_scr = nc.alloc_sbuf_tensor("vq_scr", [P, m_bulk], bf16).ap()

    dma_sems = [nc.alloc_semaphore(f"dma{i}") for i in range(n_bufs)]
    sub_sem = nc.alloc_semaphore("sub_sem")
    act_sem = nc.alloc_semaphore("act_sem")
    out_sem = nc.alloc_semaphore("out_sem")
    boot_sem = nc.alloc_semaphore("boot_sem")

    def ring_slot(t):
        s = t % n_bufs
        return ring[:, s * 2 * m_bulk : s * 2 * m_bulk + 2 * 3 * rs[t]]

    def d_slot(t):
        s = t % n_dbufs
        return dbuf[:, s * m_bulk : s * m_bulk + 3 * rs[t]]

    def pt_tile(t):
        sl = pt[:, cum[t] * P : cum[t + 1] * P, :]
        return sl.rearrange("w (p r) c -> p w (r c)", p=P)

    n_head = len(head_rs)

    def issue_load(q, t):
        q.dma_start(
            out=ring_slot(t).rearrange("p (w m) -> p w m", w=2),
            in_=pt_tile(t),
        ).then_inc(dma_sems[t % n_bufs], 16)

    import contextlib

    @contextlib.contextmanager
    def _block():
        with nc.Block(no_gpsimd_drain=True) as blk:
            try:
                yield blk
            finally:
                if skip_exit_barrier:
                    nc.all_engine_barrier = lambda **kw: None
        if skip_exit_barrier:
            del nc.all_engine_barrier

    with _block() as block:

        @block.sync
        def _(sync):
            for t in range(2, n_tiles):
                if t >= n_bufs:
                    # ring slot free once its previous occupant was subbed
                    u = t - n_bufs
                    sync.wait_ge(sub_sem, sub_done[u] - (1 if u in sq_dve else 0))
                issue_load(sync, t)
            if n_dve:
                # DVE squares all drained once the last DVE tile's pair ran;
                # this flush overlaps scalar's final acc_a sliver flush.
                sync.wait_ge(sub_sem, sub_done[dve_tiles[-1]])
                sync.dma_start(
                    out=out[:, n_act : n_act + n_dve], in_=acc_v[:, 0:n_dve]
                ).then_inc(out_sem, 16)
            sync.wait_ge(out_sem, 16 * n_flush)

        @block.vector
        def _(vector):
            for t in range(n_tiles):
                vector.wait_ge(dma_sems[t % n_bufs], 16 * (t // n_bufs + 1))
                u = t - n_dbufs
                if u >= 0 and u not in sq_dve:
                    # d slot free once its previous occupant was squared
                    vector.wait_ge(act_sem, act_done[u])
                buf = ring_slot(t)
                mt = 3 * rs[t]
                vector.tensor_tensor(
                    out=d_slot(t),
                    in0=buf[:, 0:mt],
                    in1=buf[:, mt : 2 * mt],
                    op=mybir.AluOpType.subtract,
                ).then_inc(sub_sem, 1)
                if t in sq_dve:
                    c = vcol[t]
                    vector.scalar_tensor_tensor(
                        out=vq_scr[:, 0 : 3 * rs[t]],
                        in0=d_slot(t),
                        scalar=1.0,
                        in1=d_slot(t),
                        op0=mybir.AluOpType.mult,
                        op1=mybir.AluOpType.mult,
                        accum_out=acc_v[:, c : c + 1],
                    ).then_inc(sub_sem, 1)

        @block.scalar
        def _(scalar):
            # One small head tile on this queue: it drains before the sync
            # stream builds up, buying the 0.6us the sync engine's boot lags.
            issue_load(scalar, 1)
            scalar.wait_ge(boot_sem, 1)
            flush0 = act_tiles[-2] if n_act > 1 else None
            for t in act_tiles:
                scalar.wait_ge(sub_sem, sub_done[t])
                c = acol[t]
                scalar.activation(
                    out=sq_scr[:, 0 : 3 * rs[t]],
                    in_=d_slot(t),
                    func=mybir.ActivationFunctionType.Square,
                    accum_out=acc_a[:, c : c + 1],
                ).then_inc(act_sem, 1)
                if t == flush0:
                    scalar.wait_ge(act_sem, n_act - 1)
                    scalar.dma_start(
                        out=out[:, 0 : n_act - 1], in_=acc_a[:, 0 : n_act - 1]
                    ).then_inc(out_sem, 16)
            scalar.wait_ge(act_sem, n_act)
            lo = max(n_act - 2, 0) if flush0 is not None else 0
            scalar.dma_start(
                out=out[:, lo:n_act], in_=acc_a[:, lo:n_act]
            ).then_inc(out_sem, 16)

        @block.gpsimd
        def _(gpsimd):
            issue_load(gpsimd, 0)
            # Zero accs so a flush-before-drain race reads zeros, not garbage.
            gpsimd.memset(acc_a, 0.0)
            gpsimd.memset(acc_v, 0.0)
            gpsimd.sem_inc(boot_sem, 1)

    nc.compile()
    nc._v5_order = (act_tiles, dve_tiles)
    return nc


def gram_from_partials_v4(partials: np.ndarray, plan) -> np.ndarray:
    plan_dve, plan_act = plan
    s = partials.astype(np.float64).reshape(-1, partials.shape[-1]).sum(axis=0)
    g = np.zeros((3, 3), dtype=np.float64)
    for c, (i, j) in enumerate(plan_dve + plan_act):
        if i == j:
            g[i, i] += s[c]
        else:
            g[i, j] += s[c]
            g[j, i] += s[c]
    return g


def gram_from_partials_v3(partials: np.ndarray, n_tiles: int, group: int) -> np.ndarray:
    n_groups = n_tiles // group
    ncr = 3 * n_groups
    s = partials.astype(np.float64).reshape(-1, partials.shape[-1]).sum(axis=0)
    cr = s[:ncr].reshape(-1, 3).sum(axis=0)
    sq = s[ncr:].reshape(-1, 3).sum(axis=0)
    g = np.empty((3, 3), dtype=np.float64)
    g[0, 0], g[1, 1], g[2, 2] = sq
    for k, (i, j) in enumerate(_PAIRS):
        g[i, j] = g[j, i] = cr[k]
    return g


_NC_CACHE: dict[tuple, object] = {}


def _get_nc(n_rows: int, n_tiles: int, use_act: bool, raw: bool = False,
            group: int = 4, version: int = 4, n_bufs: int = 12,
            strip_barriers: bool = True, gp_stride: int = 0,
            bulk_r: int = 512, tail_rs: tuple = (256, 128, 64, 32, 32),
            head_rs: tuple = (128, 256, 256, 384), dpad: int = 3,
            head_dma_on_vector: bool = True):
    key = (n_rows, n_tiles, use_act, raw, group, version, n_bufs,
           strip_barriers, gp_stride, bulk_r, tail_rs, head_rs, dpad,
           head_dma_on_vector)
    if key not in _NC_CACHE:
        if version == 5:
            _NC_CACHE[key] = build_diag_kernel_v5(
                n_rows, strip_barriers=strip_barriers)
        elif version == 4:
            _NC_CACHE[key] = build_gram_kernel_v4(
                n_rows, bulk_r=bulk_r, n_bufs=n_bufs, group=group,
                gp_stride=gp_stride, head_rs=head_rs, tail_rs=tail_rs,
                dpad=dpad, head_dma_on_vector=head_dma_on_vector,
                strip_barriers=strip_barriers)
        elif version == 3:
            _NC_CACHE[key] = build_gram_kernel_v3(
                n_rows, n_tiles, n_bufs=n_bufs, group=group,
                strip_barriers=strip_barriers)
        elif raw:
            _NC_CACHE[key] = build_gram_kernel_raw(n_rows, n_tiles, group=group)
        else:
            _NC_CACHE[key] = build_gram_kernel(n_rows, n_tiles, use_act)
    return _NC_CACHE[key]


def gram_from_partials(partials: np.ndarray, n_tiles: int | None = None) -> np.ndarray:
    """[..., 128, 6*slots] partials -> full 3x3 Gram matrix (float64)."""
    slots = partials.shape[-1] // 6
    s = partials.astype(np.float64).reshape(-1, 6 * slots).sum(axis=0)
    sq = s[: 3 * slots].reshape(slots, 3).sum(axis=0)
    cr = s[3 * slots :].reshape(slots, 3).sum(axis=0)
    g = np.empty((3, 3), dtype=np.float64)
    g[0, 0], g[1, 1], g[2, 2] = sq
    for k, (i, j) in enumerate(_PAIRS):
        g[i, j] = g[j, i] = cr[k]
    return g


def run_device_partials(predictions: np.ndarray, targets: np.ndarray,
                        n_tiles: int = 4, use_act: bool = True,
                        raw: bool = False, group: int = 4, version: int = 4,
                        n_bufs: int = 12, strip_barriers: bool = True,
                        gp_stride: int = 0, bulk_r: int = 512,
                        tail_rs: tuple = (256, 128, 64, 32, 32),
                        head_rs: tuple = (128, 256, 256, 384), dpad: int = 3,
                        head_dma_on_vector: bool = True,
                        **run_kwargs):
    """Shard over N_CORES, run on device, return per-core partials + results."""
    b = predictions.shape[0]
    assert b % N_CORES == 0
    n_rows = b // N_CORES
    nc = _get_nc(n_rows, n_tiles, use_act, raw, group, version, n_bufs,
                 strip_barriers, gp_stride, bulk_r, tail_rs, head_rs, dpad,
                 head_dma_on_vector)
    preds = np.ascontiguousarray(predictions, dtype=np.float32).reshape(
        N_CORES, n_rows, 3
    )
    targs = np.ascontiguousarray(targets, dtype=np.float32).reshape(
        N_CORES, n_rows, 3
    )
    in_maps = [
        {"pt": np.stack([preds[c], targs[c]])} for c in range(N_CORES)
    ]
    res = run_bass_kernel_spmd(nc, in_maps, list(range(N_CORES)), **run_kwargs)
    partials = np.stack([r["partials"] for r in res.results])
    return partials, res, nc


def _host_loss(predictions, targets, sigma_inv, logdet, lo=0, hi=None):
    """Exact (float64) loss over rows [lo, hi) on the host, chunked."""
    hi = predictions.shape[0] if hi is None else hi
    tot = 0.0
    for s in range(lo, hi, 1 << 20):
        e = min(s + (1 << 20), hi)
        d = predictions[s:e].astype(np.float64) - targets[s:e].astype(np.float64)
        tot += float(np.einsum("bi,ij,bj->", d, sigma_inv, d))
    return abs(logdet + tot / (hi - lo))


def _sigma_inv_is_scalar(sigma_inv: np.ndarray) -> bool:
    """True iff sigma_inv == c*I to fp64 precision (the shipped input)."""
    d = np.diag(sigma_inv)
    off = sigma_inv - np.diag(d)
    tol = 1e-9 * float(np.abs(d).min())
    return (float(np.abs(off).max()) <= tol
            and float(np.abs(d - d[0]).max()) <= 1e-9 * abs(float(d[0])))


def kernel(predictions: np.ndarray, targets: np.ndarray, sigma: np.ndarray) -> np.ndarray:
    predictions = np.asarray(predictions, dtype=np.float32)
    targets = np.asarray(targets, dtype=np.float32)
    sigma64 = np.asarray(sigma, dtype=np.float64)
    sigma_inv = np.linalg.inv(sigma64)
    _, logdet = np.linalg.slogdet(sigma64)

    # Cheap subsample estimate (~0.3% rel) to sanity-gate the device result.
    est = _host_loss(predictions, targets, sigma_inv, logdet,
                     0, min(1 << 16, predictions.shape[0]))

    use_v5 = _sigma_inv_is_scalar(sigma_inv)
    loss = None
    for _attempt in range(2):
        if use_v5:
            partials, _, _ = run_device_partials(predictions, targets, version=5)
            s = float(partials.astype(np.float64).sum())
            mean_mahal = float(sigma_inv[0, 0]) * s / predictions.shape[0]
        else:
            partials, _, nc = run_device_partials(predictions, targets, version=4)
            g = gram_from_partials_v4(partials, nc._v4_plan)
            mean_mahal = float((sigma_inv * g).sum()) / predictions.shape[0]
        loss = abs(logdet + mean_mahal)
        if np.isfinite(loss) and abs(loss - est) <= 0.05 * max(abs(est), 1e-9):
            return np.float32(loss)
    # Device result failed the sanity gate twice: fall back to exact host.
    return np.float32(_host_loss(predictions, targets, sigma_inv, logdet))



# revision 23
# speedup vs baseline: 1.2537x; 1.0132x over previous
"""Trainium2 Bass kernel for CustomLossWithCovariance.

loss = abs(logdet(sigma) + mean_b[(p_b - t_b)^T sigma^{-1} (p_b - t_b)])

Only the 3x3 Gram matrix G = sum_b d_b d_b^T (d = pred - targ) requires
touching the [B, 3] data; the device computes per-core partial pair-sums
of G, and the host finishes with the tiny 3x3 algebra:
    mean_mahalanobis = <sigma_inv, G> / B
    loss = |logdet(sigma) + mean_mahalanobis|

Sharding: data-parallel over the batch across 8 NeuronCores (each core
streams a contiguous [B/8, 3] shard; partial sums gathered on host).

Production path: build_gram_kernel_v4 (raw Bacc, manual semaphores).
Per tile: one dma_start brings pred|targ halves; DVE subtracts into a
row-interleaved bf16 d buffer (unit-stride write — scatter writes are
4.6x slower on DVE); DVE fused multiply-reduces (stride-3 component
reads, grouped across tiles) produce the cross sums and ACT Square
accumulate produces the diagonals. Tile sizes ramp up at the head (so
DVE starts ~5us earlier) and shrink at the tail (so the post-last-DMA
dependency chain is short). The fp32 ring slot is freed by the sub
alone, letting the DMA stream run n_bufs tiles ahead. Bass's two
__init__ all-engine barriers are stripped (saves ~1.5us; the one real
dependency — gpsimd const memsets before ACT bias reads — is re-fenced
with boot_sem). Accumulator tensors are pre-zeroed so any flush race
degrades to ~1e-4 error instead of garbage, and kernel() additionally
sanity-gates the device result against a host subsample estimate with
retry + exact-host fallback.

Older variants (build_gram_kernel, build_gram_kernel_raw,
build_gram_kernel_v3) are kept for reference only.
"""

import numpy as np

import concourse.bass as bass
import concourse.bacc as bacc
import concourse.mybir as mybir
from concourse import tile
from concourse.bass_utils import run_bass_kernel_spmd

N_CORES = 8
B_FULL = 8388608
P = 128

_PAIRS = [(0, 1), (0, 2), (1, 2)]


def build_gram_kernel(n_rows: int, n_tiles: int, use_act: bool = True):
    """Build the per-core Bass module.

    Input: pt [2, n_rows, 3] f32 (pred stacked with targ)
    Output: partials [128, 6 * n_tiles] f32
        col t*3+i            : sum over this tile/partition of d_i^2
        col 3*n_tiles + t*3+k: sum of d_i*d_j for pair k in _PAIRS
    """
    assert n_rows % (P * n_tiles) == 0
    r = n_rows // (P * n_tiles)  # rows per partition per tile
    m = 3 * r                    # flat f32 elements per partition per tile
    f32 = mybir.dt.float32

    # Bacc (not plain Bass): its compile() pass legalizes semaphore waits
    # (each TRN2 instruction holds at most one wait slot).
    nc = bacc.Bacc("TRN2", target_bir_lowering=False, debug=False)
    pt = nc.dram_tensor("pt", [2, n_rows, 3], f32, kind="ExternalInput")
    out = nc.dram_tensor("partials", [P, 6 * n_tiles], f32, kind="ExternalOutput")

    # [t][p][w(2), m] — per tile/partition: pred chunk and targ chunk, each
    # m contiguous f32 in DRAM.
    pt_v = pt[:].rearrange("w (t p r) c -> t p w (r c)", t=n_tiles, p=P)

    with tile.TileContext(nc) as tc:
        with (
            tc.tile_pool(name="io", bufs=3) as io_pool,
            tc.tile_pool(name="dve_scr", bufs=2) as dve_scr,
            tc.tile_pool(name="act_scr", bufs=2) as act_scr,
            tc.tile_pool(name="acc", bufs=1) as acc_pool,
        ):
            acc_sq = acc_pool.tile([P, 3 * n_tiles], f32)
            acc_cr = acc_pool.tile([P, 3 * n_tiles], f32)

            for t in range(n_tiles):
                buf = io_pool.tile([P, 2 * m], f32, tag="buf")
                nc.sync.dma_start(
                    out=buf[:].rearrange("p (w m) -> p w m", w=2),
                    in_=pt_v[t],
                )

                # In-place: d = pred - targ, overwriting the pred half.
                nc.vector.tensor_tensor(
                    out=buf[:, 0:m],
                    in0=buf[:, 0:m],
                    in1=buf[:, m : 2 * m],
                    op=mybir.AluOpType.subtract,
                )
                d3 = buf[:, 0:m].rearrange("p (r c) -> p c r", c=3)

                # Diagonal sums on the scalar engine (Square + accum_out),
                # overlapping with the DVE cross-products.
                if use_act:
                    for i in range(3):
                        sq = act_scr.tile([P, r], f32, tag="sq")
                        nc.scalar.activation(
                            out=sq[:],
                            in_=d3[:, i, :],
                            func=mybir.ActivationFunctionType.Square,
                            accum_out=acc_sq[:, t * 3 + i : t * 3 + i + 1],
                        )
                else:
                    for i in range(3):
                        sq = dve_scr.tile([P, r], f32, tag="pr")
                        nc.vector.scalar_tensor_tensor(
                            out=sq[:],
                            in0=d3[:, i, :],
                            scalar=1.0,
                            in1=d3[:, i, :],
                            op0=mybir.AluOpType.mult,
                            op1=mybir.AluOpType.mult,
                            accum_out=acc_sq[:, t * 3 + i : t * 3 + i + 1],
                        )
                # Cross sums: fused multiply+reduce on DVE
                # (scalar_tensor_tensor: out = (in0 * 1.0) * in1, accum = sum).
                for k, (i, j) in enumerate(_PAIRS):
                    pr = dve_scr.tile([P, r], f32, tag="pr")
                    nc.vector.scalar_tensor_tensor(
                        out=pr[:],
                        in0=d3[:, i, :],
                        scalar=1.0,
                        in1=d3[:, j, :],
                        op0=mybir.AluOpType.mult,
                        op1=mybir.AluOpType.mult,
                        accum_out=acc_cr[:, t * 3 + k : t * 3 + k + 1],
                    )

            nc.sync.dma_start(out=out[:, 0 : 3 * n_tiles], in_=acc_sq[:])
            nc.sync.dma_start(out=out[:, 3 * n_tiles : 6 * n_tiles], in_=acc_cr[:])

    nc.compile()
    return nc


def build_gram_kernel_raw(n_rows: int, n_tiles: int = 32, n_bufs: int = 24,
                          group: int = 4, skip_exit_barrier: bool = True):
    """Raw-Bacc variant: manual semaphores, no TileContext.

    Skips Tile's prologue/epilogue (drain + two all-engine EVSEM
    barriers, ~16 us) — the only sync needed is a three-semaphore chain:
    DMA loads (one HWDGE ring) -> DVE -> ACT.

    The ring of tile buffers lives in ONE SBUF tensor so the fused
    multiply-reduces can span `group` consecutive tiles with a single
    instruction (free-dim AP [group, r]) — amortizing the per-op fixed
    cost and the accumulator-drain, which keeps both compute engines
    well under the DMA pace.

    Input: pt [2, n_rows, 3] f32. Output: partials [128, 6 * n_groups]
    (same slot layout as build_gram_kernel, with n_groups slots).
    """
    assert n_tiles % group == 0 and n_bufs % group == 0
    assert n_rows % (P * n_tiles) == 0
    n_groups = n_tiles // group
    r = n_rows // (P * n_tiles)
    m = 3 * r
    f32 = mybir.dt.float32

    nc = bacc.Bacc("TRN2", target_bir_lowering=False, debug=False)
    pt = nc.dram_tensor("pt", [2, n_rows, 3], f32, kind="ExternalInput")
    out = nc.dram_tensor("partials", [P, 6 * n_groups], f32, kind="ExternalOutput")
    pt_v = pt[:].rearrange("w (t p r) c -> t p w (r c)", t=n_tiles, p=P)

    ring = nc.alloc_sbuf_tensor("ring", [P, n_bufs * 2 * m], f32).ap()

    def buf(t):
        s = t % n_bufs
        return ring[:, s * 2 * m : (s + 1) * 2 * m]

    def dgroup(g, i):
        # component i of the diff halves of tiles 4g..4g+3: [128, group, r]
        s0 = (g * group) % n_bufs
        w = ring[:, s0 * 2 * m : (s0 + group) * 2 * m]
        return w.rearrange("p (t w r c) -> p t w c r", t=group, w=2, c=3)[:, :, 0, i, :]

    acc_sq = nc.alloc_sbuf_tensor("acc_sq", [P, 3 * n_groups], f32).ap()
    acc_cr = nc.alloc_sbuf_tensor("acc_cr", [P, 3 * n_groups], f32).ap()
    # Rotated scratch (dead stores of the fused ops), 2 groups deep so each
    # group's single stale semaphore wait also covers the scratch WAW from
    # two groups back.
    pr_scrs = [
        nc.alloc_sbuf_tensor(f"pr_scr{k}", [P, group * r], f32).ap() for k in range(6)
    ]
    sq_scrs = [
        nc.alloc_sbuf_tensor(f"sq_scr{k}", [P, group * r], f32).ap() for k in range(6)
    ]

    # One DMA-completion semaphore per ring buffer: a single shared sem
    # would be unsound — each dma_start is split across 16 SDMA engines
    # whose sub-completions interleave across in-flight DMAs.
    dma_sems = [nc.alloc_semaphore(f"dma_sem{i}") for i in range(n_bufs)]
    out_sem = nc.alloc_semaphore("out_sem")
    dve_sem = nc.alloc_semaphore("dve_sem")
    act_sem = nc.alloc_semaphore("act_sem")

    # DVE emission order: subs run ahead; the grouped multiply-reduces for
    # group g are emitted after sub(4g+4) so their drain-wait on the last
    # sub of the group is already satisfied when it executes (DVE writes
    # drain asynchronously). Only the last group trails the final sub.
    dve_order = []
    for t in range(n_tiles):
        dve_order.append(("sub", t))
        if t % group == 0 and t >= group:
            # one sub of stagger after the group's last sub
            dve_order.append(("stt", t // group - 1))
    dve_order.append(("stt", n_groups - 1))
    sub_done, sttg_done = {}, {}
    v = 0
    for kind, x in dve_order:
        if kind == "sub":
            v += 1
            sub_done[x] = v
        else:
            v += 3
            sttg_done[x] = v

    # Output chunks: flush finished accumulator columns while later tiles
    # still stream, so the tail only waits on the last small chunk.
    chunk = max(1, n_groups // 2)
    chunks = [(c, min(c + chunk, n_groups)) for c in range(0, n_groups, chunk)]

    import contextlib

    @contextlib.contextmanager
    def _block():
        # no_gpsimd_drain=True emits per-engine drains explicitly and then a
        # sem-only all-engine butterfly. The butterfly only delays NEFF end
        # (outputs are already fenced by the sequencer's out_sem wait), so
        # optionally no-op it during Block.__exit__.
        with nc.Block(no_gpsimd_drain=True) as blk:
            try:
                yield blk
            finally:
                if skip_exit_barrier:
                    nc.all_engine_barrier = lambda **kw: None
        if skip_exit_barrier:
            del nc.all_engine_barrier  # restore class method

    with _block() as block:

        @block.sync
        def _(sync):
            for t in range(n_tiles):
                if head_dma_on_vector and t < h0:
                    continue  # issued from the vector queue (clears the
                              # NEFF entry barrier ~1.4us before sync)
                if t >= n_bufs:
                    # ring reuse: all consumers of the buffer's previous
                    # occupant (tile t - n_bufs) must be done
                    prev = t - n_bufs
                    sync.wait_ge(dve_sem, sttg_done[prev // group])
                    sync.wait_ge(act_sem, 3 * (prev // group + 1))
                sync.dma_start(
                    out=buf(t).rearrange("p (w m) -> p w m", w=2),
                    in_=pt_v[t],
                ).then_inc(dma_sems[t % n_bufs], 16)
            n_out = 0
            for lo, hi in chunks:
                sync.wait_ge(act_sem, 3 * hi)
                sync.dma_start(
                    out=out[:, 3 * lo : 3 * hi], in_=acc_sq[:, 3 * lo : 3 * hi]
                ).then_inc(out_sem, 16)
                sync.wait_ge(dve_sem, sttg_done[hi - 1])
                sync.dma_start(
                    out=out[:, 3 * (n_groups + lo) : 3 * (n_groups + hi)],
                    in_=acc_cr[:, 3 * lo : 3 * hi],
                ).then_inc(out_sem, 16)
                n_out += 32
            sync.wait_ge(out_sem, n_out)

        @block.vector
        def _(vector):
            for kind, x in dve_order:
                if kind == "sub":
                    b = buf(x)
                    vector.wait_ge(dma_sems[x % n_bufs], 16 * (x // n_bufs + 1))
                    vector.tensor_tensor(
                        out=b[:, 0:m],
                        in0=b[:, 0:m],
                        in1=b[:, m : 2 * m],
                        op=mybir.AluOpType.subtract,
                    ).then_inc(dve_sem, 1)
                else:
                    vector.wait_ge(dve_sem, sub_done[(x + 1) * group - 1])
                    for k, (i, j) in enumerate(_PAIRS):
                        vector.scalar_tensor_tensor(
                            out=pr_scrs[(x % 2) * 3 + k][:].rearrange(
                                "p (t r) -> p t r", t=group
                            ),
                            in0=dgroup(x, i),
                            scalar=1.0,
                            in1=dgroup(x, j),
                            op0=mybir.AluOpType.mult,
                            op1=mybir.AluOpType.mult,
                            accum_out=acc_cr[:, x * 3 + k : x * 3 + k + 1],
                        ).then_inc(dve_sem, 1)

        @block.scalar
        def _(scalar):
            for g in range(n_groups):
                scalar.wait_ge(dve_sem, sub_done[(g + 1) * group - 1])
                if g >= 2:
                    # scratch slot reuse from two groups back
                    scalar.wait_ge(act_sem, 3 * (g - 1))
                for i in range(3):
                    scalar.activation(
                        out=sq_scrs[(g % 2) * 3 + i][:].rearrange(
                            "p (t r) -> p t r", t=group
                        ),
                        in_=dgroup(g, i),
                        func=mybir.ActivationFunctionType.Square,
                        accum_out=acc_sq[:, g * 3 + i : g * 3 + i + 1],
                    ).then_inc(act_sem, 1)

    nc.compile()
    return nc

def _strip_entry_barriers(nc):
    """Remove the two all-engine entry barriers Bass.__init__ emits.

    They serialize ~4us of semaphore round-trips before the first DMA can
    issue. The only cross-engine ordering they provide that this kernel
    needs is gpsimd-const-AP-memset -> ACT-bias-read, which is re-fenced
    explicitly with boot_sem in build_gram_kernel_v3.
    """
    bar = set(nc.barrier_sems)
    blk = nc.main_func.blocks[0]
    drop = []
    for ins in blk.instructions:
        si = getattr(ins, "sync_info", None)
        if si is None:
            continue
        sems = {w.id for w in si.on_wait or []}
        sems |= {u.id for u in si.on_update or []}
        if sems & bar:
            drop.append(ins)
    for ins in drop:
        blk.instructions.remove(ins)
    return len(drop)


def build_gram_kernel_v3(n_rows: int, n_tiles: int = 16, n_bufs: int = 8,
                         group: int = 4, strip_barriers: bool = True,
                         skip_exit_barrier: bool = True):
    """v3: planar-bf16 d + 2x DVE reduces + ACT squares.

    Per tile: DMA both halves -> DVE sub (fp32 in, planar bf16 out:
    component planes x|y|z so reduce operands are unit-stride 2-byte,
    unlocking the DVE 2x perf mode) -> DVE cross-product reduces (grouped
    `group` tiles per instr) + ACT Square reduces (grouped; per-tile for
    the last group so the post-DMA tail stays short).

    The fp32 ring slot is freed by the sub alone (d lives in its own
    full-size buffer), so the DMA stream runs ~n_bufs tiles ahead of
    compute and never stalls on the reduce bursts.

    Output layout [128, 3*n_groups + 3*(n_groups-1) + 3*group]:
      cols 0 .. 3*n_groups-1: cross sums (group g, pair k at 3g+k)
      then squares: full groups 0..n_groups-2 (3 each), then the last
      group's tiles individually (3 each).
    """
    assert n_rows % (P * n_tiles) == 0 and n_tiles % group == 0
    r = n_rows // (P * n_tiles)
    m = 3 * r
    n_groups = n_tiles // group
    full_sq = n_groups - 1           # square-groups emitted grouped
    tail0 = full_sq * group          # first per-tile-squares tile
    ncr = 3 * n_groups
    nsq = 3 * full_sq + 3 * group
    f32, bf16 = mybir.dt.float32, mybir.dt.bfloat16

    nc = bacc.Bacc("TRN2", target_bir_lowering=False, debug=False)
    if strip_barriers:
        _strip_entry_barriers(nc)
    pt = nc.dram_tensor("pt", [2, n_rows, 3], f32, kind="ExternalInput")
    out = nc.dram_tensor("partials", [P, ncr + nsq], f32, kind="ExternalOutput")
    pt_v = pt[:].rearrange("w (t p r) c -> t p w (r c)", t=n_tiles, p=P)

    ring = nc.alloc_sbuf_tensor("ring", [P, n_bufs * 2 * m], f32).ap()
    d_all = nc.alloc_sbuf_tensor("d_all", [P, n_tiles * m], bf16).ap()
    d_t = d_all.rearrange("p (t c r) -> p t c r", t=n_tiles, c=3)
    acc_cr = nc.alloc_sbuf_tensor("acc_cr", [P, ncr], f32).ap()
    acc_sq = nc.alloc_sbuf_tensor("acc_sq", [P, nsq], f32).ap()
    # Dead stores of the fused reduces; single slot per engine (each
    # engine executes its own stream in order, so WAW is safe).
    cr_scr = nc.alloc_sbuf_tensor("cr_scr", [P, group * r], bf16).ap()
    sq_scr = nc.alloc_sbuf_tensor("sq_scr", [P, group * r], bf16).ap()

    dma_sems = [nc.alloc_semaphore(f"dma{i}") for i in range(n_bufs)]
    sub_sem = nc.alloc_semaphore("sub_sem")
    red_sem = nc.alloc_semaphore("red_sem")
    act_sem = nc.alloc_semaphore("act_sem")
    out_sem = nc.alloc_semaphore("out_sem")
    boot_sem = nc.alloc_semaphore("boot_sem")

    def dcomp(t0, nt, i):
        # component i of tiles t0..t0+nt-1: [128, nt, r] unit-stride bf16
        v = d_t[:, t0 : t0 + nt, i, :]
        return v

    import contextlib

    @contextlib.contextmanager
    def _block():
        with nc.Block(no_gpsimd_drain=True) as blk:
            try:
                yield blk
            finally:
                if skip_exit_barrier:
                    nc.all_engine_barrier = lambda **kw: None
        if skip_exit_barrier:
            del nc.all_engine_barrier  # restore class method

    with _block() as block:

        @block.gpsimd
        def _(gpsimd):
            # Const-AP memsets (ACT bias) are earlier in gpsimd's stream;
            # this inc publishes their completion to the scalar queue.
            gpsimd.sem_inc(boot_sem, 1)

        @block.sync
        def _(sync):
            for t in range(n_tiles):
                if head_dma_on_vector and t < h0:
                    continue  # issued from the vector queue (clears the
                              # NEFF entry barrier ~1.4us before sync)
                if t >= n_bufs:
                    # ring slot free once its previous occupant was subbed
                    sync.wait_ge(sub_sem, t - n_bufs + 1)
                sync.dma_start(
                    out=ring[:, (t % n_bufs) * 2 * m : (t % n_bufs + 1) * 2 * m]
                    .rearrange("p (w m) -> p w m", w=2),
                    in_=pt_v[t],
                ).then_inc(dma_sems[t % n_bufs], 16)
            # accumulator flush: big chunks early, last-group slivers at end
            sync.wait_ge(red_sem, 3 * (n_groups - 1))
            sync.dma_start(
                out=out[:, 0 : 3 * (n_groups - 1)],
                in_=acc_cr[:, 0 : 3 * (n_groups - 1)],
            ).then_inc(out_sem, 16)
            sync.wait_ge(act_sem, 3 * full_sq)
            sync.dma_start(
                out=out[:, ncr : ncr + 3 * full_sq],
                in_=acc_sq[:, 0 : 3 * full_sq],
            ).then_inc(out_sem, 16)
            sync.wait_ge(red_sem, 3 * n_groups)
            sync.dma_start(
                out=out[:, 3 * (n_groups - 1) : ncr],
                in_=acc_cr[:, 3 * (n_groups - 1) : ncr],
            ).then_inc(out_sem, 16)
            sync.wait_ge(act_sem, nsq)
            sync.dma_start(
                out=out[:, ncr + 3 * full_sq : ncr + nsq],
                in_=acc_sq[:, 3 * full_sq : nsq],
            ).then_inc(out_sem, 16)
            sync.wait_ge(out_sem, 64)

        @block.vector
        def _(vector):
            for t in range(n_tiles):
                s = t % n_bufs
                buf = ring[:, s * 2 * m : (s + 1) * 2 * m]
                vector.wait_ge(dma_sems[s], 16 * (t // n_bufs + 1))
                # d = pred - targ, downcast to bf16, scattered into
                # component planes (write AP [r, 3] w/ strides [1, r])
                vector.tensor_tensor(
                    out=d_all[:, t * m : (t + 1) * m].rearrange(
                        "p (c r) -> p r c", c=3
                    ),
                    in0=buf[:, 0:m],
                    in1=buf[:, m : 2 * m],
                    op=mybir.AluOpType.subtract,
                ).then_inc(sub_sem, 1)
                if t % group == group - 1:
                    g = t // group
                    for k, (i, j) in enumerate(_PAIRS):
                        vector.scalar_tensor_tensor(
                            out=cr_scr[:].rearrange("p (t r) -> p t r", t=group),
                            in0=dcomp(g * group, group, i),
                            scalar=1.0,
                            in1=dcomp(g * group, group, j),
                            op0=mybir.AluOpType.mult,
                            op1=mybir.AluOpType.mult,
                            accum_out=acc_cr[:, g * 3 + k : g * 3 + k + 1],
                        ).then_inc(red_sem, 1)

        @block.scalar
        def _(scalar):
            scalar.wait_ge(boot_sem, 1)
            for g in range(full_sq):
                scalar.wait_ge(sub_sem, group * (g + 1))
                for i in range(3):
                    scalar.activation(
                        out=sq_scr[:].rearrange("p (t r) -> p t r", t=group),
                        in_=dcomp(g * group, group, i),
                        func=mybir.ActivationFunctionType.Square,
                        accum_out=acc_sq[:, g * 3 + i : g * 3 + i + 1],
                    ).then_inc(act_sem, 1)
            for w, t in enumerate(range(tail0, n_tiles)):
                scalar.wait_ge(sub_sem, t + 1)
                for i in range(3):
                    c = 3 * full_sq + w * 3 + i
                    scalar.activation(
                        out=sq_scr[:, 0:r],
                        in_=dcomp(t, 1, i),
                        func=mybir.ActivationFunctionType.Square,
                        accum_out=acc_sq[:, c : c + 1],
                    ).then_inc(act_sem, 1)

    nc.compile()
    nc._v3_meta = (n_tiles, group)
    return nc


def build_gram_kernel_v4(n_rows: int, bulk_r: int = 512, n_bufs: int = 12,
                         group: int = 4, gp_stride: int = 0,
                         head_rs: tuple = (128, 256, 256, 384),
                         tail_rs: tuple = (256, 128, 64, 32, 32),
                         dpad: int = 3, head_dma_on_vector: bool = True,
                         strip_barriers: bool = True,
                         skip_exit_barrier: bool = True):
    """v4: interleaved-bf16 d, measured-cost engine mix, shrinking tail.

    Measured HW rates (ns per 128-wide column): DVE sub fp32->bf16 unit
    1.28; DVE stt reduce bf16 stride-3 1.32; ACT Square ~1.0-1.4 + 740
    fixed; GPSIMD sub ~3.5. Writes must be unit-stride (scatter = 4.6x);
    strided reads are cheap. So d stays row-interleaved bf16.

    - bulk tiles of r=bulk_r rows/partition; every gp_stride-th bulk tile's
      sub runs on GPSIMD to keep DVE under the DMA pace.
    - cross-products: DVE stt grouped over `group` consecutive bulk tiles.
    - squares: ACT, same grouping; tail tiles per-tile; r<=32 tails on DVE.
    - tail tiles shrink so the post-last-DMA dependency chain is tiny.
    """
    R = n_rows // P
    assert n_rows % P == 0
    bulk_n = (R - sum(head_rs) - sum(tail_rs)) // bulk_r
    assert sum(head_rs) + bulk_n * bulk_r + sum(tail_rs) == R
    rs = list(head_rs) + [bulk_r] * bulk_n + list(tail_rs)
    n_tiles = len(rs)
    h0 = len(head_rs)               # first bulk tile index
    t0_tail = h0 + bulk_n           # first tail tile index
    cum = [0]
    for r in rs:
        cum.append(cum[-1] + r)
    # bulk groups: chunks of `group` (absolute tile indices)
    groups = [list(range(s, min(s + group, t0_tail)))
              for s in range(h0, t0_tail, group)]
    group_last = {g[-1]: g for g in groups}
    # per-tile (ungrouped) reduce tiles: head + tail
    per_tile = set(range(0, h0)) | set(range(t0_tail, n_tiles))
    # every gp_stride-th bulk tile's sub runs on GPSIMD (0 = none)
    gp_tiles = (set(range(h0, t0_tail, gp_stride)) if gp_stride else set())
    f32, bf16 = mybir.dt.float32, mybir.dt.bfloat16

    nc = bacc.Bacc("TRN2", target_bir_lowering=False, debug=False)
    if strip_barriers:
        _strip_entry_barriers(nc)
    pt = nc.dram_tensor("pt", [2, n_rows, 3], f32, kind="ExternalInput")

    # per-tile engine assignment of the sub + cumulative sem targets
    dve_idx, gp_idx = {}, {}
    for t in range(n_tiles):
        if t in gp_tiles:
            gp_idx[t] = len(gp_idx)
        else:
            dve_idx[t] = len(dve_idx)

    def sub_waits(last_t):
        """(sub_sem target, gsub_sem target) covering tiles 0..last_t."""
        d = sum(1 for t, i in dve_idx.items() if t <= last_t)
        g = sum(1 for t, i in gp_idx.items() if t <= last_t)
        return d, g

    # reduce slot counts (order finalized at emission)
    n_dve = 3 * len(groups) + 3 * len(per_tile) + 3 * sum(
        1 for t in per_tile if rs[t] <= 32)
    plan_act = []
    for t in sorted(per_tile):
        if t < h0 and rs[t] > 32:
            for i in range(3):
                plan_act.append((i, i))
    for g_tiles in groups:
        for i in range(3):
            plan_act.append((i, i))
    for t in sorted(per_tile):
        if t >= t0_tail and rs[t] > 32:
            for i in range(3):
                plan_act.append((i, i))
    n_act = len(plan_act)
    out = nc.dram_tensor("partials", [P, n_dve + n_act], f32,
                         kind="ExternalOutput")

    m_bulk = 3 * bulk_r
    ring = nc.alloc_sbuf_tensor("ring", [P, n_bufs * 2 * m_bulk], f32).ap()
    d_all = nc.alloc_sbuf_tensor("d_all", [P, dpad * R], bf16).ap()
    acc_dve = nc.alloc_sbuf_tensor("acc_dve", [P, n_dve], f32).ap()
    acc_act = nc.alloc_sbuf_tensor("acc_act", [P, n_act], f32).ap()
    cr_scr = nc.alloc_sbuf_tensor("cr_scr", [P, group * bulk_r], bf16).ap()
    sq_scr = nc.alloc_sbuf_tensor("sq_scr", [P, group * bulk_r], bf16).ap()

    dma_sems = [nc.alloc_semaphore(f"dma{i}") for i in range(n_bufs)]
    sub_sem = nc.alloc_semaphore("sub_sem")
    gsub_sem = nc.alloc_semaphore("gsub_sem")
    red_sem = nc.alloc_semaphore("red_sem")
    act_sem = nc.alloc_semaphore("act_sem")
    out_sem = nc.alloc_semaphore("out_sem")
    boot_sem = nc.alloc_semaphore("boot_sem")

    def ring_slot(t):
        s = t % n_bufs
        return ring[:, s * 2 * m_bulk : s * 2 * m_bulk + 2 * 3 * rs[t]]

    def pt_tile(t):
        sl = pt[:, cum[t] * P : cum[t + 1] * P, :]
        return sl.rearrange("w (p r) c -> p w (r c)", p=P)

    def dseg(t0, nt, i):
        # component i of tiles t0..t0+nt-1 (equal r), stride-dpad reads
        v = d_all[:, dpad * cum[t0] : dpad * cum[t0 + nt]]
        return v.rearrange("p (t r c) -> p t c r", t=nt, c=dpad)[:, :, i, :]

    def dsub_out(t):
        # write view for the sub: rows of 3 packed comps, dpad-elem row pitch
        v = d_all[:, dpad * cum[t] : dpad * cum[t + 1]]
        if dpad == 3:
            return v
        return v.rearrange("p (r c) -> p r c", c=dpad)[:, :, 0:3]

    # early-flush boundaries: head reduces + all-but-last bulk group are
    # guaranteed emitted before any tail reduce; ACT head+bulk likewise
    red_early = 3 * h0 + 3 * (len(groups) - 1)
    act_bulk = 3 * h0 + 3 * len(groups)
    _red_pen = max(red_early, n_dve - 6)
    _act_pen = max(act_bulk, n_act - 3)
    n_flush_dve = sum(1 for lo, hi in ((0, red_early), (red_early, _red_pen),
                                       (_red_pen, n_dve)) if hi > lo)
    act_chunks = [(act_bulk, 0, act_bulk), (_act_pen, act_bulk, _act_pen),
                  (n_act, _act_pen, n_act)]
    act_chunks = [c for c in act_chunks if c[2] > c[1]]
    n_flush_act = len(act_chunks)

    import contextlib

    @contextlib.contextmanager
    def _block():
        with nc.Block(no_gpsimd_drain=True) as blk:
            try:
                yield blk
            finally:
                if skip_exit_barrier:
                    nc.all_engine_barrier = lambda **kw: None
        if skip_exit_barrier:
            del nc.all_engine_barrier

    with _block() as block:

        @block.sync
        def _(sync):
            for t in range(n_tiles):
                if head_dma_on_vector and t < h0:
                    continue  # issued from the vector queue (clears the
                              # NEFF entry barrier ~1.4us before sync)
                if t >= n_bufs:
                    u = t - n_bufs
                    if u in gp_idx:
                        sync.wait_ge(gsub_sem, gp_idx[u] + 1)
                    else:
                        sync.wait_ge(sub_sem, dve_idx[u] + 1)
                sync.dma_start(
                    out=ring_slot(t).rearrange("p (w m) -> p w m", w=2),
                    in_=pt_tile(t),
                ).then_inc(dma_sems[t % n_bufs], 16)
            # acc_dve flushes stay here; acc_act flushes issue from the
            # scalar queue (ACT finishes ~2.5us before DVE, so they then
            # complete during the DVE tail instead of serializing after it)
            red_pen = max(red_early, n_dve - 6)   # all but last tail tile
            for val, lo, hi in ((red_early, 0, red_early),
                                (red_pen, red_early, red_pen),
                                (n_dve, red_pen, n_dve)):
                if hi <= lo:
                    continue
                sync.wait_ge(red_sem, val)
                sync.dma_start(out=out[:, lo:hi],
                               in_=acc_dve[:, lo:hi]).then_inc(out_sem, 16)
            sync.wait_ge(out_sem, 16 * (n_flush_dve + n_flush_act))

        @block.vector
        def _(vector):
            red_c = 0
            plan_dyn = []
            pending = []  # deferred bulk-group reduces, drained 1-per-sub

            def emit_one(t0, nt, i, j):
                nonlocal red_c
                r = rs[t0]
                vector.scalar_tensor_tensor(
                    out=cr_scr[:, 0 : nt * r].rearrange(
                        "p (t r) -> p t r", t=nt),
                    in0=dseg(t0, nt, i),
                    scalar=1.0,
                    in1=dseg(t0, nt, j),
                    op0=mybir.AluOpType.mult,
                    op1=mybir.AluOpType.mult,
                    accum_out=acc_dve[:, red_c : red_c + 1],
                ).then_inc(red_sem, 1)
                plan_dyn.append((i, j))
                red_c += 1

            for t in range(n_tiles):
                if t not in gp_tiles:
                    buf = ring_slot(t)
                    mt = 3 * rs[t]
                    vector.wait_ge(dma_sems[t % n_bufs], 16 * (t // n_bufs + 1))
                    vector.tensor_tensor(
                        out=dsub_out(t),
                        in0=buf[:, 0:mt],
                        in1=buf[:, mt : 2 * mt],
                        op=mybir.AluOpType.subtract,
                    ).then_inc(sub_sem, 1)
                    # spread deferred group reduces between subs so the DMA
                    # ring (freed by subs) never stalls on a reduce burst
                    if pending:
                        emit_one(*pending.pop(0))
                if t in group_last:
                    g_tiles = group_last[t]
                    _, gw = sub_waits(g_tiles[-1])
                    if gw:
                        vector.wait_ge(gsub_sem, gw)
                    for (i, j) in _PAIRS:
                        pending.append((g_tiles[0], len(g_tiles), i, j))
                elif t in per_tile:
                    for (i, j) in _PAIRS:
                        emit_one(t, 1, i, j)
                    if rs[t] <= 32:
                        for i in range(3):
                            emit_one(t, 1, i, i)
            while pending:
                emit_one(*pending.pop(0))
            nc._v4_plan_dve = plan_dyn

        @block.scalar
        def _(scalar):
            if head_dma_on_vector:
                # Scalar clears the NEFF entry-barrier chain ~2us before
                # Sync; issuing the head-tile loads here starts the stream
                # (and so the first sub) earlier. Ring slots are fresh, no
                # waits needed; the const-AP fence only matters for the
                # activations below.
                for t in range(h0):
                    scalar.dma_start(
                        out=ring_slot(t).rearrange("p (w m) -> p w m", w=2),
                        in_=pt_tile(t),
                    ).then_inc(dma_sems[t % n_bufs], 16)
            scalar.wait_ge(boot_sem, 1)
            act_c = 0

            def emit_squares(t0, nt):
                nonlocal act_c
                r = rs[t0]
                for i in range(3):
                    scalar.activation(
                        out=sq_scr[:, 0 : nt * r].rearrange(
                            "p (t r) -> p t r", t=nt),
                        in_=dseg(t0, nt, i),
                        func=mybir.ActivationFunctionType.Square,
                        accum_out=acc_act[:, act_c : act_c + 1],
                    ).then_inc(act_sem, 1)
                    act_c += 1

            def sq_waits(last_t):
                d, g = sub_waits(last_t)
                scalar.wait_ge(sub_sem, d)
                if g:
                    scalar.wait_ge(gsub_sem, g)

            for t in sorted(per_tile):
                if t < h0 and rs[t] > 32:
                    sq_waits(t)
                    emit_squares(t, 1)
            for g_tiles in groups:
                sq_waits(g_tiles[-1])
                emit_squares(g_tiles[0], len(g_tiles))
            for t in sorted(per_tile):
                if t >= t0_tail and rs[t] > 32:
                    sq_waits(t)
                    emit_squares(t, 1)
            for val, lo, hi in act_chunks:
                scalar.wait_ge(act_sem, val)
                scalar.dma_start(out=out[:, n_dve + lo : n_dve + hi],
                                 in_=acc_act[:, lo:hi]).then_inc(out_sem, 16)

        @block.gpsimd
        def _(gpsimd):
            # Zero the accumulator tensors so a (never-observed, but cheap
            # to insure against) flush-before-drain race reads zeros - a
            # ~1e-4 relative error - instead of stale SBUF garbage.
            gpsimd.memset(acc_dve, 0.0)
            gpsimd.memset(acc_act, 0.0)
            gpsimd.sem_inc(boot_sem, 1)
            for t in sorted(gp_tiles):
                buf = ring_slot(t)
                mt = 3 * rs[t]
                gpsimd.wait_ge(dma_sems[t % n_bufs], 16 * (t // n_bufs + 1))
                gpsimd.tensor_tensor(
                    out=dsub_out(t),
                    in0=buf[:, 0:mt],
                    in1=buf[:, mt : 2 * mt],
                    op=mybir.AluOpType.subtract,
                ).then_inc(gsub_sem, 1)

    nc.compile()
    assert len(nc._v4_plan_dve) == n_dve
    nc._v4_plan = (nc._v4_plan_dve, plan_act)
    return nc


def build_diag_kernel_v5(n_rows: int, bulk_r: int = 512, n_bufs: int = 14,
                         n_dbufs: int = 8,
                         head_rs: tuple = (128, 128, 256),
                         tail_rs: tuple = (384, 256, 192, 128, 64),
                         n_sq_dve: int = 2,
                         head_on_side: bool = True,
                         n_side: int = 2,
                         gp_head: bool = False,
                         strip_barriers: bool = True,
                         skip_exit_barrier: bool = True):
    """v5: diagonal-sigma fast path — per-tile sum of squared differences.

    For sigma = c*I (the shipped input), the loss needs only
    S = sum_b ||p_b - t_b||^2; no cross products. Per tile: DMA both
    halves -> DVE sub (fp32 in, bf16 interleaved out, unit-stride write)
    -> Square with accum_out (one fp32 partial column per tile), on ACT
    for most tiles. Both engines run at ~50% of the DMA pace, so the
    kernel is purely DMA-bound: the 16-engine pool sustains ~415 GB/s.

    Head: small ramp tiles issued from the gpsimd/scalar queues (they
    clear the NEFF boot chain before sync) — small, so the brief
    3-queue pool contention costs little. Tail: ramp-down sizes, with
    the squares of the last 2*n_sq_dve tiles alternating DVE/ACT so the
    post-last-load catch-up runs on both engines in parallel.

    Output: partials [128, n_tiles] (ACT tiles then DVE tiles, by the
    _v5_order attr); host sums everything (f64).
    """
    R = n_rows // P
    assert n_rows % P == 0
    bulk_n = (R - sum(head_rs) - sum(tail_rs)) // bulk_r
    assert sum(head_rs) + bulk_n * bulk_r + sum(tail_rs) == R
    rs = list(head_rs) + [bulk_r] * bulk_n + list(tail_rs)
    n_tiles = len(rs)
    cum = [0]
    for r in rs:
        cum.append(cum[-1] + r)
    m_bulk = 3 * max(rs)
    f32, bf16 = mybir.dt.float32, mybir.dt.bfloat16

    # squares of the last 2*n_sq_dve tiles alternate DVE/ACT (DVE takes
    # the even offsets from the end: ..., t-4, t-2, last)
    sq_dve = {n_tiles - 1 - 2 * k for k in range(n_sq_dve)}
    act_tiles = [t for t in range(n_tiles) if t not in sq_dve]
    dve_tiles = sorted(sq_dve)
    n_act, n_dve = len(act_tiles), len(dve_tiles)
    acol = {t: i for i, t in enumerate(act_tiles)}
    vcol = {t: i for i, t in enumerate(dve_tiles)}
    # act_sem value after the square of tile u (ACT tiles only)
    act_done = {t: i + 1 for i, t in enumerate(act_tiles)}
    # sub_sem value after DVE finished tile t (sub, plus square if DVE tile)
    sub_done = {}
    _v = 0
    for _t in range(n_tiles):
        _v += 2 if _t in sq_dve else 1
        sub_done[_t] = _v
    n_flush = (1 if n_act > 1 else 0) + 1 + (1 if n_dve else 0)

    nc = bacc.Bacc("TRN2", target_bir_lowering=False, debug=False)
    if strip_barriers:
        _strip_entry_barriers(nc)
    pt = nc.dram_tensor("pt", [2, n_rows, 3], f32, kind="ExternalInput")
    out = nc.dram_tensor("partials", [P, n_tiles], f32, kind="ExternalOutput")

    ring = nc.alloc_sbuf_tensor("ring", [P, n_bufs * 2 * m_bulk], f32).ap()
    dbuf = nc.alloc_sbuf_tensor("dbuf", [P, n_dbufs * m_bulk], bf16).ap()
    acc_a = nc.alloc_sbuf_tensor("acc_a", [P, max(n_act, 1)], f32).ap()
    acc_v = nc.alloc_sbuf_tensor("acc_v", [P, max(n_dve, 1)], f32).ap()
    sq_scr = nc.alloc_sbuf_tensor("sq_scr", [P, m_bulk], bf16).ap()
    vq_scr = nc.alloc_sbuf_tensor("vq_scr", [P, m_bulk], bf16).ap()

    dma_sems = [nc.alloc_semaphore(f"dma{i}") for i in range(n_bufs)]
    sub_sem = nc.alloc_semaphore("sub_sem")
    act_sem = nc.alloc_semaphore("act_sem")
    out_sem = nc.alloc_semaphore("out_sem")
    boot_sem = nc.alloc_semaphore("boot_sem")

    def ring_slot(t):
        s = t % n_bufs
        return ring[:, s * 2 * m_bulk : s * 2 * m_bulk + 2 * 3 * rs[t]]

    def d_slot(t):
        s = t % n_dbufs
        return dbuf[:, s * m_bulk : s * m_bulk + 3 * rs[t]]

    def pt_tile(t):
        sl = pt[:, cum[t] * P : cum[t + 1] * P, :]
        return sl.rearrange("w (p r) c -> p w (r c)", p=P)

    n_head = len(head_rs)

    def issue_load(q, t):
        q.dma_start(
            out=ring_slot(t).rearrange("p (w m) -> p w m", w=2),
            in_=pt_tile(t),
        ).then_inc(dma_sems[t % n_bufs], 16)

    import contextlib

    @contextlib.contextmanager
    def _block():
        with nc.Block(no_gpsimd_drain=True) as blk:
            try:
                yield blk
            finally:
                if skip_exit_barrier:
                    nc.all_engine_barrier = lambda **kw: None
        if skip_exit_barrier:
            del nc.all_engine_barrier

    with _block() as block:

        @block.sync
        def _(sync):
            for t in range(n_side if head_on_side else 0, n_tiles):
                if t >= n_bufs:
                    # ring slot free once its previous occupant was subbed
                    u = t - n_bufs
                    sync.wait_ge(sub_sem, sub_done[u] - (1 if u in sq_dve else 0))
                issue_load(sync, t)
            if n_dve:
                # DVE squares all drained once the last DVE tile's pair ran;
                # this flush overlaps scalar's final acc_a sliver flush.
                sync.wait_ge(sub_sem, sub_done[dve_tiles[-1]])
                sync.dma_start(
                    out=out[:, n_act : n_act + n_dve], in_=acc_v[:, 0:n_dve]
                ).then_inc(out_sem, 16)
            sync.wait_ge(out_sem, 16 * n_flush)

        @block.vector
        def _(vector):
            for t in range(n_tiles):
                vector.wait_ge(dma_sems[t % n_bufs], 16 * (t // n_bufs + 1))
                u = t - n_dbufs
                if u >= 0 and u not in sq_dve:
                    # d slot free once its previous occupant was squared
                    vector.wait_ge(act_sem, act_done[u])
                buf = ring_slot(t)
                mt = 3 * rs[t]
                vector.tensor_tensor(
                    out=d_slot(t),
                    in0=buf[:, 0:mt],
                    in1=buf[:, mt : 2 * mt],
                    op=mybir.AluOpType.subtract,
                ).then_inc(sub_sem, 1)
                if t in sq_dve:
                    c = vcol[t]
                    vector.scalar_tensor_tensor(
                        out=vq_scr[:, 0 : 3 * rs[t]],
                        in0=d_slot(t),
                        scalar=1.0,
                        in1=d_slot(t),
                        op0=mybir.AluOpType.mult,
                        op1=mybir.AluOpType.mult,
                        accum_out=acc_v[:, c : c + 1],
                    ).then_inc(sub_sem, 1)

        @block.scalar
        def _(scalar):
            # One small head tile on this queue: it drains before the sync
            # stream builds up, buying the 0.6us the sync engine's boot lags.
            if head_on_side:
                for t in range(1 if gp_head else 0, n_side):
                    issue_load(scalar, t)
            scalar.wait_ge(boot_sem, 1)
            flush0 = act_tiles[-2] if n_act > 1 else None
            for t in act_tiles:
                scalar.wait_ge(sub_sem, sub_done[t])
                c = acol[t]
                scalar.activation(
                    out=sq_scr[:, 0 : 3 * rs[t]],
                    in_=d_slot(t),
                    func=mybir.ActivationFunctionType.Square,
                    accum_out=acc_a[:, c : c + 1],
                ).then_inc(act_sem, 1)
                if t == flush0:
                    scalar.wait_ge(act_sem, n_act - 1)
                    scalar.dma_start(
                        out=out[:, 0 : n_act - 1], in_=acc_a[:, 0 : n_act - 1]
                    ).then_inc(out_sem, 16)
            scalar.wait_ge(act_sem, n_act)
            lo = max(n_act - 2, 0) if flush0 is not None else 0
            scalar.dma_start(
                out=out[:, lo:n_act], in_=acc_a[:, lo:n_act]
            ).then_inc(out_sem, 16)

        @block.gpsimd
        def _(gpsimd):
            if head_on_side and gp_head:
                issue_load(gpsimd, 0)
            # Zero accs so a flush-before-drain race reads zeros, not garbage.
            gpsimd.memset(acc_a, 0.0)
            gpsimd.memset(acc_v, 0.0)
            gpsimd.sem_inc(boot_sem, 1)

    nc.compile()
    nc._v5_order = (act_tiles, dve_tiles)
    return nc


def gram_from_partials_v4(partials: np.ndarray, plan) -> np.ndarray:
    plan_dve, plan_act = plan
    s = partials.astype(np.float64).reshape(-1, partials.shape[-1]).sum(axis=0)
    g = np.zeros((3, 3), dtype=np.float64)
    for c, (i, j) in enumerate(plan_dve + plan_act):
        if i == j:
            g[i, i] += s[c]
        else:
            g[i, j] += s[c]
            g[j, i] += s[c]
    return g


def gram_from_partials_v3(partials: np.ndarray, n_tiles: int, group: int) -> np.ndarray:
    n_groups = n_tiles // group
    ncr = 3 * n_groups
    s = partials.astype(np.float64).reshape(-1, partials.shape[-1]).sum(axis=0)
    cr = s[:ncr].reshape(-1, 3).sum(axis=0)
    sq = s[ncr:].reshape(-1, 3).sum(axis=0)
    g = np.empty((3, 3), dtype=np.float64)
    g[0, 0], g[1, 1], g[2, 2] = sq
    for k, (i, j) in enumerate(_PAIRS):
        g[i, j] = g[j, i] = cr[k]
    return g


_NC_CACHE: dict[tuple, object] = {}


def _get_nc(n_rows: int, n_tiles: int, use_act: bool, raw: bool = False,
            group: int = 4, version: int = 4, n_bufs: int = 12,
            strip_barriers: bool = True, gp_stride: int = 0,
            bulk_r: int = 512, tail_rs: tuple = (256, 128, 64, 32, 32),
            head_rs: tuple = (128, 256, 256, 384), dpad: int = 3,
            head_dma_on_vector: bool = True):
    key = (n_rows, n_tiles, use_act, raw, group, version, n_bufs,
           strip_barriers, gp_stride, bulk_r, tail_rs, head_rs, dpad,
           head_dma_on_vector)
    if key not in _NC_CACHE:
        if version == 5:
            _NC_CACHE[key] = build_diag_kernel_v5(
                n_rows, strip_barriers=strip_barriers)
        elif version == 4:
            _NC_CACHE[key] = build_gram_kernel_v4(
                n_rows, bulk_r=bulk_r, n_bufs=n_bufs, group=group,
                gp_stride=gp_stride, head_rs=head_rs, tail_rs=tail_rs,
                dpad=dpad, head_dma_on_vector=head_dma_on_vector,
                strip_barriers=strip_barriers)
        elif version == 3:
            _NC_CACHE[key] = build_gram_kernel_v3(
                n_rows, n_tiles, n_bufs=n_bufs, group=group,
                strip_barriers=strip_barriers)
        elif raw:
            _NC_CACHE[key] = build_gram_kernel_raw(n_rows, n_tiles, group=group)
        else:
            _NC_CACHE[key] = build_gram_kernel(n_rows, n_tiles, use_act)
    return _NC_CACHE[key]


def gram_from_partials(partials: np.ndarray, n_tiles: int | None = None) -> np.ndarray:
    """[..., 128, 6*slots] partials -> full 3x3 Gram matrix (float64)."""
    slots = partials.shape[-1] // 6
    s = partials.astype(np.float64).reshape(-1, 6 * slots).sum(axis=0)
    sq = s[: 3 * slots].reshape(slots, 3).sum(axis=0)
    cr = s[3 * slots :].reshape(slots, 3).sum(axis=0)
    g = np.empty((3, 3), dtype=np.float64)
    g[0, 0], g[1, 1], g[2, 2] = sq
    for k, (i, j) in enumerate(_PAIRS):
        g[i, j] = g[j, i] = cr[k]
    return g


def run_device_partials(predictions: np.ndarray, targets: np.ndarray,
                        n_tiles: int = 4, use_act: bool = True,
                        raw: bool = False, group: int = 4, version: int = 4,
                        n_bufs: int = 12, strip_barriers: bool = True,
                        gp_stride: int = 0, bulk_r: int = 512,
                        tail_rs: tuple = (256, 128, 64, 32, 32),
                        head_rs: tuple = (128, 256, 256, 384), dpad: int = 3,
                        head_dma_on_vector: bool = True,
                        **run_kwargs):
    """Shard over N_CORES, run on device, return per-core partials + results."""
    b = predictions.shape[0]
    assert b % N_CORES == 0
    n_rows = b // N_CORES
    nc = _get_nc(n_rows, n_tiles, use_act, raw, group, version, n_bufs,
                 strip_barriers, gp_stride, bulk_r, tail_rs, head_rs, dpad,
                 head_dma_on_vector)
    preds = np.ascontiguousarray(predictions, dtype=np.float32).reshape(
        N_CORES, n_rows, 3
    )
    targs = np.ascontiguousarray(targets, dtype=np.float32).reshape(
        N_CORES, n_rows, 3
    )
    in_maps = [
        {"pt": np.stack([preds[c], targs[c]])} for c in range(N_CORES)
    ]
    res = run_bass_kernel_spmd(nc, in_maps, list(range(N_CORES)), **run_kwargs)
    partials = np.stack([r["partials"] for r in res.results])
    return partials, res, nc


def _host_loss(predictions, targets, sigma_inv, logdet, lo=0, hi=None):
    """Exact (float64) loss over rows [lo, hi) on the host, chunked."""
    hi = predictions.shape[0] if hi is None else hi
    tot = 0.0
    for s in range(lo, hi, 1 << 20):
        e = min(s + (1 << 20), hi)
        d = predictions[s:e].astype(np.float64) - targets[s:e].astype(np.float64)
        tot += float(np.einsum("bi,ij,bj->", d, sigma_inv, d))
    return abs(logdet + tot / (hi - lo))


def _sigma_inv_is_scalar(sigma_inv: np.ndarray) -> bool:
    """True iff sigma_inv == c*I to fp64 precision (the shipped input)."""
    d = np.diag(sigma_inv)
    off = sigma_inv - np.diag(d)
    tol = 1e-9 * float(np.abs(d).min())
    return (float(np.abs(off).max()) <= tol
            and float(np.abs(d - d[0]).max()) <= 1e-9 * abs(float(d[0])))


def kernel(predictions: np.ndarray, targets: np.ndarray, sigma: np.ndarray) -> np.ndarray:
    predictions = np.asarray(predictions, dtype=np.float32)
    targets = np.asarray(targets, dtype=np.float32)
    sigma64 = np.asarray(sigma, dtype=np.float64)
    sigma_inv = np.linalg.inv(sigma64)
    _, logdet = np.linalg.slogdet(sigma64)

    # Cheap subsample estimate (~0.3% rel) to sanity-gate the device result.
    est = _host_loss(predictions, targets, sigma_inv, logdet,
                     0, min(1 << 16, predictions.shape[0]))

    use_v5 = _sigma_inv_is_scalar(sigma_inv)
    loss = None
    for _attempt in range(2):
        if use_v5:
            partials, _, _ = run_device_partials(predictions, targets, version=5)
            s = float(partials.astype(np.float64).sum())
            mean_mahal = float(sigma_inv[0, 0]) * s / predictions.shape[0]
        else:
            partials, _, nc = run_device_partials(predictions, targets, version=4)
            g = gram_from_partials_v4(partials, nc._v4_plan)
            mean_mahal = float((sigma_inv * g).sum()) / predictions.shape[0]
        loss = abs(logdet + mean_mahal)
        if np.isfinite(loss) and abs(loss - est) <= 0.05 * max(abs(est), 1e-9):
            return np.float32(loss)
    # Device result failed the sanity gate twice: fall back to exact host.
    return np.float32(_host_loss(predictions, targets, sigma_inv, logdet))

